# revision 1
# baseline (speedup 1.0000x reference)
"""Multi-head attention Trainium2 kernel (8 NeuronCores, tensor-parallel over heads).

Strategy:
  - 16 heads / 8 cores = 2 heads per core. x is replicated; Wq/Wk/Wv sharded by
    head; Wp row-sharded (contraction dim). Each core computes a partial
    projection output [B*T, D]; the host sums the 8 partials (+bias).
  - On chip, all contractions need the contracted dim on SBUF partitions, so the
    host passes xT = x.reshape(BT, D).T and per-core transposed weight slices.
  - qT/kT are computed packed [128 = 2 heads x 64, BT]. Scores are computed
    transposed (s on partitions, t on free) so softmax normalization can ride
    the attn@v matmul: lhsT = [v | ones] gives out rows 0..63 = unnormalized
    out^T and row 64 = the softmax denominator Z. Softmax is computed without
    max subtraction (scores are O(1), exp stays in fp32 range).
  - Causality: only lower-triangular [128s x 512t] blocks are computed; blocks
    straddling the diagonal are column-trimmed (scores/exp/attn@v only touch
    columns >= dd) and masked multiplicatively with a slice of a host-supplied
    shifted-staircase mask (on the otherwise-idle Pool engine).
  - Default cfg "b": every matmul operand is bfloat16 (PE 1 cyc/row at any
    free width, half the DMA/SBUF traffic); PSUM accumulation stays f32 and
    partial outputs are stored bf16 and summed across cores on the host in
    f64. Measured absmax-relative error ~4e-3 (gate 2e-2). cfg "r" is a
    float32r fallback (~2e-4 error, ~11% slower): the BIR verifier requires
    f32r operands to be *produced* as f32r, so every feeding tile is natively
    f32r (inputs bitcast at the DMA, engine copies/exp write f32r directly).
  - In bf16, v is computed directly in [s, e] orientation (lhsT = x tile,
    four width-128 accumulation regions per PSUM bank), skipping the PE
    transpose and its staging copy; f32r/f32 modes keep the transpose path.
  - Engine budget per core (sim): PE ~118us busy of ~153us span; ACT runs the
    exp stream (~94us) and paces the attention inner loop; DVE ~85us does the
    PSUM->SBUF copies + normalize; weights/consts load on the scalar HWDGE
    queue, x-tiles prefetch 2 blocks ahead and outputs batch-store on sync.
"""

import numpy as np

B, T, D, H, HD = 2, 2048, 1024, 16, 64
NCORES = 8
HPC = H // NCORES          # heads per core = 2
CH = HPC * HD              # channels per core = 128
BT = B * T

_CACHE = {}


def _build(b, t, d, cfg):
    """Build + compile the per-core Bass program."""
    import concourse.tile as tile
    from concourse import bacc, mybir
    from contextlib import ExitStack

    f32 = mybir.dt.float32
    f32r = mybir.dt.float32r
    bf16 = mybir.dt.bfloat16

    rmode = cfg == "r"
    bmode = cfg == "b"
    # dtype of every matmul-feeding tile
    MMDT = f32r if rmode else (bf16 if bmode else f32)
    # dtype of the DMA'd inputs (host converts for bf16)
    INDT = bf16 if bmode else f32

    def bcst(ap):
        return ap.bitcast(f32r) if rmode else ap

    bt = b * t
    KT = d // 128            # k-tiles over the model dim
    TBLK = min(512, t)       # t-block width for scores/attn
    NJ = t // TBLK           # t-blocks per batch
    NSB = bt // 128          # 128-row s-blocks over B*T
    SPT = TBLK // 128        # s-blocks per t-block

    nc = bacc.Bacc("TRN2", target_bir_lowering=False, debug=False)

    xT = nc.dram_tensor("xT", [d, bt], INDT, kind="ExternalInput").ap()
    wq = nc.dram_tensor("wq", [d, CH], INDT, kind="ExternalInput").ap()
    wk = nc.dram_tensor("wk", [d, CH], INDT, kind="ExternalInput").ap()
    wv = nc.dram_tensor("wv", [d, CH], INDT, kind="ExternalInput").ap()
    wp = nc.dram_tensor("wp", [CH, d], INDT, kind="ExternalInput").ap()
    cident = nc.dram_tensor("cident", [128, 128], INDT, kind="ExternalInput").ap()
    cmask = nc.dram_tensor("cmask", [128, TBLK + 384], INDT, kind="ExternalInput").ap()
    cones = nc.dram_tensor("cones", [128, NSB, HPC], INDT, kind="ExternalInput").ap()
    cone1 = nc.dram_tensor("cone1", [65, HD], INDT, kind="ExternalInput").ap()
    # partials are summed across cores on the host in f64; bf16 partial
    # stores halve the output DMA traffic for ~1e-3 extra absmax-rel error
    OUTDT = bf16 if bmode else f32
    out_p = nc.dram_tensor("out_p", [bt, d], OUTDT, kind="ExternalOutput").ap()

    with tile.TileContext(nc) as tc, ExitStack() as top:
        persist = top.enter_context(tc.tile_pool(name="persist", bufs=1))

        # ---- persistent tiles ----
        qT_sb = persist.tile([128, bt], MMDT, tag="qT")
        kT_sb = persist.tile([128, bt], MMDT, tag="kT")
        # [v_h0 | 1 | pad | v_h1 | 1 | pad] per 128-row s-block
        vaug = persist.tile([128, NSB, 66 * HPC], MMDT, tag="vaug")
        outT_sb = persist.tile([128, bt], MMDT, tag="outT")
        wq_sb = persist.tile([128, KT, CH], MMDT, tag="wq")
        wk_sb = persist.tile([128, KT, CH], MMDT, tag="wk")
        wv_sb = persist.tile([128, KT, CH], MMDT, tag="wv")
        wp_sb = persist.tile([128, d], MMDT, tag="wp")
        ident = persist.tile([128, 128], MMDT, tag="ident")
        # staircase mask, shifted: maskt[p, m] = 1 iff m >= p + 384
        maskt = persist.tile([128, TBLK + 384], MMDT, tag="mask")
        one1 = persist.tile([65, HD], MMDT, tag="one1")

        # startup DMAs on the scalar HWDGE queue (fast descriptor gen; the
        # Pool SWDGE takes ~1.1us per DMA), ordered by first use: ident
        # (act-table warm + block-0 transposes), big wq/wk/wv loads, then
        # attention consts; wp (needed only by the lagging proj) last.
        # wq in two halves so the first q matmuls start ~1.4us earlier
        for w_ap, w_sb, nsplit in ((wq, wq_sb, 2), (wk, wk_sb, 1), (wv, wv_sb, 1)):
            for s in range(nsplit):
                hk = slice(s * KT // nsplit, (s + 1) * KT // nsplit)
                nc.scalar.dma_start(
                    out=w_sb[:, hk, :],
                    in_=bcst(w_ap.rearrange("(kt p) m -> p kt m", p=128)[:, hk, :]),
                )
        # preload the Exp activation table under the startup DMAs
        actwarm = persist.tile([1, 8], f32, tag="actwarm")
        nc.scalar.activation(actwarm[:], wq_sb[0:1, 0, 0:8],
                             mybir.ActivationFunctionType.Exp, scale=0.125)
        # ident only feeds the PE-transpose path (non-bf16 modes), but the
        # load stays unconditional: dropping it shifts the startup DMA
        # phasing and measures 2.6us WORSE in bmode (scheduler alignment)
        nc.scalar.dma_start(out=ident[:], in_=bcst(cident))
        nc.scalar.dma_start(out=maskt[:], in_=bcst(cmask))
        nc.scalar.dma_start(out=one1[:], in_=bcst(cone1))
        for h in range(HPC):
            nc.scalar.dma_start(
                out=vaug[:, :, 66 * h + 64:66 * h + 65],
                in_=bcst(cones[:, :, h:h + 1]),
            )
        nc.scalar.dma_start(out=wp_sb[:], in_=bcst(wp))

        # ---- merged loop: per (batch, t-block): QKV -> attention -> proj ----
        # Attention for block j of batch bb needs q columns of block j and
        # k/v columns of blocks 0..j (same batch) -- all computed by the time
        # block j's QKV is done, so one fused loop pipelines everything:
        # xT loads prefetch under attention PE work, and output stores drain
        # under the next block's compute.
        PW = min(512, d)
        NIB = d // PW
        with ExitStack() as body:
            xpool = body.enter_context(tc.tile_pool(name="xpool", bufs=4 if bmode else 3))
            vtpool = body.enter_context(tc.tile_pool(name="vtpool", bufs=2))
            npool = body.enter_context(tc.tile_pool(name="npool", bufs=24 if bmode else 5))
            zpool = body.enter_context(tc.tile_pool(name="zpool", bufs=2))
            tmpool = body.enter_context(tc.tile_pool(name="tmpool", bufs=2))
            opool = body.enter_context(tc.tile_pool(name="opool", bufs=2))
            # PSUM budget (8 banks): qkv 2 + scores 2 + av 2 + tr/proj/bc 2
            ps_qkv = body.enter_context(tc.tile_pool(name="ps_qkv", bufs=2, space="PSUM"))
            ps_s = body.enter_context(tc.tile_pool(name="ps_s", bufs=2, space="PSUM"))
            ps_av = body.enter_context(tc.tile_pool(name="ps_av", bufs=2, space="PSUM"))
            ps_tp = body.enter_context(tc.tile_pool(name="ps_tp", bufs=2, space="PSUM"))

            xt_tiles = {}

            def emit_xt(bb, j, chunked=False):
                col0 = bb * t + j * TBLK
                tsl = slice(col0, col0 + TBLK)
                xt = xpool.tile([128, KT, TBLK], MMDT, tag="xt", name=f"xt_{bb}_{j}")
                if chunked:
                    for kt in range(KT):
                        nc.sync.dma_start(
                            out=xt[:, kt, :],
                            in_=bcst(xT[kt * 128:(kt + 1) * 128, tsl]),
                        )
                else:
                    nc.sync.dma_start(
                        out=xt[:],
                        in_=bcst(xT.rearrange("(kt p) c -> p kt c", p=128)[:, :, tsl]),
                    )
                xt_tiles[(bb, j)] = xt

            def emit_qkv(bb, j):
                col0 = bb * t + j * TBLK
                tsl = slice(col0, col0 + TBLK)
                xt = xt_tiles.pop((bb, j))
                for w_sb, dst in ((wq_sb, qT_sb), (wk_sb, kT_sb)):
                    ps = ps_qkv.tile([128, TBLK], f32, tag="ps_qkv",
                                     name=f"psq_{bb}_{j}_{dst.name}")
                    for kt in range(KT):
                        nc.tensor.matmul(ps[:], w_sb[:, kt, :], xt[:, kt, :],
                                         start=(kt == 0), stop=(kt == KT - 1))
                    nc.vector.tensor_copy(dst[:, tsl], ps[:])
                if bmode:
                    # bf16 runs 1 cyc/row at width 128: compute v directly in
                    # [s, e] orientation (lhsT = x tile), skipping the PE
                    # transpose and the vt staging copy entirely
                    ps = ps_qkv.tile([128, SPT, 128], f32, tag="ps_qkv",
                                     name=f"psv_{bb}_{j}")
                    for s4 in range(SPT):
                        for kt in range(KT):
                            nc.tensor.matmul(
                                ps[:, s4, :],
                                xt[:, kt, s4 * 128:(s4 + 1) * 128],
                                wv_sb[:, kt, :],
                                start=(kt == 0), stop=(kt == KT - 1),
                                skip_group_check=True)
                    for s4 in range(SPT):
                        sb_idx = (col0 // 128) + s4
                        nc.vector.tensor_copy(
                            vaug[:, sb_idx, :].rearrange(
                                "p (g c) -> p g c", g=HPC)[:, :, 0:HD],
                            ps[:, s4, :].rearrange("p (g c) -> p g c", g=HPC),
                        )
                else:
                    ps = ps_qkv.tile([128, TBLK], f32, tag="ps_qkv", name=f"psv_{bb}_{j}")
                    for kt in range(KT):
                        nc.tensor.matmul(ps[:], wv_sb[:, kt, :], xt[:, kt, :],
                                         start=(kt == 0), stop=(kt == KT - 1))
                    vt = vtpool.tile([128, TBLK], MMDT, tag="vt", name=f"vt_{bb}_{j}")
                    nc.vector.tensor_copy(vt[:], ps[:])
                    for s4 in range(SPT):
                        sb_idx = (col0 // 128) + s4
                        pt = ps_tp.tile([128, 128], MMDT, tag="ps_tp",
                                        name=f"ptr_{bb}_{j}_{s4}")
                        nc.tensor.transpose(pt[:], vt[:, s4 * 128:(s4 + 1) * 128],
                                            ident[:])
                        nc.vector.tensor_copy(
                            vaug[:, sb_idx, :].rearrange(
                                "p (g c) -> p g c", g=HPC)[:, :, 0:HD],
                            pt[:].rearrange("p (g c) -> p g c", g=HPC),
                        )

            def emit_attn(bb, j, last=False):
                col0 = bb * t + j * TBLK
                tsl = slice(col0, col0 + TBLK)
                n_i = (j + 1) * SPT
                avs = [ps_av.tile([65, TBLK], f32, tag="ps_av", name=f"av_{bb}_{j}_{h}")
                       for h in range(HPC)]

                def emit_av(i_, ddp_, nh_):
                    for h in range(HPC):
                        sb_idx = (bb * t + i_ * 128) // 128
                        nc.tensor.matmul(
                            avs[h][:, ddp_:], vaug[:, sb_idx, h * 66:h * 66 + HD + 1],
                            nh_[:, h * TBLK + ddp_:(h + 1) * TBLK],
                            start=(i_ == 0), stop=(i_ == n_i - 1),
                            skip_group_check=True)

                from collections import deque
                pend = deque()
                for i in range(n_i):
                    ssl = slice(bb * t + i * 128, bb * t + i * 128 + 128)
                    dd = 128 * i - TBLK * j
                    # column trim: scores/exp/av touch only cols >= ddp
                    # (f32r needs free dim >= 256 for the PE fast path;
                    # bf16 runs 1 cyc/row at any width so trim fully)
                    if rmode:
                        ddp = max(0, min(dd, TBLK - 256))
                    else:
                        ddp = max(0, dd)
                    nh = npool.tile([128, HPC * TBLK], MMDT, tag="nh",
                                    name=f"nh_{bb}_{j}_{i}")
                    for h in range(HPC):
                        hp = slice(h * HD, (h + 1) * HD)
                        ps = ps_s.tile([128, TBLK], f32, tag="ps_s",
                                       name=f"pss_{bb}_{j}_{i}_{h}")
                        nc.tensor.matmul(
                            ps[:, ddp:], kT_sb[hp, ssl],
                            qT_sb[hp, tsl][:, ddp:], start=True, stop=True)
                        nc.scalar.activation(
                            nh[:, h * TBLK + ddp:(h + 1) * TBLK], ps[:, ddp:],
                            mybir.ActivationFunctionType.Exp, scale=0.125)
                        if dd >= 0:
                            # mask cols [ddp, dd+128): staircase + trim slack
                            # (nh col c maps to mask col c + 384 - dd)
                            nc.gpsimd.tensor_mul(
                                nh[:, h * TBLK + ddp:h * TBLK + dd + 128],
                                nh[:, h * TBLK + ddp:h * TBLK + dd + 128],
                                maskt[:, 384 - dd + ddp:512])
                    # attn@v lags 12 i-steps (== fully deferred for most
                    # blocks): the scores/exp stream runs uninterrupted, then
                    # the AV batch runs at full PE rate against banked nh
                    # tiles -- measured best across lag 1..16
                    pend.append((i, ddp, nh))
                    if len(pend) > 12:
                        i_, ddp_, nh_ = pend.popleft()
                        emit_av(i_, ddp_, nh_)
                while pend:
                    i_, ddp_, nh_ = pend.popleft()
                    emit_av(i_, ddp_, nh_)

                # --- softmax normalization: out = av * (1/Z) ---
                rr = zpool.tile([65, HPC * TBLK], MMDT, tag="rr",
                                name=f"rrr_{bb}_{j}")
                with nc.allow_low_precision(reason="f32r PE broadcast of 1/Z"):
                    for h in range(HPC):
                        nc.vector.reciprocal(
                            rr[64:65, h * TBLK:(h + 1) * TBLK], avs[h][64:65, :])
                # h1 first: its outT write goes through a SBUF->SBUF DMA
                # (partition shift), so start it before h0's direct DVE write
                bcs_h = {}
                for h in reversed(range(HPC)):
                    # K=1 matmul broadcasts 1/Z across the 64 output partitions
                    bc = ps_tp.tile([HD, TBLK], f32, tag="ps_tp", name=f"bc_{bb}_{j}_{h}")
                    nc.tensor.matmul(bc[:], one1[64:65, :],
                                     rr[64:65, h * TBLK:(h + 1) * TBLK],
                                     start=True, stop=True)
                    # DVE may read only one PSUM operand: stage bc in SBUF
                    # (on DVE -- ACT is loaded with the exp stream)
                    bcs = tmpool.tile([HD, TBLK], f32, tag="bcs", name=f"bcs_{bb}_{j}_{h}")
                    nc.vector.tensor_copy(bcs[:], bc[:])
                    bcs_h[h] = bcs
                    if h == 0:
                        nc.vector.tensor_mul(outT_sb[0:HD, tsl], avs[h][0:HD, :], bcs[:])
                    else:
                        tmp = tmpool.tile([HD, TBLK], MMDT, tag="tmp", name=f"tm_{bb}_{j}")
                        nc.vector.tensor_mul(tmp[:], avs[h][0:HD, :], bcs[:])
                        nc.sync.dma_start(
                            out=outT_sb[h * HD:(h + 1) * HD, tsl], in_=tmp[:])

            def emit_proj(bb, j, last=False):
                col0 = bb * t + j * TBLK
                NTL = TBLK // 128
                ot = opool.tile([128, NTL, d], OUTDT, tag="ot", name=f"ot_{bb}_{j}")
                for tl in range(NTL):
                    tt = col0 // 128 + tl
                    for ib in range(NIB):
                        # drain only: scores pool is idle, alternate pools for
                        # a 4-deep ring so the matmul stream is not copy-paced
                        pools = ((ps_tp, "ps_tp"), (ps_s, "ps_s"),
                                 (ps_qkv, "ps_qkv"))
                        pp, ptag = pools[(tl * NIB + ib) % 3] if last else pools[0]
                        ps = pp.tile([128, PW], f32, tag=ptag,
                                     name=f"psp_{bb}_{j}_{tl}_{ib}")
                        nc.tensor.matmul(ps[:], outT_sb[:, tt * 128:(tt + 1) * 128],
                                         wp_sb[:, ib * PW:(ib + 1) * PW],
                                         start=True, stop=True)
                        # in the drain there is no exp stream: split copies
                        # between DVE and ACT and store per row-block pair so
                        # the store overlaps the remaining copies
                        if last and (tl * NIB + ib) % 2 == 1:
                            nc.scalar.copy(ot[:, tl, ib * PW:(ib + 1) * PW], ps[:])
                        else:
                            nc.vector.tensor_copy(
                                ot[:, tl, ib * PW:(ib + 1) * PW], ps[:])
                    if last:
                        nc.sync.dma_start(
                            out=out_p.rearrange("(tb p) c -> p tb c", p=128)[
                                :, col0 // 128 + tl:col0 // 128 + tl + 1, :],
                            in_=ot[:, tl:tl + 1, :])
                if not last:
                    # one store DMA per block: [p, tl, d] -> row-blocks of out_p
                    nc.sync.dma_start(
                        out=out_p.rearrange("(tb p) c -> p tb c", p=128)[
                            :, col0 // 128:col0 // 128 + NTL, :],
                        in_=ot[:])

            # software pipeline: QKV runs one t-block ahead of attention, and
            # the projection lags one block behind, so block-boundary DVE/DMA
            # latencies hide under attention PE work
            blocks = [(bb, j) for bb in range(b) for j in range(NJ)]
            emit_xt(*blocks[0], chunked=True)
            emit_xt(*blocks[1])
            emit_qkv(*blocks[0])
            deferred = {2, 4, 6} if len(blocks) == 8 else set()
            for idx, blk in enumerate(blocks):
                if idx + 2 < len(blocks):
                    emit_xt(*blocks[idx + 2])
                if idx + 1 < len(blocks):
                    emit_qkv(*blocks[idx + 1])
                if idx == len(blocks) - 1:
                    for dfx in sorted(deferred):
                        emit_proj(*blocks[dfx - 1])
                emit_attn(*blk, last=(idx == len(blocks) - 1))
                if idx >= 1 and idx not in deferred:
                    emit_proj(*blocks[idx - 1], last=(idx == len(blocks) - 1))
            emit_proj(*blocks[-1], last=True)

    nc.compile()
    return nc


def _get_nc(b=B, t=T, d=D, cfg="b"):
    key = (b, t, d, cfg)
    if key not in _CACHE:
        _CACHE[key] = _build(b, t, d, cfg)
    return _CACHE[key]


def _in_dtype(cfg):
    if cfg == "b":
        import ml_dtypes
        return np.dtype(ml_dtypes.bfloat16)
    return np.dtype(np.float32)


def _make_consts(b, t, d, dt):
    bt = b * t
    TBLK = min(512, t)
    NSB = bt // 128
    cident = np.eye(128, dtype=dt)
    p = np.arange(128, dtype=np.int64)[:, None]
    m = np.arange(TBLK + 384, dtype=np.int64)[None, :]
    cmask = (m >= p + 384).astype(dt)
    cones = np.ones((128, NSB, HPC), dtype=dt)
    cone1 = np.ones((65, HD), dtype=dt)
    return {"cident": cident, "cmask": cmask, "cones": cones, "cone1": cone1}


def _prepare_in_maps(x, Wq, Wk, Wv, Wp, b, t, d, cfg):
    bt = b * t
    dt = _in_dtype(cfg)
    xT = np.ascontiguousarray(x.reshape(bt, d).T.astype(dt))
    consts = _make_consts(b, t, d, dt)
    in_maps = []
    for c in range(NCORES):
        h0 = c * HPC
        wq_c = np.ascontiguousarray(Wq[h0:h0 + HPC].reshape(CH, d).T.astype(dt))
        wk_c = np.ascontiguousarray(Wk[h0:h0 + HPC].reshape(CH, d).T.astype(dt))
        wv_c = np.ascontiguousarray(Wv[h0:h0 + HPC].reshape(CH, d).T.astype(dt))
        wp_c = np.ascontiguousarray(Wp[:, c * CH:(c + 1) * CH].T.astype(dt))
        in_maps.append({"xT": xT, "wq": wq_c, "wk": wk_c, "wv": wv_c, "wp": wp_c,
                        **consts})
    return in_maps


def _run(x, Wq, Wk, Wv, Wp, bp, b, t, d, cfg, trace=False):
    from concourse.bass_utils import run_bass_kernel_spmd
    nc = _get_nc(b, t, d, cfg)
    in_maps = _prepare_in_maps(x, Wq, Wk, Wv, Wp, b, t, d, cfg)
    res = run_bass_kernel_spmd(nc, in_maps, core_ids=list(range(NCORES)), trace=trace)
    acc = np.zeros((b * t, d), dtype=np.float64)
    for r in res.results:
        acc += r["out_p"].astype(np.float64)
    out = (acc + np.asarray(bp, dtype=np.float64)).astype(np.float32)
    return out.reshape(b, t, d), res


KERNEL_CFG = "b"


def kernel(x, Wq, Wk, Wv, Wp, bp):
    out, _ = _run(np.asarray(x), np.asarray(Wq), np.asarray(Wk), np.asarray(Wv),
                  np.asarray(Wp), np.asarray(bp), B, T, D, KERNEL_CFG, trace=False)
    return out



# revision 72
# speedup vs baseline: 1.0925x; 1.0925x over previous
"""Multi-head attention Trainium2 kernel (8 NeuronCores, tensor-parallel over heads).

Sharding: 16 heads / 8 cores = 2 heads per core. x is replicated; Wq/Wk/Wv
sharded by head; Wp row-sharded (contraction dim). Each core computes a
partial projection output [B*T, D] stored bf16; the host sums the 8 partials
in f64 (+bias).

Default cfg "d" (fp8 DoubleRow; ~135us sim vs 147us for the bf16 cfg "b",
absmax-relative error ~3.6e-3 vs gate 2e-2):
  - QKV: x and W are split hi+lo fp8e4m3 on the host; W is pre-scaled x32 so
    both hi and the residual stay in fp8's NORMAL range (raw W~0.02 values
    are subnormal in e4m3, which destroys the residual trick). Contraction
    runs as 3 DoubleRow cross terms (Wh*xh + Wl*xh + Wh*xl), each packing
    K=256 at 0.5 cyc/row: 12 matmuls of N/2 vs bf16's 8 of N. The x32 is
    compensated for free: scores come out x1024 and the exp scale constant
    becomes 0.125/1024; v comes out x32 and the broadcast-ones constant used
    to expand 1/Z is 1/32.
  - Scores stay bf16 [s on partitions, t free]; both heads' scores land in
    one 2-bank PSUM tile so exp is ONE activation per s-block (halves the
    ACT per-op overhead; the exp stream is the attention-phase critical
    path). exp writes fp8 nh directly.
  - attn@v runs DoubleRow over PAIRS of s-blocks (contraction 256, cost
    width/2 per pair, 4x denser than bf16): lhsT = packed [v32|1] planes,
    rhs = nh pair planes; out rows 0..63 = unnormalized out^T x32, row 64 =
    Z. Plane-1 columns of diagonal pairs that sit above that plane's
    diagonal are memset on Pool. j=0 blocks (rows t<512, few softmax terms,
    largest weights -> fp8 noise would dominate absmax) use a bf16 attn@v
    path instead (vaug_b/masktb).
  - All vaug images are FULLY preloaded from host constants: reading
    uninitialized SBUF as fp8 can yield NaN (0xFF) on the first call.
  - Causality: lower-triangular [128s x 512t] blocks only; diagonal
    straddlers are column-trimmed and masked multiplicatively on Pool.
    Softmax runs without max subtraction (logits are O(1)).
  - Scheduling: the attention i-loop is ACT(exp)-paced, so qkv(j+1),
    proj(j-1) and x-tile prefetches are emitted as weighted "filler" pieces
    spread through it (the PE runs strictly in-order; contiguous chunks
    would starve either PE or ACT). PSUM: scores 2x2 banks + av 2 + shared
    qkv/proj/bc ring 2. The last block's h1 outT shift goes through a PE
    shift-matrix matmul instead of the ~1.9us SBUF->SBUF DMA.
cfg "b": all-bf16 fallback (the previous baseline), cfg "r": float32r.
"""

import numpy as np

B, T, D, H, HD = 2, 2048, 1024, 16, 64
NCORES = 8
HPC = H // NCORES          # heads per core = 2
CH = HPC * HD              # channels per core = 128
BT = B * T

_CACHE = {}


def _build_d(b, t, d):
    """cfg "d": fp8 DoubleRow build.

    - QKV: x and Wq/Wk/Wv are split hi/lo fp8e4m3 on the host (x = xh + xl
      exactly captures x to ~0.2%); contraction runs as 3 DoubleRow cross
      terms (Wh*xh + Wh*xl + Wl*xh) of K=256 each -> 12 matmuls of
      cost N/2 instead of 8 of cost N (sim model: fp8 DoubleRow is 0.5
      cycles/row with 2 K-planes packed per matmul).
    - Scores stay bf16 [s, t]; exp is ONE activation per s-block covering
      both heads (scores psum is a 2-bank [128, 2, TBLK] tile).
    - exp output is fp8 directly; attn@v runs DoubleRow over PAIRS of
      s-blocks (contraction 256): cost width/2 per pair instead of
      2*width. vaug holds fp8 v planes [128, NSB/2, 2, 160] with the
      softmax-denominator ones column at 64/144 per head.
    - Diagonal pairs: plane-1 columns below its own diagonal trim but
      inside the pair's matmul slice are memset to 0 on Pool.
    - Normalization/proj unchanged from cfg "b" (bf16).
    """
    import concourse.tile as tile
    from concourse import bacc, mybir
    from contextlib import ExitStack
    from collections import deque

    f32 = mybir.dt.float32
    bf16 = mybir.dt.bfloat16
    f8 = mybir.dt.float8e4
    DR = mybir.MatmulPerfMode.DoubleRow

    bt = b * t
    KT = d // 128
    KP = KT // 2             # DoubleRow k-pair steps
    TBLK = min(512, t)
    NJ = t // TBLK
    NSB = bt // 128
    SPT = TBLK // 128

    nc = bacc.Bacc("TRN2", target_bir_lowering=False, debug=False)

    xTh = nc.dram_tensor("xTh", [d, bt], f8, kind="ExternalInput").ap()
    xTl = nc.dram_tensor("xTl", [d, bt], f8, kind="ExternalInput").ap()
    w_in = {}
    for nm in ("wqh", "wql", "wkh", "wkl", "wvh", "wvl"):
        w_in[nm] = nc.dram_tensor(nm, [d, CH], f8, kind="ExternalInput").ap()
    wp = nc.dram_tensor("wp", [CH, d], bf16, kind="ExternalInput").ap()
    cmask = nc.dram_tensor("cmask", [128, TBLK + 384], f8, kind="ExternalInput").ap()
    cmaskb = nc.dram_tensor("cmaskb", [128, TBLK + 384], bf16,
                            kind="ExternalInput").ap()
    # FULL vaug images (ones columns + zero padding): loading the whole tile
    # avoids any read of uninitialized SBUF (fp8 garbage can be NaN) on the
    # first call
    cones = nc.dram_tensor("cones", [128, NSB // 2, 2, 160], f8,
                           kind="ExternalInput").ap()
    conesb = nc.dram_tensor("conesb", [128, b, SPT, 132], bf16,
                            kind="ExternalInput").ap()
    cone1 = nc.dram_tensor("cone1", [65, HD], bf16, kind="ExternalInput").ap()
    cshf = nc.dram_tensor("cshf", [HD, 128], bf16, kind="ExternalInput").ap()
    out_p = nc.dram_tensor("out_p", [bt, d], bf16, kind="ExternalOutput").ap()

    with tile.TileContext(nc) as tc, ExitStack() as top:
        persist = top.enter_context(tc.tile_pool(name="persist", bufs=1))

        qT_sb = persist.tile([128, bt], bf16, tag="qT")
        kT_sb = persist.tile([128, bt], bf16, tag="kT")
        # v planes: per (pair, plane): [v_h0 | 1 | pad @80 | v_h1 | 1 | pad]
        vaug = persist.tile([128, NSB // 2, 2, 160], f8, tag="vaug")
        # bf16 v for the first 4 s-blocks of each batch: the j=0 attention
        # blocks (rows t<512, where softmax weights are largest and fp8
        # noise dominates the absmax error) run a bf16 attn@v path
        vaug_b = persist.tile([128, b, SPT, 132], bf16, tag="vaug_b")
        outT_sb = persist.tile([128, bt], bf16, tag="outT")
        w_sb = {}
        for nm in ("wqh", "wql", "wkh", "wkl", "wvh", "wvl"):
            w_sb[nm] = persist.tile([128, KP, 2, CH], f8, tag=nm, name=nm)
        wp_sb = persist.tile([128, d], bf16, tag="wp")
        maskt = persist.tile([128, TBLK + 384], f8, tag="mask")
        masktb = persist.tile([128, TBLK + 384], bf16, tag="maskb")
        one1 = persist.tile([65, HD], bf16, tag="one1")
        # shift matrix: shf64[p, 64+p] = 1 moves rows 0..63 -> 64..127 via PE
        shf64 = persist.tile([HD, 128], bf16, tag="shf64")

        # startup: only the first-needed wq halves ride the shared HWDGE
        # (which the x-tile stream also needs); everything else goes through
        # the Pool SWDGE queue so it doesn't stall the critical path
        def wload(nm, eng, nsplit=1):
            for s in range(nsplit):
                hk = slice(s * KP // nsplit, (s + 1) * KP // nsplit)
                eng.dma_start(
                    out=w_sb[nm][:, hk, :, :],
                    in_=w_in[nm].rearrange(
                        "(kp pl p) m -> p kp pl m", p=128, pl=2)[:, hk, :, :],
                )
        wload("wqh", nc.scalar, 1)
        # warm the Exp act table under the startup DMAs
        actwarm = persist.tile([1, 8], f32, tag="actwarm")
        nc.scalar.activation(actwarm[:], w_sb["wqh"][0:1, 0, 0, 0:8],
                             mybir.ActivationFunctionType.Exp, scale=0.125)
        wload("wkh", nc.scalar)
        wload("wql", nc.scalar)
        wload("wkl", nc.scalar)
        wload("wvh", nc.scalar)
        wload("wvl", nc.scalar)
        # the full vaug images must land before the first v copies (~7us);
        # masktb before the first attention block's masks (~10us)
        nc.scalar.dma_start(out=vaug[:], in_=cones)
        nc.scalar.dma_start(out=vaug_b[:], in_=conesb)
        nc.scalar.dma_start(out=masktb[:], in_=cmaskb)
        nc.scalar.dma_start(out=maskt[:], in_=cmask)
        nc.scalar.dma_start(out=one1[:], in_=cone1)
        nc.scalar.dma_start(out=shf64[:], in_=cshf)
        nc.scalar.dma_start(out=wp_sb[:], in_=wp)

        with ExitStack() as body:
            xpool = body.enter_context(tc.tile_pool(name="xpool", bufs=4))
            npool = body.enter_context(tc.tile_pool(name="npool", bufs=9))
            npool_b = body.enter_context(tc.tile_pool(name="npool_b", bufs=3))
            zpool = body.enter_context(tc.tile_pool(name="zpool", bufs=2))
            tmpool = body.enter_context(tc.tile_pool(name="tmpool", bufs=2))
            opool = body.enter_context(tc.tile_pool(name="opool", bufs=2))
            # PSUM (8 banks): scores 2x2 + av 2 + shared(qkv/proj/bc) 2
            ps_s = body.enter_context(tc.tile_pool(name="ps_s", bufs=2, space="PSUM"))
            ps_av = body.enter_context(tc.tile_pool(name="ps_av", bufs=2, space="PSUM"))
            ps_sh = body.enter_context(tc.tile_pool(name="ps_sh", bufs=2, space="PSUM"))

            xt_tiles = {}

            def emit_xt(bb, j, chunked=False):
                col0 = bb * t + j * TBLK
                tsl = slice(col0, col0 + TBLK)
                srcs = (("xh", xTh), ("xl", xTl))
                pair = [xpool.tile([128, KT, TBLK], f8, tag=nm,
                                   name=f"{nm}_{bb}_{j}")
                        for nm, _ in srcs]
                if chunked:
                    # half-tile pieces, hi/lo interleaved, so the first matmul
                    # group's inputs land as early as possible
                    for kh in range(2):
                        for xt, (nm, src) in zip(pair, srcs):
                            nc.sync.dma_start(
                                out=xt[:, 4 * kh:4 * kh + 4, :],
                                in_=src.rearrange(
                                    "(kt p) c -> p kt c",
                                    p=128)[:, 4 * kh:4 * kh + 4, tsl])
                else:
                    for xt, (nm, src) in zip(pair, srcs):
                        nc.sync.dma_start(
                            out=xt[:],
                            in_=src.rearrange("(kt p) c -> p kt c", p=128)[:, :, tsl])
                xt_tiles[(bb, j)] = pair

            def qkv_units(bb, j):
                """One block's QKV as weighted filler pieces (weight ~= us of
                PE work, used to spread emission across the attention i-loop
                without starving the ACT exp stream)."""
                col0 = bb * t + j * TBLK
                tsl = slice(col0, col0 + TBLK)
                state = {}
                # x-residual term order: hi*hi, lo_w*hi, hi*lo_x -- the lo x
                # tile is only needed by the last 4 matmuls of each group
                QTERMS = (("h", "xh"), ("l", "xh"), ("h", "xl"))

                def qk_mm(wt, dst, ti):
                    xts = dict(zip(("xh", "xl"), xt_tiles[(bb, j)]))
                    key = "ps" + wt
                    if ti == 0:
                        state[key] = ps_sh.tile([128, TBLK], f32, tag="ps_sh",
                                                name=f"psq_{bb}_{j}_{dst.name}")
                    ps = state[key]
                    ws, xn = QTERMS[ti]
                    for kp in range(KP):
                        nc.tensor.matmul(
                            ps[:], w_sb[wt + ws][:, kp, :, :],
                            xts[xn][:, 2 * kp:2 * kp + 2, :],
                            start=(ti == 0 and kp == 0),
                            stop=(ti == 2 and kp == KP - 1),
                            perf_mode=DR)
                    if ti == 2:
                        nc.vector.tensor_copy(dst[:, tsl], ps[:])

                def v_mm(s4):
                    xth, xtl = xt_tiles[(bb, j)]
                    if "vps" not in state:
                        state["vps"] = ps_sh.tile([128, SPT, 128], f32,
                                                  tag="ps_sh",
                                                  name=f"psv_{bb}_{j}")
                    ps = state["vps"]
                    idx = 0
                    for xh, wn in ((xth, "wvh"), (xth, "wvl"), (xtl, "wvh")):
                        for kp in range(KP):
                            nc.tensor.matmul(
                                ps[:, s4, :],
                                xh[:, 2 * kp:2 * kp + 2,
                                   s4 * 128:(s4 + 1) * 128],
                                w_sb[wn][:, kp, :, :],
                                start=(idx == 0), stop=(idx == 3 * KP - 1),
                                perf_mode=DR, skip_group_check=True)
                            idx += 1
                    sb_idx = (col0 // 128) + s4
                    m, pl = divmod(sb_idx, 2)
                    nc.vector.tensor_copy(
                        vaug[:, m, pl, :].rearrange(
                            "p (g c) -> p g c", g=2)[:, :, 0:HD],
                        ps[:, s4, :].rearrange("p (g c) -> p g c", g=HPC),
                    )
                    if j == 0:
                        # bf16 copy for the j=0 attention path
                        nc.vector.tensor_copy(
                            vaug_b[:, bb, s4, :].rearrange(
                                "p (g c) -> p g c", g=2)[:, :, 0:HD],
                            ps[:, s4, :].rearrange("p (g c) -> p g c", g=HPC),
                        )
                    if s4 == SPT - 1:
                        xt_tiles.pop((bb, j))

                units = []
                for wt, dst in (("wq", qT_sb), ("wk", kT_sb)):
                    for ti in range(3):
                        units.append((0.45, (lambda wt_=wt, dst_=dst, ti_=ti:
                                             qk_mm(wt_, dst_, ti_))))
                for s4 in range(SPT):
                    units.append((0.9, (lambda s4_=s4: v_mm(s4_))))
                return units

            def emit_qkv(bb, j):
                for _, u in qkv_units(bb, j):
                    u()

            def emit_attn(bb, j, fillers=(), last=False):
                col0 = bb * t + j * TBLK
                tsl = slice(col0, col0 + TBLK)
                n_i = (j + 1) * SPT
                n_pairs = n_i // 2
                avs = [ps_av.tile([65, TBLK], f32, tag="ps_av",
                                  name=f"av_{bb}_{j}_{h}")
                       for h in range(HPC)]

                def emit_av(m_, ddp_, nh_):
                    sbp = bb * (t // 256) + m_
                    for h in range(HPC):
                        nc.tensor.matmul(
                            avs[h][:, ddp_:],
                            vaug[:, sbp, :, 80 * h:80 * h + HD + 1],
                            nh_[:, :, h, ddp_:],
                            start=(m_ == 0), stop=(m_ == n_pairs - 1),
                            perf_mode=DR, skip_group_check=True)

                def emit_av_b(i_, ddp_, nh_, pl_):
                    # bf16 path (j=0 blocks): per-s-block standard matmul
                    for h in range(HPC):
                        nc.tensor.matmul(
                            avs[h][:, ddp_:],
                            vaug_b[:, bb, i_, 66 * h:66 * h + HD + 1],
                            nh_[:, pl_, h, ddp_:],
                            start=(i_ == 0), stop=(i_ == n_i - 1),
                            skip_group_check=True)

                fillers = list(fillers)
                wtotal = sum(w for w, _ in fillers) or 1.0
                nfill = 0
                wdone = 0.0

                pend = deque()
                nh_m = None
                pair_ddp = 0
                for i in range(n_i):
                    # spread deferred qkv/proj/xt work through the i-loop by
                    # cumulative PE-work weight, so the PE has in-order work
                    # during exp waits without long bursts that starve ACT
                    want = (i / n_i) * wtotal
                    while nfill < len(fillers) and wdone < want:
                        w, fn = fillers[nfill]
                        fn()
                        wdone += w
                        nfill += 1
                    ssl = slice(bb * t + i * 128, bb * t + i * 128 + 128)
                    dd = 128 * i - TBLK * j
                    ddp = max(0, dd)
                    m, pl = divmod(i, 2)
                    nhdt = bf16 if j == 0 else f8
                    nhmask = masktb if j == 0 else maskt
                    if pl == 0:
                        np_ = npool_b if j == 0 else npool
                        nh_m = np_.tile([128, 2, HPC, TBLK], nhdt,
                                        tag="nhb" if j == 0 else "nh",
                                        name=f"nh_{bb}_{j}_{m}")
                        pair_ddp = ddp
                    ps = ps_s.tile([128, HPC, TBLK], f32, tag="ps_s",
                                   name=f"pss_{bb}_{j}_{i}")
                    for h in range(HPC):
                        hp = slice(h * HD, (h + 1) * HD)
                        nc.tensor.matmul(
                            ps[:, h, ddp:], kT_sb[hp, ssl],
                            qT_sb[hp, tsl][:, ddp:], start=True, stop=True)
                    nc.scalar.activation(
                        nh_m[:, pl, :, ddp:], ps[:, :, ddp:],
                        mybir.ActivationFunctionType.Exp, scale=0.125)
                    if dd >= 0:
                        for h in range(HPC):
                            nc.gpsimd.tensor_mul(
                                nh_m[:, pl, h, ddp:dd + 128],
                                nh_m[:, pl, h, ddp:dd + 128],
                                nhmask[:, 384 - dd + ddp:512])
                    if j == 0:
                        pend.append((i, ddp, nh_m, pl))
                        if len(pend) > 2:
                            emit_av_b(*pend.popleft())
                    elif pl == 1:
                        if ddp > pair_ddp:
                            # plane-1 cols [pair_ddp, ddp) are inside the AV
                            # slice but above this plane's diagonal: zero them
                            nc.gpsimd.memset(nh_m[:, 1, :, pair_ddp:ddp], 0.0)
                        pend.append((m, pair_ddp, nh_m))
                        if len(pend) > 4:
                            emit_av(*pend.popleft())
                while pend:
                    if j == 0:
                        emit_av_b(*pend.popleft())
                    else:
                        emit_av(*pend.popleft())

                # --- softmax normalization: out = av * (1/Z) (as cfg "b") ---
                rr = zpool.tile([65, HPC * TBLK], bf16, tag="rr",
                                name=f"rrr_{bb}_{j}")
                with nc.allow_low_precision(reason="bf16 1/Z broadcast"):
                    for h in range(HPC):
                        nc.vector.reciprocal(
                            rr[64:65, h * TBLK:(h + 1) * TBLK], avs[h][64:65, :])
                for h in reversed(range(HPC)):
                    bc = ps_sh.tile([HD, TBLK], f32, tag="ps_sh",
                                    name=f"bc_{bb}_{j}_{h}")
                    nc.tensor.matmul(bc[:], one1[64:65, :],
                                     rr[64:65, h * TBLK:(h + 1) * TBLK],
                                     start=True, stop=True)
                    bcs = tmpool.tile([HD, TBLK], f32, tag="bcs",
                                      name=f"bcs_{bb}_{j}_{h}")
                    nc.vector.tensor_copy(bcs[:], bc[:])
                    if h == 0:
                        nc.vector.tensor_mul(outT_sb[0:HD, tsl], avs[h][0:HD, :],
                                             bcs[:])
                    else:
                        tmp = tmpool.tile([HD, TBLK], bf16, tag="tmp",
                                          name=f"tm_{bb}_{j}")
                        nc.vector.tensor_mul(tmp[:], avs[h][0:HD, :], bcs[:])
                        if last:
                            # end-game: SBUF->SBUF DMA costs ~1.9us latency in
                            # the serial tail; shift partitions via PE instead
                            # (reuses a scores-pool tile -- the exp stream is
                            # finished by now, so no extra PSUM footprint)
                            pt = ps_s.tile([128, HPC, TBLK], f32, tag="ps_s",
                                           name=f"shf_{bb}_{j}")
                            nc.tensor.matmul(pt[:, 0, :], shf64[:], tmp[:],
                                             start=True, stop=True)
                            nc.vector.tensor_copy(
                                outT_sb[h * HD:(h + 1) * HD, tsl],
                                pt[h * HD:(h + 1) * HD, 0, :])
                        else:
                            nc.sync.dma_start(
                                out=outT_sb[h * HD:(h + 1) * HD, tsl],
                                in_=tmp[:])

                # leftover fillers run after the normalize chain is queued so
                # the recip/mult don't sit behind filler copies on DVE
                while nfill < len(fillers):
                    fillers[nfill][1]()
                    nfill += 1

            PW = min(512, d)
            NIB = d // PW

            def proj_units(bb, j, last=False):
                col0 = bb * t + j * TBLK
                NTL = TBLK // 128
                state = {}

                def piece(tl, ib):
                    if "ot" not in state:
                        state["ot"] = opool.tile([128, NTL, d], bf16, tag="ot",
                                                 name=f"ot_{bb}_{j}")
                    ot = state["ot"]
                    tt = col0 // 128 + tl
                    ps = ps_sh.tile([128, PW], f32, tag="ps_sh",
                                    name=f"psp_{bb}_{j}_{tl}_{ib}")
                    nc.tensor.matmul(ps[:], outT_sb[:, tt * 128:(tt + 1) * 128],
                                     wp_sb[:, ib * PW:(ib + 1) * PW],
                                     start=True, stop=True)
                    if last and (tl * NIB + ib) % 2 == 1:
                        nc.scalar.copy(ot[:, tl, ib * PW:(ib + 1) * PW], ps[:])
                    else:
                        nc.vector.tensor_copy(
                            ot[:, tl, ib * PW:(ib + 1) * PW], ps[:])
                    if ib == NIB - 1 and last:
                        # split the very last tile's store so the final
                        # DMA (+sem) tail is half as long
                        nsp = 2 if tl == NTL - 1 else 1
                        for sp in range(nsp):
                            csl = slice(sp * d // nsp, (sp + 1) * d // nsp)
                            nc.sync.dma_start(
                                out=out_p.rearrange(
                                    "(tb p) c -> p tb c", p=128)[
                                    :, col0 // 128 + tl:col0 // 128 + tl + 1,
                                    csl],
                                in_=ot[:, tl:tl + 1, csl])
                    if tl == NTL - 1 and ib == NIB - 1 and not last:
                        nc.sync.dma_start(
                            out=out_p.rearrange("(tb p) c -> p tb c", p=128)[
                                :, col0 // 128:col0 // 128 + NTL, :],
                            in_=ot[:])

                return [
                    (0.5, (lambda tl_, ib_: lambda: piece(tl_, ib_))(tl, ib))
                    for tl in range(NTL) for ib in range(NIB)
                ]

            def emit_proj(bb, j, last=False):
                for _, u in proj_units(bb, j, last):
                    u()

            blocks = [(bb, j) for bb in range(b) for j in range(NJ)]
            emit_xt(*blocks[0], chunked=True)
            emit_xt(*blocks[1])
            emit_qkv(*blocks[0])
            for idx, blk in enumerate(blocks):
                qk_u = (qkv_units(*blocks[idx + 1])
                        if idx + 1 < len(blocks) else [])
                pr_u = (proj_units(*blocks[idx - 1],
                                   last=(idx == len(blocks) - 1))
                        if idx >= 1 else [])
                fillers = []
                if idx + 2 < len(blocks):
                    bbn, jn = blocks[idx + 2]
                    fillers.append(
                        (0.1, lambda bbn=bbn, jn=jn: emit_xt(bbn, jn)))
                # round-robin qkv and proj pieces: qkv early enough for the
                # next block, proj (which waits on this block's outT
                # predecessor) spread across the span
                qi = pi = 0
                while qi < len(qk_u) or pi < len(pr_u):
                    if qi < len(qk_u):
                        fillers.append(qk_u[qi])
                        qi += 1
                    if pi < len(pr_u):
                        fillers.append(pr_u[pi])
                        pi += 1
                emit_attn(*blk, fillers=fillers, last=(idx == len(blocks) - 1))
            emit_proj(*blocks[-1], last=True)

    nc.compile()
    return nc


def _build(b, t, d, cfg):
    """Build + compile the per-core Bass program."""
    if cfg == "d":
        return _build_d(b, t, d)
    import concourse.tile as tile
    from concourse import bacc, mybir
    from contextlib import ExitStack

    f32 = mybir.dt.float32
    f32r = mybir.dt.float32r
    bf16 = mybir.dt.bfloat16

    rmode = cfg == "r"
    bmode = cfg == "b"
    # dtype of every matmul-feeding tile
    MMDT = f32r if rmode else (bf16 if bmode else f32)
    # dtype of the DMA'd inputs (host converts for bf16)
    INDT = bf16 if bmode else f32

    def bcst(ap):
        return ap.bitcast(f32r) if rmode else ap

    bt = b * t
    KT = d // 128            # k-tiles over the model dim
    TBLK = min(512, t)       # t-block width for scores/attn
    NJ = t // TBLK           # t-blocks per batch
    NSB = bt // 128          # 128-row s-blocks over B*T
    SPT = TBLK // 128        # s-blocks per t-block

    nc = bacc.Bacc("TRN2", target_bir_lowering=False, debug=False)

    xT = nc.dram_tensor("xT", [d, bt], INDT, kind="ExternalInput").ap()
    wq = nc.dram_tensor("wq", [d, CH], INDT, kind="ExternalInput").ap()
    wk = nc.dram_tensor("wk", [d, CH], INDT, kind="ExternalInput").ap()
    wv = nc.dram_tensor("wv", [d, CH], INDT, kind="ExternalInput").ap()
    wp = nc.dram_tensor("wp", [CH, d], INDT, kind="ExternalInput").ap()
    cident = nc.dram_tensor("cident", [128, 128], INDT, kind="ExternalInput").ap()
    cmask = nc.dram_tensor("cmask", [128, TBLK + 384], INDT, kind="ExternalInput").ap()
    cones = nc.dram_tensor("cones", [128, NSB, HPC], INDT, kind="ExternalInput").ap()
    cone1 = nc.dram_tensor("cone1", [65, HD], INDT, kind="ExternalInput").ap()
    # partials are summed across cores on the host in f64; bf16 partial
    # stores halve the output DMA traffic for ~1e-3 extra absmax-rel error
    OUTDT = bf16 if bmode else f32
    out_p = nc.dram_tensor("out_p", [bt, d], OUTDT, kind="ExternalOutput").ap()

    with tile.TileContext(nc) as tc, ExitStack() as top:
        persist = top.enter_context(tc.tile_pool(name="persist", bufs=1))

        # ---- persistent tiles ----
        qT_sb = persist.tile([128, bt], MMDT, tag="qT")
        kT_sb = persist.tile([128, bt], MMDT, tag="kT")
        # [v_h0 | 1 | pad | v_h1 | 1 | pad] per 128-row s-block
        vaug = persist.tile([128, NSB, 66 * HPC], MMDT, tag="vaug")
        outT_sb = persist.tile([128, bt], MMDT, tag="outT")
        wq_sb = persist.tile([128, KT, CH], MMDT, tag="wq")
        wk_sb = persist.tile([128, KT, CH], MMDT, tag="wk")
        wv_sb = persist.tile([128, KT, CH], MMDT, tag="wv")
        wp_sb = persist.tile([128, d], MMDT, tag="wp")
        ident = persist.tile([128, 128], MMDT, tag="ident")
        # staircase mask, shifted: maskt[p, m] = 1 iff m >= p + 384
        maskt = persist.tile([128, TBLK + 384], MMDT, tag="mask")
        one1 = persist.tile([65, HD], MMDT, tag="one1")

        # startup DMAs on the scalar HWDGE queue (fast descriptor gen; the
        # Pool SWDGE takes ~1.1us per DMA), ordered by first use: ident
        # (act-table warm + block-0 transposes), big wq/wk/wv loads, then
        # attention consts; wp (needed only by the lagging proj) last.
        # wq in two halves so the first q matmuls start ~1.4us earlier
        for w_ap, w_sb, nsplit in ((wq, wq_sb, 2), (wk, wk_sb, 1), (wv, wv_sb, 1)):
            for s in range(nsplit):
                hk = slice(s * KT // nsplit, (s + 1) * KT // nsplit)
                nc.scalar.dma_start(
                    out=w_sb[:, hk, :],
                    in_=bcst(w_ap.rearrange("(kt p) m -> p kt m", p=128)[:, hk, :]),
                )
        # preload the Exp activation table under the startup DMAs
        actwarm = persist.tile([1, 8], f32, tag="actwarm")
        nc.scalar.activation(actwarm[:], wq_sb[0:1, 0, 0:8],
                             mybir.ActivationFunctionType.Exp, scale=0.125)
        # ident only feeds the PE-transpose path (non-bf16 modes), but the
        # load stays unconditional: dropping it shifts the startup DMA
        # phasing and measures 2.6us WORSE in bmode (scheduler alignment)
        nc.scalar.dma_start(out=ident[:], in_=bcst(cident))
        nc.scalar.dma_start(out=maskt[:], in_=bcst(cmask))
        nc.scalar.dma_start(out=one1[:], in_=bcst(cone1))
        for h in range(HPC):
            nc.scalar.dma_start(
                out=vaug[:, :, 66 * h + 64:66 * h + 65],
                in_=bcst(cones[:, :, h:h + 1]),
            )
        nc.scalar.dma_start(out=wp_sb[:], in_=bcst(wp))

        # ---- merged loop: per (batch, t-block): QKV -> attention -> proj ----
        # Attention for block j of batch bb needs q columns of block j and
        # k/v columns of blocks 0..j (same batch) -- all computed by the time
        # block j's QKV is done, so one fused loop pipelines everything:
        # xT loads prefetch under attention PE work, and output stores drain
        # under the next block's compute.
        PW = min(512, d)
        NIB = d // PW
        with ExitStack() as body:
            xpool = body.enter_context(tc.tile_pool(name="xpool", bufs=4 if bmode else 3))
            vtpool = body.enter_context(tc.tile_pool(name="vtpool", bufs=2))
            npool = body.enter_context(tc.tile_pool(name="npool", bufs=24 if bmode else 5))
            zpool = body.enter_context(tc.tile_pool(name="zpool", bufs=2))
            tmpool = body.enter_context(tc.tile_pool(name="tmpool", bufs=2))
            opool = body.enter_context(tc.tile_pool(name="opool", bufs=2))
            # PSUM budget (8 banks): qkv 2 + scores 2 + av 2 + tr/proj/bc 2
            ps_qkv = body.enter_context(tc.tile_pool(name="ps_qkv", bufs=2, space="PSUM"))
            ps_s = body.enter_context(tc.tile_pool(name="ps_s", bufs=2, space="PSUM"))
            ps_av = body.enter_context(tc.tile_pool(name="ps_av", bufs=2, space="PSUM"))
            ps_tp = body.enter_context(tc.tile_pool(name="ps_tp", bufs=2, space="PSUM"))

            xt_tiles = {}

            def emit_xt(bb, j, chunked=False):
                col0 = bb * t + j * TBLK
                tsl = slice(col0, col0 + TBLK)
                xt = xpool.tile([128, KT, TBLK], MMDT, tag="xt", name=f"xt_{bb}_{j}")
                if chunked:
                    for kt in range(KT):
                        nc.sync.dma_start(
                            out=xt[:, kt, :],
                            in_=bcst(xT[kt * 128:(kt + 1) * 128, tsl]),
                        )
                else:
                    nc.sync.dma_start(
                        out=xt[:],
                        in_=bcst(xT.rearrange("(kt p) c -> p kt c", p=128)[:, :, tsl]),
                    )
                xt_tiles[(bb, j)] = xt

            def emit_qkv(bb, j):
                col0 = bb * t + j * TBLK
                tsl = slice(col0, col0 + TBLK)
                xt = xt_tiles.pop((bb, j))
                for w_sb, dst in ((wq_sb, qT_sb), (wk_sb, kT_sb)):
                    ps = ps_qkv.tile([128, TBLK], f32, tag="ps_qkv",
                                     name=f"psq_{bb}_{j}_{dst.name}")
                    for kt in range(KT):
                        nc.tensor.matmul(ps[:], w_sb[:, kt, :], xt[:, kt, :],
                                         start=(kt == 0), stop=(kt == KT - 1))
                    nc.vector.tensor_copy(dst[:, tsl], ps[:])
                if bmode:
                    # bf16 runs 1 cyc/row at width 128: compute v directly in
                    # [s, e] orientation (lhsT = x tile), skipping the PE
                    # transpose and the vt staging copy entirely
                    ps = ps_qkv.tile([128, SPT, 128], f32, tag="ps_qkv",
                                     name=f"psv_{bb}_{j}")
                    for s4 in range(SPT):
                        for kt in range(KT):
                            nc.tensor.matmul(
                                ps[:, s4, :],
                                xt[:, kt, s4 * 128:(s4 + 1) * 128],
                                wv_sb[:, kt, :],
                                start=(kt == 0), stop=(kt == KT - 1),
                                skip_group_check=True)
                    for s4 in range(SPT):
                        sb_idx = (col0 // 128) + s4
                        nc.vector.tensor_copy(
                            vaug[:, sb_idx, :].rearrange(
                                "p (g c) -> p g c", g=HPC)[:, :, 0:HD],
                            ps[:, s4, :].rearrange("p (g c) -> p g c", g=HPC),
                        )
                else:
                    ps = ps_qkv.tile([128, TBLK], f32, tag="ps_qkv", name=f"psv_{bb}_{j}")
                    for kt in range(KT):
                        nc.tensor.matmul(ps[:], wv_sb[:, kt, :], xt[:, kt, :],
                                         start=(kt == 0), stop=(kt == KT - 1))
                    vt = vtpool.tile([128, TBLK], MMDT, tag="vt", name=f"vt_{bb}_{j}")
                    nc.vector.tensor_copy(vt[:], ps[:])
                    for s4 in range(SPT):
                        sb_idx = (col0 // 128) + s4
                        pt = ps_tp.tile([128, 128], MMDT, tag="ps_tp",
                                        name=f"ptr_{bb}_{j}_{s4}")
                        nc.tensor.transpose(pt[:], vt[:, s4 * 128:(s4 + 1) * 128],
                                            ident[:])
                        nc.vector.tensor_copy(
                            vaug[:, sb_idx, :].rearrange(
                                "p (g c) -> p g c", g=HPC)[:, :, 0:HD],
                            pt[:].rearrange("p (g c) -> p g c", g=HPC),
                        )

            def emit_attn(bb, j, last=False):
                col0 = bb * t + j * TBLK
                tsl = slice(col0, col0 + TBLK)
                n_i = (j + 1) * SPT
                avs = [ps_av.tile([65, TBLK], f32, tag="ps_av", name=f"av_{bb}_{j}_{h}")
                       for h in range(HPC)]

                def emit_av(i_, ddp_, nh_):
                    for h in range(HPC):
                        sb_idx = (bb * t + i_ * 128) // 128
                        nc.tensor.matmul(
                            avs[h][:, ddp_:], vaug[:, sb_idx, h * 66:h * 66 + HD + 1],
                            nh_[:, h * TBLK + ddp_:(h + 1) * TBLK],
                            start=(i_ == 0), stop=(i_ == n_i - 1),
                            skip_group_check=True)

                from collections import deque
                pend = deque()
                for i in range(n_i):
                    ssl = slice(bb * t + i * 128, bb * t + i * 128 + 128)
                    dd = 128 * i - TBLK * j
                    # column trim: scores/exp/av touch only cols >= ddp
                    # (f32r needs free dim >= 256 for the PE fast path;
                    # bf16 runs 1 cyc/row at any width so trim fully)
                    if rmode:
                        ddp = max(0, min(dd, TBLK - 256))
                    else:
                        ddp = max(0, dd)
                    nh = npool.tile([128, HPC * TBLK], MMDT, tag="nh",
                                    name=f"nh_{bb}_{j}_{i}")
                    for h in range(HPC):
                        hp = slice(h * HD, (h + 1) * HD)
                        ps = ps_s.tile([128, TBLK], f32, tag="ps_s",
                                       name=f"pss_{bb}_{j}_{i}_{h}")
                        nc.tensor.matmul(
                            ps[:, ddp:], kT_sb[hp, ssl],
                            qT_sb[hp, tsl][:, ddp:], start=True, stop=True)
                        nc.scalar.activation(
                            nh[:, h * TBLK + ddp:(h + 1) * TBLK], ps[:, ddp:],
                            mybir.ActivationFunctionType.Exp, scale=0.125)
                        if dd >= 0:
                            # mask cols [ddp, dd+128): staircase + trim slack
                            # (nh col c maps to mask col c + 384 - dd)
                            nc.gpsimd.tensor_mul(
                                nh[:, h * TBLK + ddp:h * TBLK + dd + 128],
                                nh[:, h * TBLK + ddp:h * TBLK + dd + 128],
                                maskt[:, 384 - dd + ddp:512])
                    # attn@v lags 12 i-steps (== fully deferred for most
                    # blocks): the scores/exp stream runs uninterrupted, then
                    # the AV batch runs at full PE rate against banked nh
                    # tiles -- measured best across lag 1..16
                    pend.append((i, ddp, nh))
                    if len(pend) > 12:
                        i_, ddp_, nh_ = pend.popleft()
                        emit_av(i_, ddp_, nh_)
                while pend:
                    i_, ddp_, nh_ = pend.popleft()
                    emit_av(i_, ddp_, nh_)

                # --- softmax normalization: out = av * (1/Z) ---
                rr = zpool.tile([65, HPC * TBLK], MMDT, tag="rr",
                                name=f"rrr_{bb}_{j}")
                with nc.allow_low_precision(reason="f32r PE broadcast of 1/Z"):
                    for h in range(HPC):
                        nc.vector.reciprocal(
                            rr[64:65, h * TBLK:(h + 1) * TBLK], avs[h][64:65, :])
                # h1 first: its outT write goes through a SBUF->SBUF DMA
                # (partition shift), so start it before h0's direct DVE write
                bcs_h = {}
                for h in reversed(range(HPC)):
                    # K=1 matmul broadcasts 1/Z across the 64 output partitions
                    bc = ps_tp.tile([HD, TBLK], f32, tag="ps_tp", name=f"bc_{bb}_{j}_{h}")
                    nc.tensor.matmul(bc[:], one1[64:65, :],
                                     rr[64:65, h * TBLK:(h + 1) * TBLK],
                                     start=True, stop=True)
                    # DVE may read only one PSUM operand: stage bc in SBUF
                    # (on DVE -- ACT is loaded with the exp stream)
                    bcs = tmpool.tile([HD, TBLK], f32, tag="bcs", name=f"bcs_{bb}_{j}_{h}")
                    nc.vector.tensor_copy(bcs[:], bc[:])
                    bcs_h[h] = bcs
                    if h == 0:
                        nc.vector.tensor_mul(outT_sb[0:HD, tsl], avs[h][0:HD, :], bcs[:])
                    else:
                        tmp = tmpool.tile([HD, TBLK], MMDT, tag="tmp", name=f"tm_{bb}_{j}")
                        nc.vector.tensor_mul(tmp[:], avs[h][0:HD, :], bcs[:])
                        nc.sync.dma_start(
                            out=outT_sb[h * HD:(h + 1) * HD, tsl], in_=tmp[:])

            def emit_proj(bb, j, last=False):
                col0 = bb * t + j * TBLK
                NTL = TBLK // 128
                ot = opool.tile([128, NTL, d], OUTDT, tag="ot", name=f"ot_{bb}_{j}")
                for tl in range(NTL):
                    tt = col0 // 128 + tl
                    for ib in range(NIB):
                        # drain only: scores pool is idle, alternate pools for
                        # a 4-deep ring so the matmul stream is not copy-paced
                        pools = ((ps_tp, "ps_tp"), (ps_s, "ps_s"),
                                 (ps_qkv, "ps_qkv"))
                        pp, ptag = pools[(tl * NIB + ib) % 3] if last else pools[0]
                        ps = pp.tile([128, PW], f32, tag=ptag,
                                     name=f"psp_{bb}_{j}_{tl}_{ib}")
                        nc.tensor.matmul(ps[:], outT_sb[:, tt * 128:(tt + 1) * 128],
                                         wp_sb[:, ib * PW:(ib + 1) * PW],
                                         start=True, stop=True)
                        # in the drain there is no exp stream: split copies
                        # between DVE and ACT and store per row-block pair so
                        # the store overlaps the remaining copies
                        if last and (tl * NIB + ib) % 2 == 1:
                            nc.scalar.copy(ot[:, tl, ib * PW:(ib + 1) * PW], ps[:])
                        else:
                            nc.vector.tensor_copy(
                                ot[:, tl, ib * PW:(ib + 1) * PW], ps[:])
                    if last:
                        nc.sync.dma_start(
                            out=out_p.rearrange("(tb p) c -> p tb c", p=128)[
                                :, col0 // 128 + tl:col0 // 128 + tl + 1, :],
                            in_=ot[:, tl:tl + 1, :])
                if not last:
                    # one store DMA per block: [p, tl, d] -> row-blocks of out_p
                    nc.sync.dma_start(
                        out=out_p.rearrange("(tb p) c -> p tb c", p=128)[
                            :, col0 // 128:col0 // 128 + NTL, :],
                        in_=ot[:])

            # software pipeline: QKV runs one t-block ahead of attention, and
            # the projection lags one block behind, so block-boundary DVE/DMA
            # latencies hide under attention PE work
            blocks = [(bb, j) for bb in range(b) for j in range(NJ)]
            emit_xt(*blocks[0], chunked=True)
            emit_xt(*blocks[1])
            emit_qkv(*blocks[0])
            deferred = {2, 4, 6} if len(blocks) == 8 else set()
            for idx, blk in enumerate(blocks):
                if idx + 2 < len(blocks):
                    emit_xt(*blocks[idx + 2])
                if idx + 1 < len(blocks):
                    emit_qkv(*blocks[idx + 1])
                if idx == len(blocks) - 1:
                    for dfx in sorted(deferred):
                        emit_proj(*blocks[dfx - 1])
                emit_attn(*blk, last=(idx == len(blocks) - 1))
                if idx >= 1 and idx not in deferred:
                    emit_proj(*blocks[idx - 1], last=(idx == len(blocks) - 1))
            emit_proj(*blocks[-1], last=True)

    nc.compile()
    return nc


def _get_nc(b=B, t=T, d=D, cfg="b"):
    key = (b, t, d, cfg)
    if key not in _CACHE:
        _CACHE[key] = _build(b, t, d, cfg)
    return _CACHE[key]


def _in_dtype(cfg):
    if cfg == "b":
        import ml_dtypes
        return np.dtype(ml_dtypes.bfloat16)
    return np.dtype(np.float32)


def _make_consts(b, t, d, dt):
    bt = b * t
    TBLK = min(512, t)
    NSB = bt // 128
    cident = np.eye(128, dtype=dt)
    p = np.arange(128, dtype=np.int64)[:, None]
    m = np.arange(TBLK + 384, dtype=np.int64)[None, :]
    cmask = (m >= p + 384).astype(dt)
    cones = np.ones((128, NSB, HPC), dtype=dt)
    cone1 = np.ones((65, HD), dtype=dt)
    return {"cident": cident, "cmask": cmask, "cones": cones, "cone1": cone1}


def _hilo(a):
    import ml_dtypes
    f8 = np.dtype(ml_dtypes.float8_e4m3)
    hi = a.astype(f8)
    lo = (a.astype(np.float32) - hi.astype(np.float32)).astype(f8)
    return np.ascontiguousarray(hi), np.ascontiguousarray(lo)


def _prepare_in_maps_d(x, Wq, Wk, Wv, Wp, b, t, d):
    import ml_dtypes
    f8 = np.dtype(ml_dtypes.float8_e4m3)
    bf = np.dtype(ml_dtypes.bfloat16)
    bt = b * t
    TBLK = min(512, t)
    NSB = bt // 128
    SPT = TBLK // 128
    xT = x.reshape(bt, d).T.astype(np.float32)
    xTh, xTl = _hilo(xT)
    p = np.arange(128, dtype=np.int64)[:, None]
    m = np.arange(TBLK + 384, dtype=np.int64)[None, :]
    cmask = (m >= p + 384).astype(f8)
    cmaskb = (m >= p + 384).astype(bf)
    cones = np.zeros((128, NSB // 2, 2, 160), dtype=f8)
    cones[:, :, :, 64] = 1.0
    cones[:, :, :, 144] = 1.0
    conesb = np.zeros((128, b, SPT, 132), dtype=bf)
    conesb[:, :, :, 64] = 1.0
    conesb[:, :, :, 130] = 1.0
    cone1 = np.ones((65, HD), dtype=bf)
    cshf = np.zeros((HD, 128), dtype=bf)
    cshf[np.arange(HD), HD + np.arange(HD)] = 1.0
    in_maps = []
    for c in range(NCORES):
        h0 = c * HPC
        im = {"xTh": xTh, "xTl": xTl, "cmask": cmask, "cmaskb": cmaskb,
              "cones": cones, "conesb": conesb, "cone1": cone1, "cshf": cshf}
        for nm, W in (("wq", Wq), ("wk", Wk), ("wv", Wv)):
            w_c = W[h0:h0 + HPC].reshape(CH, d).T.astype(np.float32)
            im[nm + "h"], im[nm + "l"] = _hilo(w_c)
        im["wp"] = np.ascontiguousarray(
            Wp[:, c * CH:(c + 1) * CH].T.astype(bf))
        in_maps.append(im)
    return in_maps


def _prepare_in_maps(x, Wq, Wk, Wv, Wp, b, t, d, cfg):
    if cfg == "d":
        return _prepare_in_maps_d(x, Wq, Wk, Wv, Wp, b, t, d)
    bt = b * t
    dt = _in_dtype(cfg)
    xT = np.ascontiguousarray(x.reshape(bt, d).T.astype(dt))
    consts = _make_consts(b, t, d, dt)
    in_maps = []
    for c in range(NCORES):
        h0 = c * HPC
        wq_c = np.ascontiguousarray(Wq[h0:h0 + HPC].reshape(CH, d).T.astype(dt))
        wk_c = np.ascontiguousarray(Wk[h0:h0 + HPC].reshape(CH, d).T.astype(dt))
        wv_c = np.ascontiguousarray(Wv[h0:h0 + HPC].reshape(CH, d).T.astype(dt))
        wp_c = np.ascontiguousarray(Wp[:, c * CH:(c + 1) * CH].T.astype(dt))
        in_maps.append({"xT": xT, "wq": wq_c, "wk": wk_c, "wv": wv_c, "wp": wp_c,
                        **consts})
    return in_maps


def _run(x, Wq, Wk, Wv, Wp, bp, b, t, d, cfg, trace=False):
    from concourse.bass_utils import run_bass_kernel_spmd
    nc = _get_nc(b, t, d, cfg)
    in_maps = _prepare_in_maps(x, Wq, Wk, Wv, Wp, b, t, d, cfg)
    res = run_bass_kernel_spmd(nc, in_maps, core_ids=list(range(NCORES)), trace=trace)
    acc = np.zeros((b * t, d), dtype=np.float64)
    for r in res.results:
        acc += r["out_p"].astype(np.float64)
    out = (acc + np.asarray(bp, dtype=np.float64)).astype(np.float32)
    return out.reshape(b, t, d), res


KERNEL_CFG = "d"


def kernel(x, Wq, Wk, Wv, Wp, bp):
    out, _ = _run(np.asarray(x), np.asarray(Wq), np.asarray(Wk), np.asarray(Wv),
                  np.asarray(Wp), np.asarray(bp), B, T, D, KERNEL_CFG, trace=False)
    return out



# revision 79
# speedup vs baseline: 1.1262x; 1.0308x over previous
"""Multi-head attention Trainium2 kernel (8 NeuronCores, tensor-parallel over heads).

Sharding: 16 heads / 8 cores = 2 heads per core. x is replicated; Wq/Wk/Wv
sharded by head; Wp row-sharded (contraction dim). Each core computes a
partial projection output [B*T, D] stored bf16; the host sums the 8 partials
in f64 (+bias).

Default cfg "d" (fp8 DoubleRow; ~135us sim vs 147us for the bf16 cfg "b",
absmax-relative error ~3.6e-3 vs gate 2e-2):
  - QKV: x and W are split hi+lo fp8e4m3 on the host; W is pre-scaled x32 so
    both hi and the residual stay in fp8's NORMAL range (raw W~0.02 values
    are subnormal in e4m3, which destroys the residual trick). Contraction
    runs as 3 DoubleRow cross terms (Wh*xh + Wl*xh + Wh*xl), each packing
    K=256 at 0.5 cyc/row: 12 matmuls of N/2 vs bf16's 8 of N. The x32 is
    compensated for free: scores come out x1024 and the exp scale constant
    becomes 0.125/1024; v comes out x32 and the broadcast-ones constant used
    to expand 1/Z is 1/32.
  - Scores stay bf16 [s on partitions, t free]; both heads' scores land in
    one 2-bank PSUM tile so exp is ONE activation per s-block (halves the
    ACT per-op overhead; the exp stream is the attention-phase critical
    path). exp writes fp8 nh directly.
  - attn@v runs DoubleRow over PAIRS of s-blocks (contraction 256, cost
    width/2 per pair, 4x denser than bf16): lhsT = packed [v32|1] planes,
    rhs = nh pair planes; out rows 0..63 = unnormalized out^T x32, row 64 =
    Z. Plane-1 columns of diagonal pairs that sit above that plane's
    diagonal are memset on Pool. j=0 blocks (rows t<512, few softmax terms,
    largest weights -> fp8 noise would dominate absmax) use a bf16 attn@v
    path instead (vaug_b/masktb).
  - All vaug images are FULLY preloaded from host constants: reading
    uninitialized SBUF as fp8 can yield NaN (0xFF) on the first call.
  - Causality: lower-triangular [128s x 512t] blocks only; diagonal
    straddlers are column-trimmed and masked multiplicatively on Pool.
    Softmax runs without max subtraction (logits are O(1)).
  - Scheduling: the attention i-loop is ACT(exp)-paced, so qkv(j+1),
    proj(j-1) and x-tile prefetches are emitted as weighted "filler" pieces
    spread through it (the PE runs strictly in-order; contiguous chunks
    would starve either PE or ACT). PSUM: scores 2x2 banks + av 2 + shared
    qkv/proj/bc ring 2. The last block's h1 outT shift goes through a PE
    shift-matrix matmul instead of the ~1.9us SBUF->SBUF DMA.
cfg "b": all-bf16 fallback (the previous baseline), cfg "r": float32r.
"""

import numpy as np

B, T, D, H, HD = 2, 2048, 1024, 16, 64
NCORES = 8
HPC = H // NCORES          # heads per core = 2
CH = HPC * HD              # channels per core = 128
BT = B * T

_CACHE = {}


def _build_d(b, t, d):
    """cfg "d": fp8 DoubleRow build.

    - QKV: x and Wq/Wk/Wv are split hi/lo fp8e4m3 on the host (x = xh + xl
      exactly captures x to ~0.2%); contraction runs as 3 DoubleRow cross
      terms (Wh*xh + Wh*xl + Wl*xh) of K=256 each -> 12 matmuls of
      cost N/2 instead of 8 of cost N (sim model: fp8 DoubleRow is 0.5
      cycles/row with 2 K-planes packed per matmul).
    - Scores stay bf16 [s, t]; exp is ONE activation per s-block covering
      both heads (scores psum is a 2-bank [128, 2, TBLK] tile).
    - exp output is fp8 directly; attn@v runs DoubleRow over PAIRS of
      s-blocks (contraction 256): cost width/2 per pair instead of
      2*width. vaug holds fp8 v planes [128, NSB/2, 2, 160] with the
      softmax-denominator ones column at 64/144 per head.
    - Diagonal pairs: plane-1 columns below its own diagonal trim but
      inside the pair's matmul slice are memset to 0 on Pool.
    - Normalization/proj unchanged from cfg "b" (bf16).
    """
    import concourse.tile as tile
    from concourse import bacc, mybir
    from contextlib import ExitStack
    from collections import deque

    f32 = mybir.dt.float32
    bf16 = mybir.dt.bfloat16
    f8 = mybir.dt.float8e4
    DR = mybir.MatmulPerfMode.DoubleRow

    bt = b * t
    KT = d // 128
    KP = KT // 2             # DoubleRow k-pair steps
    TBLK = min(512, t)
    NJ = t // TBLK
    NSB = bt // 128
    SPT = TBLK // 128

    nc = bacc.Bacc("TRN2", target_bir_lowering=False, debug=False)

    xTh = nc.dram_tensor("xTh", [d, bt], f8, kind="ExternalInput").ap()
    xTl = nc.dram_tensor("xTl", [d, bt], f8, kind="ExternalInput").ap()
    w_in = {}
    for nm in ("wqh", "wql", "wkh", "wkl", "wvh", "wvl"):
        w_in[nm] = nc.dram_tensor(nm, [d, CH], f8, kind="ExternalInput").ap()
    wp = nc.dram_tensor("wp", [CH, d], bf16, kind="ExternalInput").ap()
    cmask = nc.dram_tensor("cmask", [128, TBLK + 384], f8, kind="ExternalInput").ap()
    cmaskb = nc.dram_tensor("cmaskb", [128, TBLK + 384], bf16,
                            kind="ExternalInput").ap()
    # FULL vaug images (ones columns + zero padding): loading the whole tile
    # avoids any read of uninitialized SBUF (fp8 garbage can be NaN) on the
    # first call
    cones = nc.dram_tensor("cones", [128, NSB // 2, 2, 160], f8,
                           kind="ExternalInput").ap()
    conesb = nc.dram_tensor("conesb", [128, b, SPT, 132], bf16,
                            kind="ExternalInput").ap()
    cone1 = nc.dram_tensor("cone1", [65, HD], bf16, kind="ExternalInput").ap()
    cshf = nc.dram_tensor("cshf", [HD, 128], bf16, kind="ExternalInput").ap()
    out_p = nc.dram_tensor("out_p", [bt, d], bf16, kind="ExternalOutput").ap()

    with tile.TileContext(nc) as tc, ExitStack() as top:
        persist = top.enter_context(tc.tile_pool(name="persist", bufs=1))

        qT_sb = persist.tile([128, bt], bf16, tag="qT")
        kT_sb = persist.tile([128, bt], bf16, tag="kT")
        # v planes: per (pair, plane): [v_h0 | 1 | pad @80 | v_h1 | 1 | pad]
        vaug = persist.tile([128, NSB // 2, 2, 160], f8, tag="vaug")
        # bf16 v for the first 4 s-blocks of each batch: the j=0 attention
        # blocks (rows t<512, where softmax weights are largest and fp8
        # noise dominates the absmax error) run a bf16 attn@v path
        vaug_b = persist.tile([128, b, SPT, 132], bf16, tag="vaug_b")
        outT_sb = persist.tile([128, bt], bf16, tag="outT")
        w_sb = {}
        for nm in ("wqh", "wql", "wkh", "wkl", "wvh", "wvl"):
            w_sb[nm] = persist.tile([128, KP, 2, CH], f8, tag=nm, name=nm)
        wp_sb = persist.tile([128, d], bf16, tag="wp")
        maskt = persist.tile([128, TBLK + 384], f8, tag="mask")
        masktb = persist.tile([128, TBLK + 384], bf16, tag="maskb")
        one1 = persist.tile([65, HD], bf16, tag="one1")
        # shift matrix: shf64[p, 64+p] = 1 moves rows 0..63 -> 64..127 via PE
        shf64 = persist.tile([HD, 128], bf16, tag="shf64")

        # startup: only the first-needed wq halves ride the shared HWDGE
        # (which the x-tile stream also needs); everything else goes through
        # the Pool SWDGE queue so it doesn't stall the critical path
        def wload(nm, eng, nsplit=1):
            for s in range(nsplit):
                hk = slice(s * KP // nsplit, (s + 1) * KP // nsplit)
                eng.dma_start(
                    out=w_sb[nm][:, hk, :, :],
                    in_=w_in[nm].rearrange(
                        "(kp pl p) m -> p kp pl m", p=128, pl=2)[:, hk, :, :],
                )
        wload("wqh", nc.scalar, 1)
        # warm the Exp act table under the startup DMAs
        actwarm = persist.tile([1, 8], f32, tag="actwarm")
        nc.scalar.activation(actwarm[:], w_sb["wqh"][0:1, 0, 0, 0:8],
                             mybir.ActivationFunctionType.Exp, scale=0.125)
        wload("wkh", nc.scalar)
        wload("wql", nc.scalar)
        wload("wkl", nc.scalar)
        wload("wvh", nc.scalar)
        wload("wvl", nc.scalar)
        # the full vaug images must land before the first v copies (~7us);
        # masktb before the first attention block's masks (~10us)
        nc.scalar.dma_start(out=vaug[:], in_=cones)
        nc.scalar.dma_start(out=vaug_b[:], in_=conesb)
        nc.scalar.dma_start(out=masktb[:], in_=cmaskb)
        nc.scalar.dma_start(out=maskt[:], in_=cmask)
        nc.scalar.dma_start(out=one1[:], in_=cone1)
        nc.scalar.dma_start(out=shf64[:], in_=cshf)
        nc.scalar.dma_start(out=wp_sb[:], in_=wp)

        with ExitStack() as body:
            xpool = body.enter_context(tc.tile_pool(name="xpool", bufs=4))
            npool = body.enter_context(tc.tile_pool(name="npool", bufs=9))
            npool_b = body.enter_context(tc.tile_pool(name="npool_b", bufs=3))
            zpool = body.enter_context(tc.tile_pool(name="zpool", bufs=2))
            tmpool = body.enter_context(tc.tile_pool(name="tmpool", bufs=2))
            opool = body.enter_context(tc.tile_pool(name="opool", bufs=3))
            # PSUM (8 banks): scores 2x2 + av 2 + shared(qkv/proj/bc) 2
            ps_s = body.enter_context(tc.tile_pool(name="ps_s", bufs=2, space="PSUM"))
            ps_av = body.enter_context(tc.tile_pool(name="ps_av", bufs=2, space="PSUM"))
            ps_sh = body.enter_context(tc.tile_pool(name="ps_sh", bufs=2, space="PSUM"))

            xt_tiles = {}

            def emit_xt(bb, j, chunked=False):
                col0 = bb * t + j * TBLK
                tsl = slice(col0, col0 + TBLK)
                srcs = (("xh", xTh), ("xl", xTl))
                pair = [xpool.tile([128, KT, TBLK], f8, tag=nm,
                                   name=f"{nm}_{bb}_{j}")
                        for nm, _ in srcs]
                if chunked:
                    # half-tile pieces, hi/lo interleaved, so the first matmul
                    # group's inputs land as early as possible
                    for kh in range(2):
                        for xt, (nm, src) in zip(pair, srcs):
                            nc.sync.dma_start(
                                out=xt[:, 4 * kh:4 * kh + 4, :],
                                in_=src.rearrange(
                                    "(kt p) c -> p kt c",
                                    p=128)[:, 4 * kh:4 * kh + 4, tsl])
                else:
                    for xt, (nm, src) in zip(pair, srcs):
                        nc.sync.dma_start(
                            out=xt[:],
                            in_=src.rearrange("(kt p) c -> p kt c", p=128)[:, :, tsl])
                xt_tiles[(bb, j)] = pair

            def qkv_units(bb, j):
                """One block's QKV as weighted filler pieces (weight ~= us of
                PE work, used to spread emission across the attention i-loop
                without starving the ACT exp stream)."""
                col0 = bb * t + j * TBLK
                tsl = slice(col0, col0 + TBLK)
                state = {}
                # x-residual term order: hi*hi, lo_w*hi, hi*lo_x -- the lo x
                # tile is only needed by the last 4 matmuls of each group
                QTERMS = (("h", "xh"), ("l", "xh"), ("h", "xl"))

                def qk_mm(wt, dst, ti):
                    xts = dict(zip(("xh", "xl"), xt_tiles[(bb, j)]))
                    key = "ps" + wt
                    if ti == 0:
                        state[key] = ps_sh.tile([128, TBLK], f32, tag="ps_sh",
                                                name=f"psq_{bb}_{j}_{dst.name}")
                    ps = state[key]
                    ws, xn = QTERMS[ti]
                    for kp in range(KP):
                        nc.tensor.matmul(
                            ps[:], w_sb[wt + ws][:, kp, :, :],
                            xts[xn][:, 2 * kp:2 * kp + 2, :],
                            start=(ti == 0 and kp == 0),
                            stop=(ti == 2 and kp == KP - 1),
                            perf_mode=DR)
                    if ti == 2:
                        nc.vector.tensor_copy(dst[:, tsl], ps[:])

                def v_mm(s4):
                    xth, xtl = xt_tiles[(bb, j)]
                    if "vps" not in state:
                        state["vps"] = ps_sh.tile([128, SPT, 128], f32,
                                                  tag="ps_sh",
                                                  name=f"psv_{bb}_{j}")
                    ps = state["vps"]
                    idx = 0
                    for xh, wn in ((xth, "wvh"), (xth, "wvl"), (xtl, "wvh")):
                        for kp in range(KP):
                            nc.tensor.matmul(
                                ps[:, s4, :],
                                xh[:, 2 * kp:2 * kp + 2,
                                   s4 * 128:(s4 + 1) * 128],
                                w_sb[wn][:, kp, :, :],
                                start=(idx == 0), stop=(idx == 3 * KP - 1),
                                perf_mode=DR, skip_group_check=True)
                            idx += 1
                    sb_idx = (col0 // 128) + s4
                    m, pl = divmod(sb_idx, 2)
                    nc.vector.tensor_copy(
                        vaug[:, m, pl, :].rearrange(
                            "p (g c) -> p g c", g=2)[:, :, 0:HD],
                        ps[:, s4, :].rearrange("p (g c) -> p g c", g=HPC),
                    )
                    if j == 0:
                        # bf16 copy for the j=0 attention path
                        nc.vector.tensor_copy(
                            vaug_b[:, bb, s4, :].rearrange(
                                "p (g c) -> p g c", g=2)[:, :, 0:HD],
                            ps[:, s4, :].rearrange("p (g c) -> p g c", g=HPC),
                        )
                    if s4 == SPT - 1:
                        xt_tiles.pop((bb, j))

                units = []
                for wt, dst in (("wq", qT_sb), ("wk", kT_sb)):
                    for ti in range(3):
                        units.append((0.45, (lambda wt_=wt, dst_=dst, ti_=ti:
                                             qk_mm(wt_, dst_, ti_))))
                for s4 in range(SPT):
                    units.append((0.9, (lambda s4_=s4: v_mm(s4_))))
                return units

            def emit_qkv(bb, j):
                for _, u in qkv_units(bb, j):
                    u()

            def emit_attn(bb, j, fillers=(), last=False):
                col0 = bb * t + j * TBLK
                tsl = slice(col0, col0 + TBLK)
                n_i = (j + 1) * SPT
                n_pairs = n_i // 2
                avs = [ps_av.tile([65, TBLK], f32, tag="ps_av",
                                  name=f"av_{bb}_{j}_{h}")
                       for h in range(HPC)]

                def emit_av(m_, ddp_, nh_):
                    sbp = bb * (t // 256) + m_
                    for h in range(HPC):
                        nc.tensor.matmul(
                            avs[h][:, ddp_:],
                            vaug[:, sbp, :, 80 * h:80 * h + HD + 1],
                            nh_[:, :, h, ddp_:],
                            start=(m_ == 0), stop=(m_ == n_pairs - 1),
                            perf_mode=DR, skip_group_check=True)

                def emit_av_b(i_, ddp_, nh_, pl_):
                    # bf16 path (j=0 blocks): per-s-block standard matmul
                    for h in range(HPC):
                        nc.tensor.matmul(
                            avs[h][:, ddp_:],
                            vaug_b[:, bb, i_, 66 * h:66 * h + HD + 1],
                            nh_[:, pl_, h, ddp_:],
                            start=(i_ == 0), stop=(i_ == n_i - 1),
                            skip_group_check=True)

                fillers = list(fillers)
                wtotal = sum(w for w, _ in fillers) or 1.0
                nfill = 0
                wdone = 0.0

                pend = deque()
                nh_m = None
                pair_ddp = 0
                for i in range(n_i):
                    # spread deferred qkv/proj/xt work through the i-loop by
                    # cumulative PE-work weight, so the PE has in-order work
                    # during exp waits without long bursts that starve ACT
                    # finish fillers ~2 i-steps early so the DVE queue is
                    # drained when the block-end normalize chain needs it
                    want = (i / max(1, n_i - 2)) * wtotal
                    while nfill < len(fillers) and wdone < want:
                        w, fn = fillers[nfill]
                        fn()
                        wdone += w
                        nfill += 1
                    ssl = slice(bb * t + i * 128, bb * t + i * 128 + 128)
                    dd = 128 * i - TBLK * j
                    ddp = max(0, dd)
                    m, pl = divmod(i, 2)
                    nhdt = bf16 if j == 0 else f8
                    nhmask = masktb if j == 0 else maskt
                    if pl == 0:
                        np_ = npool_b if j == 0 else npool
                        nh_m = np_.tile([128, 2, HPC, TBLK], nhdt,
                                        tag="nhb" if j == 0 else "nh",
                                        name=f"nh_{bb}_{j}_{m}")
                        pair_ddp = ddp
                    ps = ps_s.tile([128, HPC, TBLK], f32, tag="ps_s",
                                   name=f"pss_{bb}_{j}_{i}")
                    for h in range(HPC):
                        hp = slice(h * HD, (h + 1) * HD)
                        nc.tensor.matmul(
                            ps[:, h, ddp:], kT_sb[hp, ssl],
                            qT_sb[hp, tsl][:, ddp:], start=True, stop=True)
                    nc.scalar.activation(
                        nh_m[:, pl, :, ddp:], ps[:, :, ddp:],
                        mybir.ActivationFunctionType.Exp, scale=0.125)
                    if dd >= 0:
                        for h in range(HPC):
                            nc.gpsimd.tensor_mul(
                                nh_m[:, pl, h, ddp:dd + 128],
                                nh_m[:, pl, h, ddp:dd + 128],
                                nhmask[:, 384 - dd + ddp:512])
                    if j == 0:
                        pend.append((i, ddp, nh_m, pl))
                        if len(pend) > 2:
                            emit_av_b(*pend.popleft())
                    elif pl == 1:
                        if ddp > pair_ddp:
                            # plane-1 cols [pair_ddp, ddp) are inside the AV
                            # slice but above this plane's diagonal: zero them
                            nc.gpsimd.memset(nh_m[:, 1, :, pair_ddp:ddp], 0.0)
                        pend.append((m, pair_ddp, nh_m))
                        if len(pend) > 4:
                            emit_av(*pend.popleft())
                while pend:
                    if j == 0:
                        emit_av_b(*pend.popleft())
                    else:
                        emit_av(*pend.popleft())

                # --- softmax normalization: out = av * (1/Z) (as cfg "b") ---
                rr = zpool.tile([65, HPC * TBLK], bf16, tag="rr",
                                name=f"rrr_{bb}_{j}")
                with nc.allow_low_precision(reason="bf16 1/Z broadcast"):
                    for h in range(HPC):
                        nc.vector.reciprocal(
                            rr[64:65, h * TBLK:(h + 1) * TBLK], avs[h][64:65, :])
                for h in reversed(range(HPC)):
                    bc = ps_sh.tile([HD, TBLK], f32, tag="ps_sh",
                                    name=f"bc_{bb}_{j}_{h}")
                    nc.tensor.matmul(bc[:], one1[64:65, :],
                                     rr[64:65, h * TBLK:(h + 1) * TBLK],
                                     start=True, stop=True)
                    bcs = tmpool.tile([HD, TBLK], f32, tag="bcs",
                                      name=f"bcs_{bb}_{j}_{h}")
                    nc.vector.tensor_copy(bcs[:], bc[:])
                    if h == 0:
                        nc.vector.tensor_mul(outT_sb[0:HD, tsl], avs[h][0:HD, :],
                                             bcs[:])
                    else:
                        tmp = tmpool.tile([HD, TBLK], bf16, tag="tmp",
                                          name=f"tm_{bb}_{j}")
                        nc.vector.tensor_mul(tmp[:], avs[h][0:HD, :], bcs[:])
                        if last:
                            # end-game: SBUF->SBUF DMA costs ~1.9us latency in
                            # the serial tail; shift partitions via PE instead
                            # (reuses a scores-pool tile -- the exp stream is
                            # finished by now, so no extra PSUM footprint)
                            pt = ps_s.tile([128, HPC, TBLK], f32, tag="ps_s",
                                           name=f"shf_{bb}_{j}")
                            nc.tensor.matmul(pt[:, 0, :], shf64[:], tmp[:],
                                             start=True, stop=True)
                            nc.vector.tensor_copy(
                                outT_sb[h * HD:(h + 1) * HD, tsl],
                                pt[h * HD:(h + 1) * HD, 0, :])
                        else:
                            nc.sync.dma_start(
                                out=outT_sb[h * HD:(h + 1) * HD, tsl],
                                in_=tmp[:])

                # leftover fillers run after the normalize chain is queued so
                # the recip/mult don't sit behind filler copies on DVE
                while nfill < len(fillers):
                    fillers[nfill][1]()
                    nfill += 1

            PW = min(512, d)
            NIB = d // PW

            def proj_units(bb, j, last=False):
                col0 = bb * t + j * TBLK
                NTL = TBLK // 128
                state = {}

                def piece(tl, ib):
                    if "ot" not in state:
                        state["ot"] = opool.tile([128, NTL, d], bf16, tag="ot",
                                                 name=f"ot_{bb}_{j}")
                    ot = state["ot"]
                    tt = col0 // 128 + tl
                    ps = ps_sh.tile([128, PW], f32, tag="ps_sh",
                                    name=f"psp_{bb}_{j}_{tl}_{ib}")
                    nc.tensor.matmul(ps[:], outT_sb[:, tt * 128:(tt + 1) * 128],
                                     wp_sb[:, ib * PW:(ib + 1) * PW],
                                     start=True, stop=True)
                    if last and (tl * NIB + ib) % 2 == 1:
                        nc.scalar.copy(ot[:, tl, ib * PW:(ib + 1) * PW], ps[:])
                    else:
                        nc.vector.tensor_copy(
                            ot[:, tl, ib * PW:(ib + 1) * PW], ps[:])
                    if ib == NIB - 1 and last:
                        # split the very last tile's store so the final
                        # DMA (+sem) tail is half as long
                        nsp = 2 if tl == NTL - 1 else 1
                        for sp in range(nsp):
                            csl = slice(sp * d // nsp, (sp + 1) * d // nsp)
                            nc.sync.dma_start(
                                out=out_p.rearrange(
                                    "(tb p) c -> p tb c", p=128)[
                                    :, col0 // 128 + tl:col0 // 128 + tl + 1,
                                    csl],
                                in_=ot[:, tl:tl + 1, csl])
                    if tl == NTL - 1 and ib == NIB - 1 and not last:
                        nc.sync.dma_start(
                            out=out_p.rearrange("(tb p) c -> p tb c", p=128)[
                                :, col0 // 128:col0 // 128 + NTL, :],
                            in_=ot[:])

                return [
                    (0.5, (lambda tl_, ib_: lambda: piece(tl_, ib_))(tl, ib))
                    for tl in range(NTL) for ib in range(NIB)
                ]

            def emit_proj(bb, j, last=False):
                for _, u in proj_units(bb, j, last):
                    u()

            blocks = [(bb, j) for bb in range(b) for j in range(NJ)]
            # proj spans: the j0/j1 attention spans are PE-over-budget (their
            # exp streams are short) while j2/j3 spans have ACT-paced PE
            # slack, so proj(j) is deferred into a later, slack-rich span of
            # the same batch instead of lagging exactly one block
            span_proj = {i: [] for i in range(len(blocks))}
            for bb in range(b):
                base = bb * NJ
                span_proj[base + 2].append(base + 0)
                span_proj[base + 3].extend([base + 1, base + 2])
                if bb + 1 < b:
                    span_proj[(bb + 1) * NJ + 1].append(base + 3)
            emit_xt(*blocks[0], chunked=True)
            emit_xt(*blocks[1])
            emit_qkv(*blocks[0])
            for idx, blk in enumerate(blocks):
                qk_u = (qkv_units(*blocks[idx + 1])
                        if idx + 1 < len(blocks) else [])
                pr_u = []
                for k in span_proj[idx]:
                    pr_u.extend(proj_units(*blocks[k]))
                fillers = []
                if idx + 2 < len(blocks):
                    bbn, jn = blocks[idx + 2]
                    fillers.append(
                        (0.1, lambda bbn=bbn, jn=jn: emit_xt(bbn, jn)))
                # round-robin qkv and proj pieces: qkv early enough for the
                # next block, proj (which waits on this block's outT
                # predecessor) spread across the span
                qi = pi = 0
                while qi < len(qk_u) or pi < len(pr_u):
                    if qi < len(qk_u):
                        fillers.append(qk_u[qi])
                        qi += 1
                    if pi < len(pr_u):
                        fillers.append(pr_u[pi])
                        pi += 1
                emit_attn(*blk, fillers=fillers, last=(idx == len(blocks) - 1))
            emit_proj(*blocks[-1], last=True)

    nc.compile()
    return nc


def _build(b, t, d, cfg):
    """Build + compile the per-core Bass program."""
    if cfg == "d":
        return _build_d(b, t, d)
    import concourse.tile as tile
    from concourse import bacc, mybir
    from contextlib import ExitStack

    f32 = mybir.dt.float32
    f32r = mybir.dt.float32r
    bf16 = mybir.dt.bfloat16

    rmode = cfg == "r"
    bmode = cfg == "b"
    # dtype of every matmul-feeding tile
    MMDT = f32r if rmode else (bf16 if bmode else f32)
    # dtype of the DMA'd inputs (host converts for bf16)
    INDT = bf16 if bmode else f32

    def bcst(ap):
        return ap.bitcast(f32r) if rmode else ap

    bt = b * t
    KT = d // 128            # k-tiles over the model dim
    TBLK = min(512, t)       # t-block width for scores/attn
    NJ = t // TBLK           # t-blocks per batch
    NSB = bt // 128          # 128-row s-blocks over B*T
    SPT = TBLK // 128        # s-blocks per t-block

    nc = bacc.Bacc("TRN2", target_bir_lowering=False, debug=False)

    xT = nc.dram_tensor("xT", [d, bt], INDT, kind="ExternalInput").ap()
    wq = nc.dram_tensor("wq", [d, CH], INDT, kind="ExternalInput").ap()
    wk = nc.dram_tensor("wk", [d, CH], INDT, kind="ExternalInput").ap()
    wv = nc.dram_tensor("wv", [d, CH], INDT, kind="ExternalInput").ap()
    wp = nc.dram_tensor("wp", [CH, d], INDT, kind="ExternalInput").ap()
    cident = nc.dram_tensor("cident", [128, 128], INDT, kind="ExternalInput").ap()
    cmask = nc.dram_tensor("cmask", [128, TBLK + 384], INDT, kind="ExternalInput").ap()
    cones = nc.dram_tensor("cones", [128, NSB, HPC], INDT, kind="ExternalInput").ap()
    cone1 = nc.dram_tensor("cone1", [65, HD], INDT, kind="ExternalInput").ap()
    # partials are summed across cores on the host in f64; bf16 partial
    # stores halve the output DMA traffic for ~1e-3 extra absmax-rel error
    OUTDT = bf16 if bmode else f32
    out_p = nc.dram_tensor("out_p", [bt, d], OUTDT, kind="ExternalOutput").ap()

    with tile.TileContext(nc) as tc, ExitStack() as top:
        persist = top.enter_context(tc.tile_pool(name="persist", bufs=1))

        # ---- persistent tiles ----
        qT_sb = persist.tile([128, bt], MMDT, tag="qT")
        kT_sb = persist.tile([128, bt], MMDT, tag="kT")
        # [v_h0 | 1 | pad | v_h1 | 1 | pad] per 128-row s-block
        vaug = persist.tile([128, NSB, 66 * HPC], MMDT, tag="vaug")
        outT_sb = persist.tile([128, bt], MMDT, tag="outT")
        wq_sb = persist.tile([128, KT, CH], MMDT, tag="wq")
        wk_sb = persist.tile([128, KT, CH], MMDT, tag="wk")
        wv_sb = persist.tile([128, KT, CH], MMDT, tag="wv")
        wp_sb = persist.tile([128, d], MMDT, tag="wp")
        ident = persist.tile([128, 128], MMDT, tag="ident")
        # staircase mask, shifted: maskt[p, m] = 1 iff m >= p + 384
        maskt = persist.tile([128, TBLK + 384], MMDT, tag="mask")
        one1 = persist.tile([65, HD], MMDT, tag="one1")

        # startup DMAs on the scalar HWDGE queue (fast descriptor gen; the
        # Pool SWDGE takes ~1.1us per DMA), ordered by first use: ident
        # (act-table warm + block-0 transposes), big wq/wk/wv loads, then
        # attention consts; wp (needed only by the lagging proj) last.
        # wq in two halves so the first q matmuls start ~1.4us earlier
        for w_ap, w_sb, nsplit in ((wq, wq_sb, 2), (wk, wk_sb, 1), (wv, wv_sb, 1)):
            for s in range(nsplit):
                hk = slice(s * KT // nsplit, (s + 1) * KT // nsplit)
                nc.scalar.dma_start(
                    out=w_sb[:, hk, :],
                    in_=bcst(w_ap.rearrange("(kt p) m -> p kt m", p=128)[:, hk, :]),
                )
        # preload the Exp activation table under the startup DMAs
        actwarm = persist.tile([1, 8], f32, tag="actwarm")
        nc.scalar.activation(actwarm[:], wq_sb[0:1, 0, 0:8],
                             mybir.ActivationFunctionType.Exp, scale=0.125)
        # ident only feeds the PE-transpose path (non-bf16 modes), but the
        # load stays unconditional: dropping it shifts the startup DMA
        # phasing and measures 2.6us WORSE in bmode (scheduler alignment)
        nc.scalar.dma_start(out=ident[:], in_=bcst(cident))
        nc.scalar.dma_start(out=maskt[:], in_=bcst(cmask))
        nc.scalar.dma_start(out=one1[:], in_=bcst(cone1))
        for h in range(HPC):
            nc.scalar.dma_start(
                out=vaug[:, :, 66 * h + 64:66 * h + 65],
                in_=bcst(cones[:, :, h:h + 1]),
            )
        nc.scalar.dma_start(out=wp_sb[:], in_=bcst(wp))

        # ---- merged loop: per (batch, t-block): QKV -> attention -> proj ----
        # Attention for block j of batch bb needs q columns of block j and
        # k/v columns of blocks 0..j (same batch) -- all computed by the time
        # block j's QKV is done, so one fused loop pipelines everything:
        # xT loads prefetch under attention PE work, and output stores drain
        # under the next block's compute.
        PW = min(512, d)
        NIB = d // PW
        with ExitStack() as body:
            xpool = body.enter_context(tc.tile_pool(name="xpool", bufs=4 if bmode else 3))
            vtpool = body.enter_context(tc.tile_pool(name="vtpool", bufs=2))
            npool = body.enter_context(tc.tile_pool(name="npool", bufs=24 if bmode else 5))
            zpool = body.enter_context(tc.tile_pool(name="zpool", bufs=2))
            tmpool = body.enter_context(tc.tile_pool(name="tmpool", bufs=2))
            opool = body.enter_context(tc.tile_pool(name="opool", bufs=3))
            # PSUM budget (8 banks): qkv 2 + scores 2 + av 2 + tr/proj/bc 2
            ps_qkv = body.enter_context(tc.tile_pool(name="ps_qkv", bufs=2, space="PSUM"))
            ps_s = body.enter_context(tc.tile_pool(name="ps_s", bufs=2, space="PSUM"))
            ps_av = body.enter_context(tc.tile_pool(name="ps_av", bufs=2, space="PSUM"))
            ps_tp = body.enter_context(tc.tile_pool(name="ps_tp", bufs=2, space="PSUM"))

            xt_tiles = {}

            def emit_xt(bb, j, chunked=False):
                col0 = bb * t + j * TBLK
                tsl = slice(col0, col0 + TBLK)
                xt = xpool.tile([128, KT, TBLK], MMDT, tag="xt", name=f"xt_{bb}_{j}")
                if chunked:
                    for kt in range(KT):
                        nc.sync.dma_start(
                            out=xt[:, kt, :],
                            in_=bcst(xT[kt * 128:(kt + 1) * 128, tsl]),
                        )
                else:
                    nc.sync.dma_start(
                        out=xt[:],
                        in_=bcst(xT.rearrange("(kt p) c -> p kt c", p=128)[:, :, tsl]),
                    )
                xt_tiles[(bb, j)] = xt

            def emit_qkv(bb, j):
                col0 = bb * t + j * TBLK
                tsl = slice(col0, col0 + TBLK)
                xt = xt_tiles.pop((bb, j))
                for w_sb, dst in ((wq_sb, qT_sb), (wk_sb, kT_sb)):
                    ps = ps_qkv.tile([128, TBLK], f32, tag="ps_qkv",
                                     name=f"psq_{bb}_{j}_{dst.name}")
                    for kt in range(KT):
                        nc.tensor.matmul(ps[:], w_sb[:, kt, :], xt[:, kt, :],
                                         start=(kt == 0), stop=(kt == KT - 1))
                    nc.vector.tensor_copy(dst[:, tsl], ps[:])
                if bmode:
                    # bf16 runs 1 cyc/row at width 128: compute v directly in
                    # [s, e] orientation (lhsT = x tile), skipping the PE
                    # transpose and the vt staging copy entirely
                    ps = ps_qkv.tile([128, SPT, 128], f32, tag="ps_qkv",
                                     name=f"psv_{bb}_{j}")
                    for s4 in range(SPT):
                        for kt in range(KT):
                            nc.tensor.matmul(
                                ps[:, s4, :],
                                xt[:, kt, s4 * 128:(s4 + 1) * 128],
                                wv_sb[:, kt, :],
                                start=(kt == 0), stop=(kt == KT - 1),
                                skip_group_check=True)
                    for s4 in range(SPT):
                        sb_idx = (col0 // 128) + s4
                        nc.vector.tensor_copy(
                            vaug[:, sb_idx, :].rearrange(
                                "p (g c) -> p g c", g=HPC)[:, :, 0:HD],
                            ps[:, s4, :].rearrange("p (g c) -> p g c", g=HPC),
                        )
                else:
                    ps = ps_qkv.tile([128, TBLK], f32, tag="ps_qkv", name=f"psv_{bb}_{j}")
                    for kt in range(KT):
                        nc.tensor.matmul(ps[:], wv_sb[:, kt, :], xt[:, kt, :],
                                         start=(kt == 0), stop=(kt == KT - 1))
                    vt = vtpool.tile([128, TBLK], MMDT, tag="vt", name=f"vt_{bb}_{j}")
                    nc.vector.tensor_copy(vt[:], ps[:])
                    for s4 in range(SPT):
                        sb_idx = (col0 // 128) + s4
                        pt = ps_tp.tile([128, 128], MMDT, tag="ps_tp",
                                        name=f"ptr_{bb}_{j}_{s4}")
                        nc.tensor.transpose(pt[:], vt[:, s4 * 128:(s4 + 1) * 128],
                                            ident[:])
                        nc.vector.tensor_copy(
                            vaug[:, sb_idx, :].rearrange(
                                "p (g c) -> p g c", g=HPC)[:, :, 0:HD],
                            pt[:].rearrange("p (g c) -> p g c", g=HPC),
                        )

            def emit_attn(bb, j, last=False):
                col0 = bb * t + j * TBLK
                tsl = slice(col0, col0 + TBLK)
                n_i = (j + 1) * SPT
                avs = [ps_av.tile([65, TBLK], f32, tag="ps_av", name=f"av_{bb}_{j}_{h}")
                       for h in range(HPC)]

                def emit_av(i_, ddp_, nh_):
                    for h in range(HPC):
                        sb_idx = (bb * t + i_ * 128) // 128
                        nc.tensor.matmul(
                            avs[h][:, ddp_:], vaug[:, sb_idx, h * 66:h * 66 + HD + 1],
                            nh_[:, h * TBLK + ddp_:(h + 1) * TBLK],
                            start=(i_ == 0), stop=(i_ == n_i - 1),
                            skip_group_check=True)

                from collections import deque
                pend = deque()
                for i in range(n_i):
                    ssl = slice(bb * t + i * 128, bb * t + i * 128 + 128)
                    dd = 128 * i - TBLK * j
                    # column trim: scores/exp/av touch only cols >= ddp
                    # (f32r needs free dim >= 256 for the PE fast path;
                    # bf16 runs 1 cyc/row at any width so trim fully)
                    if rmode:
                        ddp = max(0, min(dd, TBLK - 256))
                    else:
                        ddp = max(0, dd)
                    nh = npool.tile([128, HPC * TBLK], MMDT, tag="nh",
                                    name=f"nh_{bb}_{j}_{i}")
                    for h in range(HPC):
                        hp = slice(h * HD, (h + 1) * HD)
                        ps = ps_s.tile([128, TBLK], f32, tag="ps_s",
                                       name=f"pss_{bb}_{j}_{i}_{h}")
                        nc.tensor.matmul(
                            ps[:, ddp:], kT_sb[hp, ssl],
                            qT_sb[hp, tsl][:, ddp:], start=True, stop=True)
                        nc.scalar.activation(
                            nh[:, h * TBLK + ddp:(h + 1) * TBLK], ps[:, ddp:],
                            mybir.ActivationFunctionType.Exp, scale=0.125)
                        if dd >= 0:
                            # mask cols [ddp, dd+128): staircase + trim slack
                            # (nh col c maps to mask col c + 384 - dd)
                            nc.gpsimd.tensor_mul(
                                nh[:, h * TBLK + ddp:h * TBLK + dd + 128],
                                nh[:, h * TBLK + ddp:h * TBLK + dd + 128],
                                maskt[:, 384 - dd + ddp:512])
                    # attn@v lags 12 i-steps (== fully deferred for most
                    # blocks): the scores/exp stream runs uninterrupted, then
                    # the AV batch runs at full PE rate against banked nh
                    # tiles -- measured best across lag 1..16
                    pend.append((i, ddp, nh))
                    if len(pend) > 12:
                        i_, ddp_, nh_ = pend.popleft()
                        emit_av(i_, ddp_, nh_)
                while pend:
                    i_, ddp_, nh_ = pend.popleft()
                    emit_av(i_, ddp_, nh_)

                # --- softmax normalization: out = av * (1/Z) ---
                rr = zpool.tile([65, HPC * TBLK], MMDT, tag="rr",
                                name=f"rrr_{bb}_{j}")
                with nc.allow_low_precision(reason="f32r PE broadcast of 1/Z"):
                    for h in range(HPC):
                        nc.vector.reciprocal(
                            rr[64:65, h * TBLK:(h + 1) * TBLK], avs[h][64:65, :])
                # h1 first: its outT write goes through a SBUF->SBUF DMA
                # (partition shift), so start it before h0's direct DVE write
                bcs_h = {}
                for h in reversed(range(HPC)):
                    # K=1 matmul broadcasts 1/Z across the 64 output partitions
                    bc = ps_tp.tile([HD, TBLK], f32, tag="ps_tp", name=f"bc_{bb}_{j}_{h}")
                    nc.tensor.matmul(bc[:], one1[64:65, :],
                                     rr[64:65, h * TBLK:(h + 1) * TBLK],
                                     start=True, stop=True)
                    # DVE may read only one PSUM operand: stage bc in SBUF
                    # (on DVE -- ACT is loaded with the exp stream)
                    bcs = tmpool.tile([HD, TBLK], f32, tag="bcs", name=f"bcs_{bb}_{j}_{h}")
                    nc.vector.tensor_copy(bcs[:], bc[:])
                    bcs_h[h] = bcs
                    if h == 0:
                        nc.vector.tensor_mul(outT_sb[0:HD, tsl], avs[h][0:HD, :], bcs[:])
                    else:
                        tmp = tmpool.tile([HD, TBLK], MMDT, tag="tmp", name=f"tm_{bb}_{j}")
                        nc.vector.tensor_mul(tmp[:], avs[h][0:HD, :], bcs[:])
                        nc.sync.dma_start(
                            out=outT_sb[h * HD:(h + 1) * HD, tsl], in_=tmp[:])

            def emit_proj(bb, j, last=False):
                col0 = bb * t + j * TBLK
                NTL = TBLK // 128
                ot = opool.tile([128, NTL, d], OUTDT, tag="ot", name=f"ot_{bb}_{j}")
                for tl in range(NTL):
                    tt = col0 // 128 + tl
                    for ib in range(NIB):
                        # drain only: scores pool is idle, alternate pools for
                        # a 4-deep ring so the matmul stream is not copy-paced
                        pools = ((ps_tp, "ps_tp"), (ps_s, "ps_s"),
                                 (ps_qkv, "ps_qkv"))
                        pp, ptag = pools[(tl * NIB + ib) % 3] if last else pools[0]
                        ps = pp.tile([128, PW], f32, tag=ptag,
                                     name=f"psp_{bb}_{j}_{tl}_{ib}")
                        nc.tensor.matmul(ps[:], outT_sb[:, tt * 128:(tt + 1) * 128],
                                         wp_sb[:, ib * PW:(ib + 1) * PW],
                                         start=True, stop=True)
                        # in the drain there is no exp stream: split copies
                        # between DVE and ACT and store per row-block pair so
                        # the store overlaps the remaining copies
                        if last and (tl * NIB + ib) % 2 == 1:
                            nc.scalar.copy(ot[:, tl, ib * PW:(ib + 1) * PW], ps[:])
                        else:
                            nc.vector.tensor_copy(
                                ot[:, tl, ib * PW:(ib + 1) * PW], ps[:])
                    if last:
                        nc.sync.dma_start(
                            out=out_p.rearrange("(tb p) c -> p tb c", p=128)[
                                :, col0 // 128 + tl:col0 // 128 + tl + 1, :],
                            in_=ot[:, tl:tl + 1, :])
                if not last:
                    # one store DMA per block: [p, tl, d] -> row-blocks of out_p
                    nc.sync.dma_start(
                        out=out_p.rearrange("(tb p) c -> p tb c", p=128)[
                            :, col0 // 128:col0 // 128 + NTL, :],
                        in_=ot[:])

            # software pipeline: QKV runs one t-block ahead of attention, and
            # the projection lags one block behind, so block-boundary DVE/DMA
            # latencies hide under attention PE work
            blocks = [(bb, j) for bb in range(b) for j in range(NJ)]
            emit_xt(*blocks[0], chunked=True)
            emit_xt(*blocks[1])
            emit_qkv(*blocks[0])
            deferred = {2, 4, 6} if len(blocks) == 8 else set()
            for idx, blk in enumerate(blocks):
                if idx + 2 < len(blocks):
                    emit_xt(*blocks[idx + 2])
                if idx + 1 < len(blocks):
                    emit_qkv(*blocks[idx + 1])
                if idx == len(blocks) - 1:
                    for dfx in sorted(deferred):
                        emit_proj(*blocks[dfx - 1])
                emit_attn(*blk, last=(idx == len(blocks) - 1))
                if idx >= 1 and idx not in deferred:
                    emit_proj(*blocks[idx - 1], last=(idx == len(blocks) - 1))
            emit_proj(*blocks[-1], last=True)

    nc.compile()
    return nc


def _get_nc(b=B, t=T, d=D, cfg="b"):
    key = (b, t, d, cfg)
    if key not in _CACHE:
        _CACHE[key] = _build(b, t, d, cfg)
    return _CACHE[key]


def _in_dtype(cfg):
    if cfg == "b":
        import ml_dtypes
        return np.dtype(ml_dtypes.bfloat16)
    return np.dtype(np.float32)


def _make_consts(b, t, d, dt):
    bt = b * t
    TBLK = min(512, t)
    NSB = bt // 128
    cident = np.eye(128, dtype=dt)
    p = np.arange(128, dtype=np.int64)[:, None]
    m = np.arange(TBLK + 384, dtype=np.int64)[None, :]
    cmask = (m >= p + 384).astype(dt)
    cones = np.ones((128, NSB, HPC), dtype=dt)
    cone1 = np.ones((65, HD), dtype=dt)
    return {"cident": cident, "cmask": cmask, "cones": cones, "cone1": cone1}


def _hilo(a):
    import ml_dtypes
    f8 = np.dtype(ml_dtypes.float8_e4m3)
    hi = a.astype(f8)
    lo = (a.astype(np.float32) - hi.astype(np.float32)).astype(f8)
    return np.ascontiguousarray(hi), np.ascontiguousarray(lo)


def _prepare_in_maps_d(x, Wq, Wk, Wv, Wp, b, t, d):
    import ml_dtypes
    f8 = np.dtype(ml_dtypes.float8_e4m3)
    bf = np.dtype(ml_dtypes.bfloat16)
    bt = b * t
    TBLK = min(512, t)
    NSB = bt // 128
    SPT = TBLK // 128
    xT = x.reshape(bt, d).T.astype(np.float32)
    xTh, xTl = _hilo(xT)
    p = np.arange(128, dtype=np.int64)[:, None]
    m = np.arange(TBLK + 384, dtype=np.int64)[None, :]
    cmask = (m >= p + 384).astype(f8)
    cmaskb = (m >= p + 384).astype(bf)
    cones = np.zeros((128, NSB // 2, 2, 160), dtype=f8)
    cones[:, :, :, 64] = 1.0
    cones[:, :, :, 144] = 1.0
    conesb = np.zeros((128, b, SPT, 132), dtype=bf)
    conesb[:, :, :, 64] = 1.0
    conesb[:, :, :, 130] = 1.0
    cone1 = np.ones((65, HD), dtype=bf)
    cshf = np.zeros((HD, 128), dtype=bf)
    cshf[np.arange(HD), HD + np.arange(HD)] = 1.0
    in_maps = []
    for c in range(NCORES):
        h0 = c * HPC
        im = {"xTh": xTh, "xTl": xTl, "cmask": cmask, "cmaskb": cmaskb,
              "cones": cones, "conesb": conesb, "cone1": cone1, "cshf": cshf}
        for nm, W in (("wq", Wq), ("wk", Wk), ("wv", Wv)):
            w_c = W[h0:h0 + HPC].reshape(CH, d).T.astype(np.float32)
            im[nm + "h"], im[nm + "l"] = _hilo(w_c)
        im["wp"] = np.ascontiguousarray(
            Wp[:, c * CH:(c + 1) * CH].T.astype(bf))
        in_maps.append(im)
    return in_maps


def _prepare_in_maps(x, Wq, Wk, Wv, Wp, b, t, d, cfg):
    if cfg == "d":
        return _prepare_in_maps_d(x, Wq, Wk, Wv, Wp, b, t, d)
    bt = b * t
    dt = _in_dtype(cfg)
    xT = np.ascontiguousarray(x.reshape(bt, d).T.astype(dt))
    consts = _make_consts(b, t, d, dt)
    in_maps = []
    for c in range(NCORES):
        h0 = c * HPC
        wq_c = np.ascontiguousarray(Wq[h0:h0 + HPC].reshape(CH, d).T.astype(dt))
        wk_c = np.ascontiguousarray(Wk[h0:h0 + HPC].reshape(CH, d).T.astype(dt))
        wv_c = np.ascontiguousarray(Wv[h0:h0 + HPC].reshape(CH, d).T.astype(dt))
        wp_c = np.ascontiguousarray(Wp[:, c * CH:(c + 1) * CH].T.astype(dt))
        in_maps.append({"xT": xT, "wq": wq_c, "wk": wk_c, "wv": wv_c, "wp": wp_c,
                        **consts})
    return in_maps


def _run(x, Wq, Wk, Wv, Wp, bp, b, t, d, cfg, trace=False):
    from concourse.bass_utils import run_bass_kernel_spmd
    nc = _get_nc(b, t, d, cfg)
    in_maps = _prepare_in_maps(x, Wq, Wk, Wv, Wp, b, t, d, cfg)
    res = run_bass_kernel_spmd(nc, in_maps, core_ids=list(range(NCORES)), trace=trace)
    acc = np.zeros((b * t, d), dtype=np.float64)
    for r in res.results:
        acc += r["out_p"].astype(np.float64)
    out = (acc + np.asarray(bp, dtype=np.float64)).astype(np.float32)
    return out.reshape(b, t, d), res


KERNEL_CFG = "d"


def kernel(x, Wq, Wk, Wv, Wp, bp):
    out, _ = _run(np.asarray(x), np.asarray(Wq), np.asarray(Wk), np.asarray(Wv),
                  np.asarray(Wp), np.asarray(bp), B, T, D, KERNEL_CFG, trace=False)
    return out



# revision 86
# speedup vs baseline: 1.1366x; 1.0092x over previous
"""Multi-head attention Trainium2 kernel (8 NeuronCores, tensor-parallel over heads).

Sharding: 16 heads / 8 cores = 2 heads per core. x is replicated; Wq/Wk/Wv
sharded by head; Wp row-sharded (contraction dim). Each core computes a
partial projection output [B*T, D] stored bf16; the host sums the 8 partials
in f64 (+bias).

Default cfg "d" (fp8 DoubleRow; ~135us sim vs 147us for the bf16 cfg "b",
absmax-relative error ~3.6e-3 vs gate 2e-2):
  - QKV: x and W are split hi+lo fp8e4m3 on the host; W is pre-scaled x32 so
    both hi and the residual stay in fp8's NORMAL range (raw W~0.02 values
    are subnormal in e4m3, which destroys the residual trick). Contraction
    runs as 3 DoubleRow cross terms (Wh*xh + Wl*xh + Wh*xl), each packing
    K=256 at 0.5 cyc/row: 12 matmuls of N/2 vs bf16's 8 of N. The x32 is
    compensated for free: scores come out x1024 and the exp scale constant
    becomes 0.125/1024; v comes out x32 and the broadcast-ones constant used
    to expand 1/Z is 1/32.
  - Scores stay bf16 [s on partitions, t free]; both heads' scores land in
    one 2-bank PSUM tile so exp is ONE activation per s-block (halves the
    ACT per-op overhead; the exp stream is the attention-phase critical
    path). exp writes fp8 nh directly.
  - attn@v runs DoubleRow over PAIRS of s-blocks (contraction 256, cost
    width/2 per pair, 4x denser than bf16): lhsT = packed [v32|1] planes,
    rhs = nh pair planes; out rows 0..63 = unnormalized out^T x32, row 64 =
    Z. Plane-1 columns of diagonal pairs that sit above that plane's
    diagonal are memset on Pool. j=0 blocks (rows t<512, few softmax terms,
    largest weights -> fp8 noise would dominate absmax) use a bf16 attn@v
    path instead (vaug_b/masktb).
  - All vaug images are FULLY preloaded from host constants: reading
    uninitialized SBUF as fp8 can yield NaN (0xFF) on the first call.
  - Causality: lower-triangular [128s x 512t] blocks only; diagonal
    straddlers are column-trimmed and masked multiplicatively on Pool.
    Softmax runs without max subtraction (logits are O(1)).
  - Scheduling: the attention i-loop is ACT(exp)-paced, so qkv(j+1),
    proj(j-1) and x-tile prefetches are emitted as weighted "filler" pieces
    spread through it (the PE runs strictly in-order; contiguous chunks
    would starve either PE or ACT). PSUM: scores 2x2 banks + av 2 + shared
    qkv/proj/bc ring 2. The last block's h1 outT shift goes through a PE
    shift-matrix matmul instead of the ~1.9us SBUF->SBUF DMA.
cfg "b": all-bf16 fallback (the previous baseline), cfg "r": float32r.
"""

import numpy as np

B, T, D, H, HD = 2, 2048, 1024, 16, 64
NCORES = 8
HPC = H // NCORES          # heads per core = 2
CH = HPC * HD              # channels per core = 128
BT = B * T

_CACHE = {}


def _build_d(b, t, d):
    """cfg "d": fp8 DoubleRow build.

    - QKV: x and Wq/Wk/Wv are split hi/lo fp8e4m3 on the host (x = xh + xl
      exactly captures x to ~0.2%); contraction runs as 3 DoubleRow cross
      terms (Wh*xh + Wh*xl + Wl*xh) of K=256 each -> 12 matmuls of
      cost N/2 instead of 8 of cost N (sim model: fp8 DoubleRow is 0.5
      cycles/row with 2 K-planes packed per matmul).
    - Scores stay bf16 [s, t]; exp is ONE activation per s-block covering
      both heads (scores psum is a 2-bank [128, 2, TBLK] tile).
    - exp output is fp8 directly; attn@v runs DoubleRow over PAIRS of
      s-blocks (contraction 256): cost width/2 per pair instead of
      2*width. vaug holds fp8 v planes [128, NSB/2, 2, 160] with the
      softmax-denominator ones column at 64/144 per head.
    - Diagonal pairs: plane-1 columns below its own diagonal trim but
      inside the pair's matmul slice are memset to 0 on Pool.
    - Normalization/proj unchanged from cfg "b" (bf16).
    """
    import concourse.tile as tile
    from concourse import bacc, mybir
    from contextlib import ExitStack
    from collections import deque

    f32 = mybir.dt.float32
    bf16 = mybir.dt.bfloat16
    f8 = mybir.dt.float8e4
    DR = mybir.MatmulPerfMode.DoubleRow

    bt = b * t
    KT = d // 128
    KP = KT // 2             # DoubleRow k-pair steps
    TBLK = min(512, t)
    NJ = t // TBLK
    NSB = bt // 128
    SPT = TBLK // 128

    nc = bacc.Bacc("TRN2", target_bir_lowering=False, debug=False)

    xTh = nc.dram_tensor("xTh", [d, bt], f8, kind="ExternalInput").ap()
    xTl = nc.dram_tensor("xTl", [d, bt], f8, kind="ExternalInput").ap()
    w_in = {}
    for nm in ("wqh", "wql", "wkh", "wkl", "wvh", "wvl"):
        w_in[nm] = nc.dram_tensor(nm, [d, CH], f8, kind="ExternalInput").ap()
    wp = nc.dram_tensor("wp", [CH, d], bf16, kind="ExternalInput").ap()
    cmask = nc.dram_tensor("cmask", [128, TBLK + 384], f8, kind="ExternalInput").ap()
    cmaskb = nc.dram_tensor("cmaskb", [128, TBLK + 384], bf16,
                            kind="ExternalInput").ap()
    # FULL vaug images (ones columns + zero padding): loading the whole tile
    # avoids any read of uninitialized SBUF (fp8 garbage can be NaN) on the
    # first call
    cones = nc.dram_tensor("cones", [128, NSB // 2, 2, 160], f8,
                           kind="ExternalInput").ap()
    conesb = nc.dram_tensor("conesb", [128, b, SPT, 132], bf16,
                            kind="ExternalInput").ap()
    cone1 = nc.dram_tensor("cone1", [65, HD], bf16, kind="ExternalInput").ap()
    cshf = nc.dram_tensor("cshf", [HD, 128], bf16, kind="ExternalInput").ap()
    out_p = nc.dram_tensor("out_p", [bt, d], bf16, kind="ExternalOutput").ap()

    with tile.TileContext(nc) as tc, ExitStack() as top:
        persist = top.enter_context(tc.tile_pool(name="persist", bufs=1))

        qT_sb = persist.tile([128, bt], bf16, tag="qT")
        kT_sb = persist.tile([128, bt], bf16, tag="kT")
        # v planes: per (pair, plane): [v_h0 | 1 | pad @80 | v_h1 | 1 | pad]
        vaug = persist.tile([128, NSB // 2, 2, 160], f8, tag="vaug")
        # bf16 v for the first 4 s-blocks of each batch: the j=0 attention
        # blocks (rows t<512, where softmax weights are largest and fp8
        # noise dominates the absmax error) run a bf16 attn@v path
        vaug_b = persist.tile([128, b, SPT, 132], bf16, tag="vaug_b")
        outT_sb = persist.tile([128, bt], bf16, tag="outT")
        w_sb = {}
        for nm in ("wqh", "wql", "wkh", "wkl", "wvh", "wvl"):
            w_sb[nm] = persist.tile([128, KP, 2, CH], f8, tag=nm, name=nm)
        wp_sb = persist.tile([128, d], bf16, tag="wp")
        maskt = persist.tile([128, TBLK + 384], f8, tag="mask")
        masktb = persist.tile([128, TBLK + 384], bf16, tag="maskb")
        one1 = persist.tile([65, HD], bf16, tag="one1")
        # shift matrix: shf64[p, 64+p] = 1 moves rows 0..63 -> 64..127 via PE
        shf64 = persist.tile([HD, 128], bf16, tag="shf64")

        # startup: only the first-needed wq halves ride the shared HWDGE
        # (which the x-tile stream also needs); everything else goes through
        # the Pool SWDGE queue so it doesn't stall the critical path
        def wload(nm, eng, nsplit=1):
            for s in range(nsplit):
                hk = slice(s * KP // nsplit, (s + 1) * KP // nsplit)
                eng.dma_start(
                    out=w_sb[nm][:, hk, :, :],
                    in_=w_in[nm].rearrange(
                        "(kp pl p) m -> p kp pl m", p=128, pl=2)[:, hk, :, :],
                )
        wload("wqh", nc.scalar, 1)
        # warm the Exp act table under the startup DMAs
        actwarm = persist.tile([1, 8], f32, tag="actwarm")
        nc.scalar.activation(actwarm[:], w_sb["wqh"][0:1, 0, 0, 0:8],
                             mybir.ActivationFunctionType.Exp, scale=0.125)
        wload("wkh", nc.scalar)
        wload("wql", nc.scalar)
        wload("wkl", nc.scalar)
        wload("wvh", nc.scalar)
        wload("wvl", nc.scalar)
        # the full vaug images must land before the first v copies (~7us);
        # masktb before the first attention block's masks (~10us)
        nc.scalar.dma_start(out=vaug[:], in_=cones)
        nc.scalar.dma_start(out=vaug_b[:], in_=conesb)
        nc.scalar.dma_start(out=masktb[:], in_=cmaskb)
        nc.scalar.dma_start(out=maskt[:], in_=cmask)
        nc.scalar.dma_start(out=one1[:], in_=cone1)
        nc.scalar.dma_start(out=shf64[:], in_=cshf)
        nc.scalar.dma_start(out=wp_sb[:], in_=wp)

        with ExitStack() as body:
            xpool = body.enter_context(tc.tile_pool(name="xpool", bufs=4))
            npool = body.enter_context(tc.tile_pool(name="npool", bufs=9))
            npool_b = body.enter_context(tc.tile_pool(name="npool_b", bufs=3))
            zpool = body.enter_context(tc.tile_pool(name="zpool", bufs=2))
            tmpool = body.enter_context(tc.tile_pool(name="tmpool", bufs=2))
            opool = body.enter_context(tc.tile_pool(name="opool", bufs=3))
            # PSUM (8 banks): scores 2x2 + av 2 + shared(qkv/proj/bc) 2
            ps_s = body.enter_context(tc.tile_pool(name="ps_s", bufs=2, space="PSUM"))
            ps_av = body.enter_context(tc.tile_pool(name="ps_av", bufs=2, space="PSUM"))
            ps_sh = body.enter_context(tc.tile_pool(name="ps_sh", bufs=2, space="PSUM"))

            xt_tiles = {}

            def emit_xt(bb, j, chunked=False):
                col0 = bb * t + j * TBLK
                tsl = slice(col0, col0 + TBLK)
                srcs = (("xh", xTh), ("xl", xTl))
                pair = [xpool.tile([128, KT, TBLK], f8, tag=nm,
                                   name=f"{nm}_{bb}_{j}")
                        for nm, _ in srcs]
                if chunked:
                    # half-tile pieces, hi/lo interleaved, so the first matmul
                    # group's inputs land as early as possible
                    for kh in range(2):
                        for xt, (nm, src) in zip(pair, srcs):
                            nc.sync.dma_start(
                                out=xt[:, 4 * kh:4 * kh + 4, :],
                                in_=src.rearrange(
                                    "(kt p) c -> p kt c",
                                    p=128)[:, 4 * kh:4 * kh + 4, tsl])
                else:
                    for xt, (nm, src) in zip(pair, srcs):
                        nc.sync.dma_start(
                            out=xt[:],
                            in_=src.rearrange("(kt p) c -> p kt c", p=128)[:, :, tsl])
                xt_tiles[(bb, j)] = pair

            def qkv_units(bb, j):
                """One block's QKV as weighted filler pieces (weight ~= us of
                PE work, used to spread emission across the attention i-loop
                without starving the ACT exp stream)."""
                col0 = bb * t + j * TBLK
                tsl = slice(col0, col0 + TBLK)
                state = {}
                # x-residual term order: hi*hi, lo_w*hi, hi*lo_x -- the lo x
                # tile is only needed by the last 4 matmuls of each group
                QTERMS = (("h", "xh"), ("l", "xh"), ("h", "xl"))

                def qk_mm(wt, dst, ti):
                    xts = dict(zip(("xh", "xl"), xt_tiles[(bb, j)]))
                    key = "ps" + wt
                    if ti == 0:
                        state[key] = ps_sh.tile([128, TBLK], f32, tag="ps_sh",
                                                name=f"psq_{bb}_{j}_{dst.name}")
                    ps = state[key]
                    ws, xn = QTERMS[ti]
                    for kp in range(KP):
                        nc.tensor.matmul(
                            ps[:], w_sb[wt + ws][:, kp, :, :],
                            xts[xn][:, 2 * kp:2 * kp + 2, :],
                            start=(ti == 0 and kp == 0),
                            stop=(ti == 2 and kp == KP - 1),
                            perf_mode=DR)
                    if ti == 2:
                        nc.vector.tensor_copy(dst[:, tsl], ps[:])

                def v_mm(s4):
                    xth, xtl = xt_tiles[(bb, j)]
                    if "vps" not in state:
                        state["vps"] = ps_sh.tile([128, SPT, 128], f32,
                                                  tag="ps_sh",
                                                  name=f"psv_{bb}_{j}")
                    ps = state["vps"]
                    idx = 0
                    for xh, wn in ((xth, "wvh"), (xth, "wvl"), (xtl, "wvh")):
                        for kp in range(KP):
                            nc.tensor.matmul(
                                ps[:, s4, :],
                                xh[:, 2 * kp:2 * kp + 2,
                                   s4 * 128:(s4 + 1) * 128],
                                w_sb[wn][:, kp, :, :],
                                start=(idx == 0), stop=(idx == 3 * KP - 1),
                                perf_mode=DR, skip_group_check=True)
                            idx += 1
                    sb_idx = (col0 // 128) + s4
                    m, pl = divmod(sb_idx, 2)
                    nc.vector.tensor_copy(
                        vaug[:, m, pl, :].rearrange(
                            "p (g c) -> p g c", g=2)[:, :, 0:HD],
                        ps[:, s4, :].rearrange("p (g c) -> p g c", g=HPC),
                    )
                    if j == 0:
                        # bf16 copy for the j=0 attention path
                        nc.vector.tensor_copy(
                            vaug_b[:, bb, s4, :].rearrange(
                                "p (g c) -> p g c", g=2)[:, :, 0:HD],
                            ps[:, s4, :].rearrange("p (g c) -> p g c", g=HPC),
                        )
                    if s4 == SPT - 1:
                        xt_tiles.pop((bb, j))

                units = []
                for wt, dst in (("wq", qT_sb), ("wk", kT_sb)):
                    for ti in range(3):
                        units.append((0.45, (lambda wt_=wt, dst_=dst, ti_=ti:
                                             qk_mm(wt_, dst_, ti_))))
                for s4 in range(SPT):
                    units.append((0.9, (lambda s4_=s4: v_mm(s4_))))
                return units

            def emit_qkv(bb, j):
                for _, u in qkv_units(bb, j):
                    u()

            def emit_attn(bb, j, fillers=(), last=False):
                col0 = bb * t + j * TBLK
                tsl = slice(col0, col0 + TBLK)
                n_i = (j + 1) * SPT
                n_pairs = n_i // 2
                avs = [ps_av.tile([65, TBLK], f32, tag="ps_av",
                                  name=f"av_{bb}_{j}_{h}")
                       for h in range(HPC)]

                def emit_av(m_, ddp_, nh_):
                    sbp = bb * (t // 256) + m_
                    for h in range(HPC):
                        nc.tensor.matmul(
                            avs[h][:, ddp_:],
                            vaug[:, sbp, :, 80 * h:80 * h + HD + 1],
                            nh_[:, :, h, ddp_:],
                            start=(m_ == 0), stop=(m_ == n_pairs - 1),
                            perf_mode=DR, skip_group_check=True)

                def emit_av_b(i_, ddp_, nh_, pl_):
                    # bf16 path (j=0 blocks): per-s-block standard matmul
                    for h in range(HPC):
                        nc.tensor.matmul(
                            avs[h][:, ddp_:],
                            vaug_b[:, bb, i_, 66 * h:66 * h + HD + 1],
                            nh_[:, pl_, h, ddp_:],
                            start=(i_ == 0), stop=(i_ == n_i - 1),
                            skip_group_check=True)

                fillers = list(fillers)
                wtotal = sum(w for w, _ in fillers) or 1.0
                nfill = 0
                wdone = 0.0

                pend = deque()
                nh_m = None
                pair_ddp = 0
                for i in range(n_i):
                    # spread deferred qkv/proj/xt work through the i-loop by
                    # cumulative PE-work weight, so the PE has in-order work
                    # during exp waits without long bursts that starve ACT
                    # finish fillers ~2 i-steps early so the DVE queue is
                    # drained when the block-end normalize chain needs it
                    want = (i / max(1, n_i - 2)) * wtotal
                    while nfill < len(fillers) and wdone < want:
                        w, fn = fillers[nfill]
                        fn()
                        wdone += w
                        nfill += 1
                    ssl = slice(bb * t + i * 128, bb * t + i * 128 + 128)
                    dd = 128 * i - TBLK * j
                    ddp = max(0, dd)
                    m, pl = divmod(i, 2)
                    nhdt = bf16 if j == 0 else f8
                    nhmask = masktb if j == 0 else maskt
                    if pl == 0:
                        np_ = npool_b if j == 0 else npool
                        nh_m = np_.tile([128, 2, HPC, TBLK], nhdt,
                                        tag="nhb" if j == 0 else "nh",
                                        name=f"nh_{bb}_{j}_{m}")
                        pair_ddp = ddp
                    ps = ps_s.tile([128, HPC, TBLK], f32, tag="ps_s",
                                   name=f"pss_{bb}_{j}_{i}")
                    for h in range(HPC):
                        hp = slice(h * HD, (h + 1) * HD)
                        nc.tensor.matmul(
                            ps[:, h, ddp:], kT_sb[hp, ssl],
                            qT_sb[hp, tsl][:, ddp:], start=True, stop=True)
                    nc.scalar.activation(
                        nh_m[:, pl, :, ddp:], ps[:, :, ddp:],
                        mybir.ActivationFunctionType.Exp, scale=0.125)
                    if dd >= 0:
                        for h in range(HPC):
                            nc.gpsimd.tensor_mul(
                                nh_m[:, pl, h, ddp:dd + 128],
                                nh_m[:, pl, h, ddp:dd + 128],
                                nhmask[:, 384 - dd + ddp:512])
                    if j == 0:
                        pend.append((i, ddp, nh_m, pl))
                        if len(pend) > 2:
                            emit_av_b(*pend.popleft())
                    elif pl == 1:
                        if ddp > pair_ddp:
                            # plane-1 cols [pair_ddp, ddp) are inside the AV
                            # slice but above this plane's diagonal: zero them
                            nc.gpsimd.memset(nh_m[:, 1, :, pair_ddp:ddp], 0.0)
                        pend.append((m, pair_ddp, nh_m))
                        if len(pend) > 4:
                            emit_av(*pend.popleft())
                while pend:
                    if j == 0:
                        emit_av_b(*pend.popleft())
                    else:
                        emit_av(*pend.popleft())

                # --- softmax normalization: out = av * (1/Z) (as cfg "b") ---
                rr = zpool.tile([65, HPC * TBLK], bf16, tag="rr",
                                name=f"rrr_{bb}_{j}")
                for h in range(HPC):
                    with nc.allow_low_precision(reason="bf16 1/Z broadcast"):
                        nc.vector.reciprocal(
                            rr[64:65, h * TBLK:(h + 1) * TBLK], avs[h][64:65, :])
                    bc = ps_sh.tile([HD, TBLK], f32, tag="ps_sh",
                                    name=f"bc_{bb}_{j}_{h}")
                    nc.tensor.matmul(bc[:], one1[64:65, :],
                                     rr[64:65, h * TBLK:(h + 1) * TBLK],
                                     start=True, stop=True)
                    bcs = tmpool.tile([HD, TBLK], f32, tag="bcs",
                                      name=f"bcs_{bb}_{j}_{h}")
                    nc.vector.tensor_copy(bcs[:], bc[:])
                    if h == 0:
                        nc.vector.tensor_mul(outT_sb[0:HD, tsl], avs[h][0:HD, :],
                                             bcs[:])
                    else:
                        tmp = tmpool.tile([HD, TBLK], bf16, tag="tmp",
                                          name=f"tm_{bb}_{j}")
                        nc.vector.tensor_mul(tmp[:], avs[h][0:HD, :], bcs[:])
                        if last:
                            # end-game: SBUF->SBUF DMA costs ~1.9us latency in
                            # the serial tail; shift partitions via PE instead
                            # (reuses a scores-pool tile -- the exp stream is
                            # finished by now, so no extra PSUM footprint)
                            pt = ps_s.tile([128, HPC, TBLK], f32, tag="ps_s",
                                           name=f"shf_{bb}_{j}")
                            nc.tensor.matmul(pt[:, 0, :], shf64[:], tmp[:],
                                             start=True, stop=True)
                            nc.vector.tensor_copy(
                                outT_sb[h * HD:(h + 1) * HD, tsl],
                                pt[h * HD:(h + 1) * HD, 0, :])
                        else:
                            nc.sync.dma_start(
                                out=outT_sb[h * HD:(h + 1) * HD, tsl],
                                in_=tmp[:])

                # leftover fillers run after the normalize chain is queued so
                # the recip/mult don't sit behind filler copies on DVE
                while nfill < len(fillers):
                    fillers[nfill][1]()
                    nfill += 1

            PW = min(512, d)
            NIB = d // PW

            def proj_units(bb, j, last=False):
                col0 = bb * t + j * TBLK
                NTL = TBLK // 128
                state = {}

                def piece(tl, ib):
                    if "ot" not in state:
                        state["ot"] = opool.tile([128, NTL, d], bf16, tag="ot",
                                                 name=f"ot_{bb}_{j}")
                    ot = state["ot"]
                    tt = col0 // 128 + tl
                    ps = ps_sh.tile([128, PW], f32, tag="ps_sh",
                                    name=f"psp_{bb}_{j}_{tl}_{ib}")
                    nc.tensor.matmul(ps[:], outT_sb[:, tt * 128:(tt + 1) * 128],
                                     wp_sb[:, ib * PW:(ib + 1) * PW],
                                     start=True, stop=True)
                    if last and (tl * NIB + ib) % 2 == 1:
                        nc.scalar.copy(ot[:, tl, ib * PW:(ib + 1) * PW], ps[:])
                    else:
                        nc.vector.tensor_copy(
                            ot[:, tl, ib * PW:(ib + 1) * PW], ps[:])
                    if ib == NIB - 1 and last:
                        # split the very last tile's store so the final
                        # DMA (+sem) tail is half as long
                        nsp = 2 if tl == NTL - 1 else 1
                        for sp in range(nsp):
                            csl = slice(sp * d // nsp, (sp + 1) * d // nsp)
                            nc.sync.dma_start(
                                out=out_p.rearrange(
                                    "(tb p) c -> p tb c", p=128)[
                                    :, col0 // 128 + tl:col0 // 128 + tl + 1,
                                    csl],
                                in_=ot[:, tl:tl + 1, csl])
                    if tl == NTL - 1 and ib == NIB - 1 and not last:
                        nc.sync.dma_start(
                            out=out_p.rearrange("(tb p) c -> p tb c", p=128)[
                                :, col0 // 128:col0 // 128 + NTL, :],
                            in_=ot[:])

                return [
                    (0.5, (lambda tl_, ib_: lambda: piece(tl_, ib_))(tl, ib))
                    for tl in range(NTL) for ib in range(NIB)
                ]

            def emit_proj(bb, j, last=False):
                for _, u in proj_units(bb, j, last):
                    u()

            blocks = [(bb, j) for bb in range(b) for j in range(NJ)]
            # proj spans: the j0/j1 attention spans are PE-over-budget (their
            # exp streams are short) while j2/j3 spans have ACT-paced PE
            # slack, so proj(j) is deferred into a later, slack-rich span of
            # the same batch instead of lagging exactly one block
            span_proj = {i: [] for i in range(len(blocks))}
            for bb in range(b):
                base = bb * NJ
                span_proj[base + 2].append(base + 0)
                span_proj[base + 3].extend([base + 1, base + 2])
                if bb + 1 < b:
                    span_proj[(bb + 1) * NJ + 1].append(base + 3)
            emit_xt(*blocks[0], chunked=True)
            emit_xt(*blocks[1])
            emit_qkv(*blocks[0])
            for idx, blk in enumerate(blocks):
                qk_u = (qkv_units(*blocks[idx + 1])
                        if idx + 1 < len(blocks) else [])
                pr_u = []
                for k in span_proj[idx]:
                    pr_u.extend(proj_units(*blocks[k]))
                fillers = []
                if idx + 2 < len(blocks):
                    bbn, jn = blocks[idx + 2]
                    fillers.append(
                        (0.1, lambda bbn=bbn, jn=jn: emit_xt(bbn, jn)))
                # round-robin qkv and proj pieces: qkv early enough for the
                # next block, proj (which waits on this block's outT
                # predecessor) spread across the span
                qi = pi = 0
                while qi < len(qk_u) or pi < len(pr_u):
                    if qi < len(qk_u):
                        fillers.append(qk_u[qi])
                        qi += 1
                    if pi < len(pr_u):
                        fillers.append(pr_u[pi])
                        pi += 1
                emit_attn(*blk, fillers=fillers, last=(idx == len(blocks) - 1))
            emit_proj(*blocks[-1], last=True)

    nc.compile()
    return nc


def _build(b, t, d, cfg):
    """Build + compile the per-core Bass program."""
    if cfg == "d":
        return _build_d(b, t, d)
    import concourse.tile as tile
    from concourse import bacc, mybir
    from contextlib import ExitStack

    f32 = mybir.dt.float32
    f32r = mybir.dt.float32r
    bf16 = mybir.dt.bfloat16

    rmode = cfg == "r"
    bmode = cfg == "b"
    # dtype of every matmul-feeding tile
    MMDT = f32r if rmode else (bf16 if bmode else f32)
    # dtype of the DMA'd inputs (host converts for bf16)
    INDT = bf16 if bmode else f32

    def bcst(ap):
        return ap.bitcast(f32r) if rmode else ap

    bt = b * t
    KT = d // 128            # k-tiles over the model dim
    TBLK = min(512, t)       # t-block width for scores/attn
    NJ = t // TBLK           # t-blocks per batch
    NSB = bt // 128          # 128-row s-blocks over B*T
    SPT = TBLK // 128        # s-blocks per t-block

    nc = bacc.Bacc("TRN2", target_bir_lowering=False, debug=False)

    xT = nc.dram_tensor("xT", [d, bt], INDT, kind="ExternalInput").ap()
    wq = nc.dram_tensor("wq", [d, CH], INDT, kind="ExternalInput").ap()
    wk = nc.dram_tensor("wk", [d, CH], INDT, kind="ExternalInput").ap()
    wv = nc.dram_tensor("wv", [d, CH], INDT, kind="ExternalInput").ap()
    wp = nc.dram_tensor("wp", [CH, d], INDT, kind="ExternalInput").ap()
    cident = nc.dram_tensor("cident", [128, 128], INDT, kind="ExternalInput").ap()
    cmask = nc.dram_tensor("cmask", [128, TBLK + 384], INDT, kind="ExternalInput").ap()
    cones = nc.dram_tensor("cones", [128, NSB, HPC], INDT, kind="ExternalInput").ap()
    cone1 = nc.dram_tensor("cone1", [65, HD], INDT, kind="ExternalInput").ap()
    # partials are summed across cores on the host in f64; bf16 partial
    # stores halve the output DMA traffic for ~1e-3 extra absmax-rel error
    OUTDT = bf16 if bmode else f32
    out_p = nc.dram_tensor("out_p", [bt, d], OUTDT, kind="ExternalOutput").ap()

    with tile.TileContext(nc) as tc, ExitStack() as top:
        persist = top.enter_context(tc.tile_pool(name="persist", bufs=1))

        # ---- persistent tiles ----
        qT_sb = persist.tile([128, bt], MMDT, tag="qT")
        kT_sb = persist.tile([128, bt], MMDT, tag="kT")
        # [v_h0 | 1 | pad | v_h1 | 1 | pad] per 128-row s-block
        vaug = persist.tile([128, NSB, 66 * HPC], MMDT, tag="vaug")
        outT_sb = persist.tile([128, bt], MMDT, tag="outT")
        wq_sb = persist.tile([128, KT, CH], MMDT, tag="wq")
        wk_sb = persist.tile([128, KT, CH], MMDT, tag="wk")
        wv_sb = persist.tile([128, KT, CH], MMDT, tag="wv")
        wp_sb = persist.tile([128, d], MMDT, tag="wp")
        ident = persist.tile([128, 128], MMDT, tag="ident")
        # staircase mask, shifted: maskt[p, m] = 1 iff m >= p + 384
        maskt = persist.tile([128, TBLK + 384], MMDT, tag="mask")
        one1 = persist.tile([65, HD], MMDT, tag="one1")

        # startup DMAs on the scalar HWDGE queue (fast descriptor gen; the
        # Pool SWDGE takes ~1.1us per DMA), ordered by first use: ident
        # (act-table warm + block-0 transposes), big wq/wk/wv loads, then
        # attention consts; wp (needed only by the lagging proj) last.
        # wq in two halves so the first q matmuls start ~1.4us earlier
        for w_ap, w_sb, nsplit in ((wq, wq_sb, 2), (wk, wk_sb, 1), (wv, wv_sb, 1)):
            for s in range(nsplit):
                hk = slice(s * KT // nsplit, (s + 1) * KT // nsplit)
                nc.scalar.dma_start(
                    out=w_sb[:, hk, :],
                    in_=bcst(w_ap.rearrange("(kt p) m -> p kt m", p=128)[:, hk, :]),
                )
        # preload the Exp activation table under the startup DMAs
        actwarm = persist.tile([1, 8], f32, tag="actwarm")
        nc.scalar.activation(actwarm[:], wq_sb[0:1, 0, 0:8],
                             mybir.ActivationFunctionType.Exp, scale=0.125)
        # ident only feeds the PE-transpose path (non-bf16 modes), but the
        # load stays unconditional: dropping it shifts the startup DMA
        # phasing and measures 2.6us WORSE in bmode (scheduler alignment)
        nc.scalar.dma_start(out=ident[:], in_=bcst(cident))
        nc.scalar.dma_start(out=maskt[:], in_=bcst(cmask))
        nc.scalar.dma_start(out=one1[:], in_=bcst(cone1))
        for h in range(HPC):
            nc.scalar.dma_start(
                out=vaug[:, :, 66 * h + 64:66 * h + 65],
                in_=bcst(cones[:, :, h:h + 1]),
            )
        nc.scalar.dma_start(out=wp_sb[:], in_=bcst(wp))

        # ---- merged loop: per (batch, t-block): QKV -> attention -> proj ----
        # Attention for block j of batch bb needs q columns of block j and
        # k/v columns of blocks 0..j (same batch) -- all computed by the time
        # block j's QKV is done, so one fused loop pipelines everything:
        # xT loads prefetch under attention PE work, and output stores drain
        # under the next block's compute.
        PW = min(512, d)
        NIB = d // PW
        with ExitStack() as body:
            xpool = body.enter_context(tc.tile_pool(name="xpool", bufs=4 if bmode else 3))
            vtpool = body.enter_context(tc.tile_pool(name="vtpool", bufs=2))
            npool = body.enter_context(tc.tile_pool(name="npool", bufs=24 if bmode else 5))
            zpool = body.enter_context(tc.tile_pool(name="zpool", bufs=2))
            tmpool = body.enter_context(tc.tile_pool(name="tmpool", bufs=2))
            opool = body.enter_context(tc.tile_pool(name="opool", bufs=3))
            # PSUM budget (8 banks): qkv 2 + scores 2 + av 2 + tr/proj/bc 2
            ps_qkv = body.enter_context(tc.tile_pool(name="ps_qkv", bufs=2, space="PSUM"))
            ps_s = body.enter_context(tc.tile_pool(name="ps_s", bufs=2, space="PSUM"))
            ps_av = body.enter_context(tc.tile_pool(name="ps_av", bufs=2, space="PSUM"))
            ps_tp = body.enter_context(tc.tile_pool(name="ps_tp", bufs=2, space="PSUM"))

            xt_tiles = {}

            def emit_xt(bb, j, chunked=False):
                col0 = bb * t + j * TBLK
                tsl = slice(col0, col0 + TBLK)
                xt = xpool.tile([128, KT, TBLK], MMDT, tag="xt", name=f"xt_{bb}_{j}")
                if chunked:
                    for kt in range(KT):
                        nc.sync.dma_start(
                            out=xt[:, kt, :],
                            in_=bcst(xT[kt * 128:(kt + 1) * 128, tsl]),
                        )
                else:
                    nc.sync.dma_start(
                        out=xt[:],
                        in_=bcst(xT.rearrange("(kt p) c -> p kt c", p=128)[:, :, tsl]),
                    )
                xt_tiles[(bb, j)] = xt

            def emit_qkv(bb, j):
                col0 = bb * t + j * TBLK
                tsl = slice(col0, col0 + TBLK)
                xt = xt_tiles.pop((bb, j))
                for w_sb, dst in ((wq_sb, qT_sb), (wk_sb, kT_sb)):
                    ps = ps_qkv.tile([128, TBLK], f32, tag="ps_qkv",
                                     name=f"psq_{bb}_{j}_{dst.name}")
                    for kt in range(KT):
                        nc.tensor.matmul(ps[:], w_sb[:, kt, :], xt[:, kt, :],
                                         start=(kt == 0), stop=(kt == KT - 1))
                    nc.vector.tensor_copy(dst[:, tsl], ps[:])
                if bmode:
                    # bf16 runs 1 cyc/row at width 128: compute v directly in
                    # [s, e] orientation (lhsT = x tile), skipping the PE
                    # transpose and the vt staging copy entirely
                    ps = ps_qkv.tile([128, SPT, 128], f32, tag="ps_qkv",
                                     name=f"psv_{bb}_{j}")
                    for s4 in range(SPT):
                        for kt in range(KT):
                            nc.tensor.matmul(
                                ps[:, s4, :],
                                xt[:, kt, s4 * 128:(s4 + 1) * 128],
                                wv_sb[:, kt, :],
                                start=(kt == 0), stop=(kt == KT - 1),
                                skip_group_check=True)
                    for s4 in range(SPT):
                        sb_idx = (col0 // 128) + s4
                        nc.vector.tensor_copy(
                            vaug[:, sb_idx, :].rearrange(
                                "p (g c) -> p g c", g=HPC)[:, :, 0:HD],
                            ps[:, s4, :].rearrange("p (g c) -> p g c", g=HPC),
                        )
                else:
                    ps = ps_qkv.tile([128, TBLK], f32, tag="ps_qkv", name=f"psv_{bb}_{j}")
                    for kt in range(KT):
                        nc.tensor.matmul(ps[:], wv_sb[:, kt, :], xt[:, kt, :],
                                         start=(kt == 0), stop=(kt == KT - 1))
                    vt = vtpool.tile([128, TBLK], MMDT, tag="vt", name=f"vt_{bb}_{j}")
                    nc.vector.tensor_copy(vt[:], ps[:])
                    for s4 in range(SPT):
                        sb_idx = (col0 // 128) + s4
                        pt = ps_tp.tile([128, 128], MMDT, tag="ps_tp",
                                        name=f"ptr_{bb}_{j}_{s4}")
                        nc.tensor.transpose(pt[:], vt[:, s4 * 128:(s4 + 1) * 128],
                                            ident[:])
                        nc.vector.tensor_copy(
                            vaug[:, sb_idx, :].rearrange(
                                "p (g c) -> p g c", g=HPC)[:, :, 0:HD],
                            pt[:].rearrange("p (g c) -> p g c", g=HPC),
                        )

            def emit_attn(bb, j, last=False):
                col0 = bb * t + j * TBLK
                tsl = slice(col0, col0 + TBLK)
                n_i = (j + 1) * SPT
                avs = [ps_av.tile([65, TBLK], f32, tag="ps_av", name=f"av_{bb}_{j}_{h}")
                       for h in range(HPC)]

                def emit_av(i_, ddp_, nh_):
                    for h in range(HPC):
                        sb_idx = (bb * t + i_ * 128) // 128
                        nc.tensor.matmul(
                            avs[h][:, ddp_:], vaug[:, sb_idx, h * 66:h * 66 + HD + 1],
                            nh_[:, h * TBLK + ddp_:(h + 1) * TBLK],
                            start=(i_ == 0), stop=(i_ == n_i - 1),
                            skip_group_check=True)

                from collections import deque
                pend = deque()
                for i in range(n_i):
                    ssl = slice(bb * t + i * 128, bb * t + i * 128 + 128)
                    dd = 128 * i - TBLK * j
                    # column trim: scores/exp/av touch only cols >= ddp
                    # (f32r needs free dim >= 256 for the PE fast path;
                    # bf16 runs 1 cyc/row at any width so trim fully)
                    if rmode:
                        ddp = max(0, min(dd, TBLK - 256))
                    else:
                        ddp = max(0, dd)
                    nh = npool.tile([128, HPC * TBLK], MMDT, tag="nh",
                                    name=f"nh_{bb}_{j}_{i}")
                    for h in range(HPC):
                        hp = slice(h * HD, (h + 1) * HD)
                        ps = ps_s.tile([128, TBLK], f32, tag="ps_s",
                                       name=f"pss_{bb}_{j}_{i}_{h}")
                        nc.tensor.matmul(
                            ps[:, ddp:], kT_sb[hp, ssl],
                            qT_sb[hp, tsl][:, ddp:], start=True, stop=True)
                        nc.scalar.activation(
                            nh[:, h * TBLK + ddp:(h + 1) * TBLK], ps[:, ddp:],
                            mybir.ActivationFunctionType.Exp, scale=0.125)
                        if dd >= 0:
                            # mask cols [ddp, dd+128): staircase + trim slack
                            # (nh col c maps to mask col c + 384 - dd)
                            nc.gpsimd.tensor_mul(
                                nh[:, h * TBLK + ddp:h * TBLK + dd + 128],
                                nh[:, h * TBLK + ddp:h * TBLK + dd + 128],
                                maskt[:, 384 - dd + ddp:512])
                    # attn@v lags 12 i-steps (== fully deferred for most
                    # blocks): the scores/exp stream runs uninterrupted, then
                    # the AV batch runs at full PE rate against banked nh
                    # tiles -- measured best across lag 1..16
                    pend.append((i, ddp, nh))
                    if len(pend) > 12:
                        i_, ddp_, nh_ = pend.popleft()
                        emit_av(i_, ddp_, nh_)
                while pend:
                    i_, ddp_, nh_ = pend.popleft()
                    emit_av(i_, ddp_, nh_)

                # --- softmax normalization: out = av * (1/Z) ---
                rr = zpool.tile([65, HPC * TBLK], MMDT, tag="rr",
                                name=f"rrr_{bb}_{j}")
                with nc.allow_low_precision(reason="f32r PE broadcast of 1/Z"):
                    for h in range(HPC):
                        nc.vector.reciprocal(
                            rr[64:65, h * TBLK:(h + 1) * TBLK], avs[h][64:65, :])
                # h1 first: its outT write goes through a SBUF->SBUF DMA
                # (partition shift), so start it before h0's direct DVE write
                bcs_h = {}
                for h in reversed(range(HPC)):
                    # K=1 matmul broadcasts 1/Z across the 64 output partitions
                    bc = ps_tp.tile([HD, TBLK], f32, tag="ps_tp", name=f"bc_{bb}_{j}_{h}")
                    nc.tensor.matmul(bc[:], one1[64:65, :],
                                     rr[64:65, h * TBLK:(h + 1) * TBLK],
                                     start=True, stop=True)
                    # DVE may read only one PSUM operand: stage bc in SBUF
                    # (on DVE -- ACT is loaded with the exp stream)
                    bcs = tmpool.tile([HD, TBLK], f32, tag="bcs", name=f"bcs_{bb}_{j}_{h}")
                    nc.vector.tensor_copy(bcs[:], bc[:])
                    bcs_h[h] = bcs
                    if h == 0:
                        nc.vector.tensor_mul(outT_sb[0:HD, tsl], avs[h][0:HD, :], bcs[:])
                    else:
                        tmp = tmpool.tile([HD, TBLK], MMDT, tag="tmp", name=f"tm_{bb}_{j}")
                        nc.vector.tensor_mul(tmp[:], avs[h][0:HD, :], bcs[:])
                        nc.sync.dma_start(
                            out=outT_sb[h * HD:(h + 1) * HD, tsl], in_=tmp[:])

            def emit_proj(bb, j, last=False):
                col0 = bb * t + j * TBLK
                NTL = TBLK // 128
                ot = opool.tile([128, NTL, d], OUTDT, tag="ot", name=f"ot_{bb}_{j}")
                for tl in range(NTL):
                    tt = col0 // 128 + tl
                    for ib in range(NIB):
                        # drain only: scores pool is idle, alternate pools for
                        # a 4-deep ring so the matmul stream is not copy-paced
                        pools = ((ps_tp, "ps_tp"), (ps_s, "ps_s"),
                                 (ps_qkv, "ps_qkv"))
                        pp, ptag = pools[(tl * NIB + ib) % 3] if last else pools[0]
                        ps = pp.tile([128, PW], f32, tag=ptag,
                                     name=f"psp_{bb}_{j}_{tl}_{ib}")
                        nc.tensor.matmul(ps[:], outT_sb[:, tt * 128:(tt + 1) * 128],
                                         wp_sb[:, ib * PW:(ib + 1) * PW],
                                         start=True, stop=True)
                        # in the drain there is no exp stream: split copies
                        # between DVE and ACT and store per row-block pair so
                        # the store overlaps the remaining copies
                        if last and (tl * NIB + ib) % 2 == 1:
                            nc.scalar.copy(ot[:, tl, ib * PW:(ib + 1) * PW], ps[:])
                        else:
                            nc.vector.tensor_copy(
                                ot[:, tl, ib * PW:(ib + 1) * PW], ps[:])
                    if last:
                        nc.sync.dma_start(
                            out=out_p.rearrange("(tb p) c -> p tb c", p=128)[
                                :, col0 // 128 + tl:col0 // 128 + tl + 1, :],
                            in_=ot[:, tl:tl + 1, :])
                if not last:
                    # one store DMA per block: [p, tl, d] -> row-blocks of out_p
                    nc.sync.dma_start(
                        out=out_p.rearrange("(tb p) c -> p tb c", p=128)[
                            :, col0 // 128:col0 // 128 + NTL, :],
                        in_=ot[:])

            # software pipeline: QKV runs one t-block ahead of attention, and
            # the projection lags one block behind, so block-boundary DVE/DMA
            # latencies hide under attention PE work
            blocks = [(bb, j) for bb in range(b) for j in range(NJ)]
            emit_xt(*blocks[0], chunked=True)
            emit_xt(*blocks[1])
            emit_qkv(*blocks[0])
            deferred = {2, 4, 6} if len(blocks) == 8 else set()
            for idx, blk in enumerate(blocks):
                if idx + 2 < len(blocks):
                    emit_xt(*blocks[idx + 2])
                if idx + 1 < len(blocks):
                    emit_qkv(*blocks[idx + 1])
                if idx == len(blocks) - 1:
                    for dfx in sorted(deferred):
                        emit_proj(*blocks[dfx - 1])
                emit_attn(*blk, last=(idx == len(blocks) - 1))
                if idx >= 1 and idx not in deferred:
                    emit_proj(*blocks[idx - 1], last=(idx == len(blocks) - 1))
            emit_proj(*blocks[-1], last=True)

    nc.compile()
    return nc


def _get_nc(b=B, t=T, d=D, cfg="b"):
    key = (b, t, d, cfg)
    if key not in _CACHE:
        _CACHE[key] = _build(b, t, d, cfg)
    return _CACHE[key]


def _in_dtype(cfg):
    if cfg == "b":
        import ml_dtypes
        return np.dtype(ml_dtypes.bfloat16)
    return np.dtype(np.float32)


def _make_consts(b, t, d, dt):
    bt = b * t
    TBLK = min(512, t)
    NSB = bt // 128
    cident = np.eye(128, dtype=dt)
    p = np.arange(128, dtype=np.int64)[:, None]
    m = np.arange(TBLK + 384, dtype=np.int64)[None, :]
    cmask = (m >= p + 384).astype(dt)
    cones = np.ones((128, NSB, HPC), dtype=dt)
    cone1 = np.ones((65, HD), dtype=dt)
    return {"cident": cident, "cmask": cmask, "cones": cones, "cone1": cone1}


def _hilo(a):
    import ml_dtypes
    f8 = np.dtype(ml_dtypes.float8_e4m3)
    hi = a.astype(f8)
    lo = (a.astype(np.float32) - hi.astype(np.float32)).astype(f8)
    return np.ascontiguousarray(hi), np.ascontiguousarray(lo)


def _prepare_in_maps_d(x, Wq, Wk, Wv, Wp, b, t, d):
    import ml_dtypes
    f8 = np.dtype(ml_dtypes.float8_e4m3)
    bf = np.dtype(ml_dtypes.bfloat16)
    bt = b * t
    TBLK = min(512, t)
    NSB = bt // 128
    SPT = TBLK // 128
    xT = x.reshape(bt, d).T.astype(np.float32)
    xTh, xTl = _hilo(xT)
    p = np.arange(128, dtype=np.int64)[:, None]
    m = np.arange(TBLK + 384, dtype=np.int64)[None, :]
    cmask = (m >= p + 384).astype(f8)
    cmaskb = (m >= p + 384).astype(bf)
    cones = np.zeros((128, NSB // 2, 2, 160), dtype=f8)
    cones[:, :, :, 64] = 1.0
    cones[:, :, :, 144] = 1.0
    conesb = np.zeros((128, b, SPT, 132), dtype=bf)
    conesb[:, :, :, 64] = 1.0
    conesb[:, :, :, 130] = 1.0
    cone1 = np.ones((65, HD), dtype=bf)
    cshf = np.zeros((HD, 128), dtype=bf)
    cshf[np.arange(HD), HD + np.arange(HD)] = 1.0
    in_maps = []
    for c in range(NCORES):
        h0 = c * HPC
        im = {"xTh": xTh, "xTl": xTl, "cmask": cmask, "cmaskb": cmaskb,
              "cones": cones, "conesb": conesb, "cone1": cone1, "cshf": cshf}
        for nm, W in (("wq", Wq), ("wk", Wk), ("wv", Wv)):
            w_c = W[h0:h0 + HPC].reshape(CH, d).T.astype(np.float32)
            im[nm + "h"], im[nm + "l"] = _hilo(w_c)
        im["wp"] = np.ascontiguousarray(
            Wp[:, c * CH:(c + 1) * CH].T.astype(bf))
        in_maps.append(im)
    return in_maps


def _prepare_in_maps(x, Wq, Wk, Wv, Wp, b, t, d, cfg):
    if cfg == "d":
        return _prepare_in_maps_d(x, Wq, Wk, Wv, Wp, b, t, d)
    bt = b * t
    dt = _in_dtype(cfg)
    xT = np.ascontiguousarray(x.reshape(bt, d).T.astype(dt))
    consts = _make_consts(b, t, d, dt)
    in_maps = []
    for c in range(NCORES):
        h0 = c * HPC
        wq_c = np.ascontiguousarray(Wq[h0:h0 + HPC].reshape(CH, d).T.astype(dt))
        wk_c = np.ascontiguousarray(Wk[h0:h0 + HPC].reshape(CH, d).T.astype(dt))
        wv_c = np.ascontiguousarray(Wv[h0:h0 + HPC].reshape(CH, d).T.astype(dt))
        wp_c = np.ascontiguousarray(Wp[:, c * CH:(c + 1) * CH].T.astype(dt))
        in_maps.append({"xT": xT, "wq": wq_c, "wk": wk_c, "wv": wv_c, "wp": wp_c,
                        **consts})
    return in_maps


def _run(x, Wq, Wk, Wv, Wp, bp, b, t, d, cfg, trace=False):
    from concourse.bass_utils import run_bass_kernel_spmd
    nc = _get_nc(b, t, d, cfg)
    in_maps = _prepare_in_maps(x, Wq, Wk, Wv, Wp, b, t, d, cfg)
    res = run_bass_kernel_spmd(nc, in_maps, core_ids=list(range(NCORES)), trace=trace)
    acc = np.zeros((b * t, d), dtype=np.float64)
    for r in res.results:
        acc += r["out_p"].astype(np.float64)
    out = (acc + np.asarray(bp, dtype=np.float64)).astype(np.float32)
    return out.reshape(b, t, d), res


KERNEL_CFG = "d"


def kernel(x, Wq, Wk, Wv, Wp, bp):
    out, _ = _run(np.asarray(x), np.asarray(Wq), np.asarray(Wk), np.asarray(Wv),
                  np.asarray(Wp), np.asarray(bp), B, T, D, KERNEL_CFG, trace=False)
    return out



# revision 90
# speedup vs baseline: 1.1394x; 1.0025x over previous
"""Multi-head attention Trainium2 kernel (8 NeuronCores, tensor-parallel over heads).

Sharding: 16 heads / 8 cores = 2 heads per core. x is replicated; Wq/Wk/Wv
sharded by head; Wp row-sharded (contraction dim). Each core computes a
partial projection output [B*T, D] stored bf16; the host sums the 8 partials
in f64 (+bias).

Default cfg "d" (fp8 DoubleRow; ~135us sim vs 147us for the bf16 cfg "b",
absmax-relative error ~3.6e-3 vs gate 2e-2):
  - QKV: x and W are split hi+lo fp8e4m3 on the host; W is pre-scaled x32 so
    both hi and the residual stay in fp8's NORMAL range (raw W~0.02 values
    are subnormal in e4m3, which destroys the residual trick). Contraction
    runs as 3 DoubleRow cross terms (Wh*xh + Wl*xh + Wh*xl), each packing
    K=256 at 0.5 cyc/row: 12 matmuls of N/2 vs bf16's 8 of N. The x32 is
    compensated for free: scores come out x1024 and the exp scale constant
    becomes 0.125/1024; v comes out x32 and the broadcast-ones constant used
    to expand 1/Z is 1/32.
  - Scores stay bf16 [s on partitions, t free]; both heads' scores land in
    one 2-bank PSUM tile so exp is ONE activation per s-block (halves the
    ACT per-op overhead; the exp stream is the attention-phase critical
    path). exp writes fp8 nh directly.
  - attn@v runs DoubleRow over PAIRS of s-blocks (contraction 256, cost
    width/2 per pair, 4x denser than bf16): lhsT = packed [v32|1] planes,
    rhs = nh pair planes; out rows 0..63 = unnormalized out^T x32, row 64 =
    Z. Plane-1 columns of diagonal pairs that sit above that plane's
    diagonal are memset on Pool. j=0 blocks (rows t<512, few softmax terms,
    largest weights -> fp8 noise would dominate absmax) use a bf16 attn@v
    path instead (vaug_b/masktb).
  - All vaug images are FULLY preloaded from host constants: reading
    uninitialized SBUF as fp8 can yield NaN (0xFF) on the first call.
  - Causality: lower-triangular [128s x 512t] blocks only; diagonal
    straddlers are column-trimmed and masked multiplicatively on Pool.
    Softmax runs without max subtraction (logits are O(1)).
  - Scheduling: the attention i-loop is ACT(exp)-paced, so qkv(j+1),
    proj(j-1) and x-tile prefetches are emitted as weighted "filler" pieces
    spread through it (the PE runs strictly in-order; contiguous chunks
    would starve either PE or ACT). PSUM: scores 2x2 banks + av 2 + shared
    qkv/proj/bc ring 2. The last block's h1 outT shift goes through a PE
    shift-matrix matmul instead of the ~1.9us SBUF->SBUF DMA.
cfg "b": all-bf16 fallback (the previous baseline), cfg "r": float32r.
"""

import numpy as np

B, T, D, H, HD = 2, 2048, 1024, 16, 64
NCORES = 8
HPC = H // NCORES          # heads per core = 2
CH = HPC * HD              # channels per core = 128
BT = B * T

_CACHE = {}


def _build_d(b, t, d):
    """cfg "d": fp8 DoubleRow build.

    - QKV: x and Wq/Wk/Wv are split hi/lo fp8e4m3 on the host (x = xh + xl
      exactly captures x to ~0.2%); contraction runs as 3 DoubleRow cross
      terms (Wh*xh + Wh*xl + Wl*xh) of K=256 each -> 12 matmuls of
      cost N/2 instead of 8 of cost N (sim model: fp8 DoubleRow is 0.5
      cycles/row with 2 K-planes packed per matmul).
    - Scores stay bf16 [s, t]; exp is ONE activation per s-block covering
      both heads (scores psum is a 2-bank [128, 2, TBLK] tile).
    - exp output is fp8 directly; attn@v runs DoubleRow over PAIRS of
      s-blocks (contraction 256): cost width/2 per pair instead of
      2*width. vaug holds fp8 v planes [128, NSB/2, 2, 160] with the
      softmax-denominator ones column at 64/144 per head.
    - Diagonal pairs: plane-1 columns below its own diagonal trim but
      inside the pair's matmul slice are memset to 0 on Pool.
    - Normalization/proj unchanged from cfg "b" (bf16).
    """
    import concourse.tile as tile
    from concourse import bacc, mybir
    from contextlib import ExitStack
    from collections import deque

    f32 = mybir.dt.float32
    bf16 = mybir.dt.bfloat16
    f8 = mybir.dt.float8e4
    DR = mybir.MatmulPerfMode.DoubleRow

    bt = b * t
    KT = d // 128
    KP = KT // 2             # DoubleRow k-pair steps
    TBLK = min(512, t)
    NJ = t // TBLK
    NSB = bt // 128
    SPT = TBLK // 128

    nc = bacc.Bacc("TRN2", target_bir_lowering=False, debug=False)

    xTh = nc.dram_tensor("xTh", [d, bt], f8, kind="ExternalInput").ap()
    xTl = nc.dram_tensor("xTl", [d, bt], f8, kind="ExternalInput").ap()
    w_in = {}
    for nm in ("wqh", "wql", "wkh", "wkl", "wvh", "wvl"):
        w_in[nm] = nc.dram_tensor(nm, [d, CH], f8, kind="ExternalInput").ap()
    wp = nc.dram_tensor("wp", [CH, d], bf16, kind="ExternalInput").ap()
    cmask = nc.dram_tensor("cmask", [128, TBLK + 384], f8, kind="ExternalInput").ap()
    cmaskb = nc.dram_tensor("cmaskb", [128, TBLK + 384], bf16,
                            kind="ExternalInput").ap()
    # FULL vaug images (ones columns + zero padding): loading the whole tile
    # avoids any read of uninitialized SBUF (fp8 garbage can be NaN) on the
    # first call
    cones = nc.dram_tensor("cones", [128, NSB // 2, 2, 160], f8,
                           kind="ExternalInput").ap()
    conesb = nc.dram_tensor("conesb", [128, b, SPT, 132], bf16,
                            kind="ExternalInput").ap()
    cone1 = nc.dram_tensor("cone1", [65, HD], bf16, kind="ExternalInput").ap()
    cshf = nc.dram_tensor("cshf", [HD, 128], bf16, kind="ExternalInput").ap()
    out_p = nc.dram_tensor("out_p", [bt, d], bf16, kind="ExternalOutput").ap()

    with tile.TileContext(nc) as tc, ExitStack() as top:
        persist = top.enter_context(tc.tile_pool(name="persist", bufs=1))

        qT_sb = persist.tile([128, bt], bf16, tag="qT")
        kT_sb = persist.tile([128, bt], bf16, tag="kT")
        # v planes: per (pair, plane): [v_h0 | 1 | pad @80 | v_h1 | 1 | pad]
        vaug = persist.tile([128, NSB // 2, 2, 160], f8, tag="vaug")
        # bf16 v for the first 4 s-blocks of each batch: the j=0 attention
        # blocks (rows t<512, where softmax weights are largest and fp8
        # noise dominates the absmax error) run a bf16 attn@v path
        vaug_b = persist.tile([128, b, SPT, 132], bf16, tag="vaug_b")
        outT_sb = persist.tile([128, bt], bf16, tag="outT")
        w_sb = {}
        for nm in ("wqh", "wql", "wkh", "wkl", "wvh", "wvl"):
            w_sb[nm] = persist.tile([128, KP, 2, CH], f8, tag=nm, name=nm)
        wp_sb = persist.tile([128, d], bf16, tag="wp")
        maskt = persist.tile([128, TBLK + 384], f8, tag="mask")
        masktb = persist.tile([128, TBLK + 384], bf16, tag="maskb")
        one1 = persist.tile([65, HD], bf16, tag="one1")
        # shift matrix: shf64[p, 64+p] = 1 moves rows 0..63 -> 64..127 via PE
        shf64 = persist.tile([HD, 128], bf16, tag="shf64")

        # startup: only the first-needed wq halves ride the shared HWDGE
        # (which the x-tile stream also needs); everything else goes through
        # the Pool SWDGE queue so it doesn't stall the critical path
        def wload(nm, eng, nsplit=1):
            for s in range(nsplit):
                hk = slice(s * KP // nsplit, (s + 1) * KP // nsplit)
                eng.dma_start(
                    out=w_sb[nm][:, hk, :, :],
                    in_=w_in[nm].rearrange(
                        "(kp pl p) m -> p kp pl m", p=128, pl=2)[:, hk, :, :],
                )
        wload("wqh", nc.scalar, 1)
        # warm the Exp act table under the startup DMAs
        actwarm = persist.tile([1, 8], f32, tag="actwarm")
        nc.scalar.activation(actwarm[:], w_sb["wqh"][0:1, 0, 0, 0:8],
                             mybir.ActivationFunctionType.Exp, scale=0.125)
        wload("wkh", nc.scalar)
        wload("wql", nc.scalar)
        wload("wkl", nc.scalar)
        wload("wvh", nc.scalar)
        wload("wvl", nc.scalar)
        # the full vaug images must land before the first v copies (~7us);
        # masktb before the first attention block's masks (~10us)
        nc.scalar.dma_start(out=vaug[:], in_=cones)
        nc.scalar.dma_start(out=vaug_b[:], in_=conesb)
        nc.scalar.dma_start(out=masktb[:], in_=cmaskb)
        nc.scalar.dma_start(out=maskt[:], in_=cmask)
        nc.scalar.dma_start(out=one1[:], in_=cone1)
        nc.scalar.dma_start(out=shf64[:], in_=cshf)
        nc.scalar.dma_start(out=wp_sb[:], in_=wp)

        with ExitStack() as body:
            xpool = body.enter_context(tc.tile_pool(name="xpool", bufs=4))
            npool = body.enter_context(tc.tile_pool(name="npool", bufs=9))
            npool_b = body.enter_context(tc.tile_pool(name="npool_b", bufs=3))
            zpool = body.enter_context(tc.tile_pool(name="zpool", bufs=2))
            tmpool = body.enter_context(tc.tile_pool(name="tmpool", bufs=2))
            opool = body.enter_context(tc.tile_pool(name="opool", bufs=3))
            # PSUM (8 banks): scores 2x2 + av 2 + shared(qkv/proj/bc) 2
            ps_s = body.enter_context(tc.tile_pool(name="ps_s", bufs=2, space="PSUM"))
            ps_av = body.enter_context(tc.tile_pool(name="ps_av", bufs=2, space="PSUM"))
            ps_sh = body.enter_context(tc.tile_pool(name="ps_sh", bufs=2, space="PSUM"))

            xt_tiles = {}

            def emit_xt(bb, j, chunked=False):
                col0 = bb * t + j * TBLK
                tsl = slice(col0, col0 + TBLK)
                srcs = (("xh", xTh), ("xl", xTl))
                pair = [xpool.tile([128, KT, TBLK], f8, tag=nm,
                                   name=f"{nm}_{bb}_{j}")
                        for nm, _ in srcs]
                if chunked:
                    # one DMA per hi/lo: fewer HWDGE slots at startup lets the
                    # k/v weight loads through ~4us earlier
                    for xt, (nm, src) in zip(pair, srcs):
                        nc.sync.dma_start(
                            out=xt[:],
                            in_=src.rearrange(
                                "(kt p) c -> p kt c", p=128)[:, :, tsl])
                else:
                    for xt, (nm, src) in zip(pair, srcs):
                        nc.sync.dma_start(
                            out=xt[:],
                            in_=src.rearrange("(kt p) c -> p kt c", p=128)[:, :, tsl])
                xt_tiles[(bb, j)] = pair

            def qkv_units(bb, j):
                """One block's QKV as weighted filler pieces (weight ~= us of
                PE work, used to spread emission across the attention i-loop
                without starving the ACT exp stream)."""
                col0 = bb * t + j * TBLK
                tsl = slice(col0, col0 + TBLK)
                state = {}
                # x-residual term order: hi*hi, lo_w*hi, hi*lo_x -- the lo x
                # tile is only needed by the last 4 matmuls of each group
                QTERMS = (("h", "xh"), ("l", "xh"), ("h", "xl"))

                def qk_mm(wt, dst, ti):
                    xts = dict(zip(("xh", "xl"), xt_tiles[(bb, j)]))
                    key = "ps" + wt
                    if ti == 0:
                        state[key] = ps_sh.tile([128, TBLK], f32, tag="ps_sh",
                                                name=f"psq_{bb}_{j}_{dst.name}")
                    ps = state[key]
                    ws, xn = QTERMS[ti]
                    for kp in range(KP):
                        nc.tensor.matmul(
                            ps[:], w_sb[wt + ws][:, kp, :, :],
                            xts[xn][:, 2 * kp:2 * kp + 2, :],
                            start=(ti == 0 and kp == 0),
                            stop=(ti == 2 and kp == KP - 1),
                            perf_mode=DR)
                    if ti == 2:
                        nc.vector.tensor_copy(dst[:, tsl], ps[:])

                def v_mm(s4):
                    xth, xtl = xt_tiles[(bb, j)]
                    if "vps" not in state:
                        state["vps"] = ps_sh.tile([128, SPT, 128], f32,
                                                  tag="ps_sh",
                                                  name=f"psv_{bb}_{j}")
                    ps = state["vps"]
                    idx = 0
                    for xh, wn in ((xth, "wvh"), (xth, "wvl"), (xtl, "wvh")):
                        for kp in range(KP):
                            nc.tensor.matmul(
                                ps[:, s4, :],
                                xh[:, 2 * kp:2 * kp + 2,
                                   s4 * 128:(s4 + 1) * 128],
                                w_sb[wn][:, kp, :, :],
                                start=(idx == 0), stop=(idx == 3 * KP - 1),
                                perf_mode=DR, skip_group_check=True)
                            idx += 1
                    sb_idx = (col0 // 128) + s4
                    m, pl = divmod(sb_idx, 2)
                    nc.vector.tensor_copy(
                        vaug[:, m, pl, :].rearrange(
                            "p (g c) -> p g c", g=2)[:, :, 0:HD],
                        ps[:, s4, :].rearrange("p (g c) -> p g c", g=HPC),
                    )
                    if j == 0:
                        # bf16 copy for the j=0 attention path
                        nc.vector.tensor_copy(
                            vaug_b[:, bb, s4, :].rearrange(
                                "p (g c) -> p g c", g=2)[:, :, 0:HD],
                            ps[:, s4, :].rearrange("p (g c) -> p g c", g=HPC),
                        )
                    if s4 == SPT - 1:
                        xt_tiles.pop((bb, j))

                units = []
                for wt, dst in (("wq", qT_sb), ("wk", kT_sb)):
                    for ti in range(3):
                        units.append((0.45, (lambda wt_=wt, dst_=dst, ti_=ti:
                                             qk_mm(wt_, dst_, ti_))))
                for s4 in range(SPT):
                    units.append((0.9, (lambda s4_=s4: v_mm(s4_))))
                return units

            def emit_qkv(bb, j):
                for _, u in qkv_units(bb, j):
                    u()

            def emit_attn(bb, j, fillers=(), last=False):
                col0 = bb * t + j * TBLK
                tsl = slice(col0, col0 + TBLK)
                n_i = (j + 1) * SPT
                n_pairs = n_i // 2
                avs = [ps_av.tile([65, TBLK], f32, tag="ps_av",
                                  name=f"av_{bb}_{j}_{h}")
                       for h in range(HPC)]

                def emit_av(m_, ddp_, nh_):
                    sbp = bb * (t // 256) + m_
                    for h in range(HPC):
                        nc.tensor.matmul(
                            avs[h][:, ddp_:],
                            vaug[:, sbp, :, 80 * h:80 * h + HD + 1],
                            nh_[:, :, h, ddp_:],
                            start=(m_ == 0), stop=(m_ == n_pairs - 1),
                            perf_mode=DR, skip_group_check=True)

                def emit_av_b(i_, ddp_, nh_, pl_):
                    # bf16 path (j=0 blocks): per-s-block standard matmul
                    for h in range(HPC):
                        nc.tensor.matmul(
                            avs[h][:, ddp_:],
                            vaug_b[:, bb, i_, 66 * h:66 * h + HD + 1],
                            nh_[:, pl_, h, ddp_:],
                            start=(i_ == 0), stop=(i_ == n_i - 1),
                            skip_group_check=True)

                fillers = list(fillers)
                wtotal = sum(w for w, _ in fillers) or 1.0
                nfill = 0
                wdone = 0.0

                pend = deque()
                nh_m = None
                pair_ddp = 0
                for i in range(n_i):
                    # spread deferred qkv/proj/xt work through the i-loop by
                    # cumulative PE-work weight, so the PE has in-order work
                    # during exp waits without long bursts that starve ACT
                    # finish fillers ~2 i-steps early so the DVE queue is
                    # drained when the block-end normalize chain needs it
                    want = (i / max(1, n_i - 2)) * wtotal
                    while nfill < len(fillers) and wdone < want:
                        w, fn = fillers[nfill]
                        fn()
                        wdone += w
                        nfill += 1
                    ssl = slice(bb * t + i * 128, bb * t + i * 128 + 128)
                    dd = 128 * i - TBLK * j
                    ddp = max(0, dd)
                    m, pl = divmod(i, 2)
                    nhdt = bf16 if j == 0 else f8
                    nhmask = masktb if j == 0 else maskt
                    if pl == 0:
                        np_ = npool_b if j == 0 else npool
                        nh_m = np_.tile([128, 2, HPC, TBLK], nhdt,
                                        tag="nhb" if j == 0 else "nh",
                                        name=f"nh_{bb}_{j}_{m}")
                        pair_ddp = ddp
                    ps = ps_s.tile([128, HPC, TBLK], f32, tag="ps_s",
                                   name=f"pss_{bb}_{j}_{i}")
                    for h in range(HPC):
                        hp = slice(h * HD, (h + 1) * HD)
                        nc.tensor.matmul(
                            ps[:, h, ddp:], kT_sb[hp, ssl],
                            qT_sb[hp, tsl][:, ddp:], start=True, stop=True)
                    nc.scalar.activation(
                        nh_m[:, pl, :, ddp:], ps[:, :, ddp:],
                        mybir.ActivationFunctionType.Exp, scale=0.125)
                    if dd >= 0:
                        for h in range(HPC):
                            nc.gpsimd.tensor_mul(
                                nh_m[:, pl, h, ddp:dd + 128],
                                nh_m[:, pl, h, ddp:dd + 128],
                                nhmask[:, 384 - dd + ddp:512])
                    if j == 0:
                        pend.append((i, ddp, nh_m, pl))
                        if len(pend) > 2:
                            emit_av_b(*pend.popleft())
                    elif pl == 1:
                        if ddp > pair_ddp:
                            # plane-1 cols [pair_ddp, ddp) are inside the AV
                            # slice but above this plane's diagonal: zero them
                            nc.gpsimd.memset(nh_m[:, 1, :, pair_ddp:ddp], 0.0)
                        pend.append((m, pair_ddp, nh_m))
                        if len(pend) > 4:
                            emit_av(*pend.popleft())
                while pend:
                    if j == 0:
                        emit_av_b(*pend.popleft())
                    else:
                        emit_av(*pend.popleft())

                # --- softmax normalization: out = av * (1/Z) (as cfg "b") ---
                rr = zpool.tile([65, HPC * TBLK], bf16, tag="rr",
                                name=f"rrr_{bb}_{j}")
                # last block: h1 first so its PE shift + copy overlap h0's
                # multiply instead of trailing it in the drain
                horder = (1, 0) if last else (0, 1)
                for h in horder:
                    with nc.allow_low_precision(reason="bf16 1/Z broadcast"):
                        nc.vector.reciprocal(
                            rr[64:65, h * TBLK:(h + 1) * TBLK], avs[h][64:65, :])
                    bc = ps_sh.tile([HD, TBLK], f32, tag="ps_sh",
                                    name=f"bc_{bb}_{j}_{h}")
                    nc.tensor.matmul(bc[:], one1[64:65, :],
                                     rr[64:65, h * TBLK:(h + 1) * TBLK],
                                     start=True, stop=True)
                    bcs = tmpool.tile([HD, TBLK], f32, tag="bcs",
                                      name=f"bcs_{bb}_{j}_{h}")
                    nc.vector.tensor_copy(bcs[:], bc[:])
                    if h == 0:
                        nc.vector.tensor_mul(outT_sb[0:HD, tsl], avs[h][0:HD, :],
                                             bcs[:])
                    else:
                        tmp = tmpool.tile([HD, TBLK], bf16, tag="tmp",
                                          name=f"tm_{bb}_{j}")
                        nc.vector.tensor_mul(tmp[:], avs[h][0:HD, :], bcs[:])
                        if last:
                            # end-game: SBUF->SBUF DMA costs ~1.9us latency in
                            # the serial tail; shift partitions via PE instead
                            # (reuses a scores-pool tile -- the exp stream is
                            # finished by now, so no extra PSUM footprint)
                            pt = ps_s.tile([128, HPC, TBLK], f32, tag="ps_s",
                                           name=f"shf_{bb}_{j}")
                            nc.tensor.matmul(pt[:, 0, :], shf64[:], tmp[:],
                                             start=True, stop=True)
                            nc.vector.tensor_copy(
                                outT_sb[h * HD:(h + 1) * HD, tsl],
                                pt[h * HD:(h + 1) * HD, 0, :])
                        else:
                            nc.sync.dma_start(
                                out=outT_sb[h * HD:(h + 1) * HD, tsl],
                                in_=tmp[:])

                # leftover fillers run after the normalize chain is queued so
                # the recip/mult don't sit behind filler copies on DVE
                while nfill < len(fillers):
                    fillers[nfill][1]()
                    nfill += 1

            PW = min(512, d)
            NIB = d // PW

            def proj_units(bb, j, last=False):
                col0 = bb * t + j * TBLK
                NTL = TBLK // 128
                state = {}

                def piece(tl, ib):
                    if "ot" not in state:
                        state["ot"] = opool.tile([128, NTL, d], bf16, tag="ot",
                                                 name=f"ot_{bb}_{j}")
                    ot = state["ot"]
                    tt = col0 // 128 + tl
                    ps = ps_sh.tile([128, PW], f32, tag="ps_sh",
                                    name=f"psp_{bb}_{j}_{tl}_{ib}")
                    nc.tensor.matmul(ps[:], outT_sb[:, tt * 128:(tt + 1) * 128],
                                     wp_sb[:, ib * PW:(ib + 1) * PW],
                                     start=True, stop=True)
                    if last and (tl * NIB + ib) % 2 == 1:
                        nc.scalar.copy(ot[:, tl, ib * PW:(ib + 1) * PW], ps[:])
                    else:
                        nc.vector.tensor_copy(
                            ot[:, tl, ib * PW:(ib + 1) * PW], ps[:])
                    if ib == NIB - 1 and last:
                        # split the very last tile's store so the final
                        # DMA (+sem) tail is half as long
                        nsp = 2 if tl == NTL - 1 else 1
                        for sp in range(nsp):
                            csl = slice(sp * d // nsp, (sp + 1) * d // nsp)
                            nc.sync.dma_start(
                                out=out_p.rearrange(
                                    "(tb p) c -> p tb c", p=128)[
                                    :, col0 // 128 + tl:col0 // 128 + tl + 1,
                                    csl],
                                in_=ot[:, tl:tl + 1, csl])
                    if tl == NTL - 1 and ib == NIB - 1 and not last:
                        nc.sync.dma_start(
                            out=out_p.rearrange("(tb p) c -> p tb c", p=128)[
                                :, col0 // 128:col0 // 128 + NTL, :],
                            in_=ot[:])

                return [
                    (0.5, (lambda tl_, ib_: lambda: piece(tl_, ib_))(tl, ib))
                    for tl in range(NTL) for ib in range(NIB)
                ]

            def emit_proj(bb, j, last=False):
                for _, u in proj_units(bb, j, last):
                    u()

            blocks = [(bb, j) for bb in range(b) for j in range(NJ)]
            # proj spans: the j0/j1 attention spans are PE-over-budget (their
            # exp streams are short) while j2/j3 spans have ACT-paced PE
            # slack, so proj(j) is deferred into a later, slack-rich span of
            # the same batch instead of lagging exactly one block
            span_proj = {i: [] for i in range(len(blocks))}
            for bb in range(b):
                base = bb * NJ
                span_proj[base + 2].append(base + 0)
                span_proj[base + 3].extend([base + 1, base + 2])
                if bb + 1 < b:
                    span_proj[(bb + 1) * NJ + 1].append(base + 3)
            emit_xt(*blocks[0], chunked=True)
            emit_xt(*blocks[1])
            emit_qkv(*blocks[0])
            for idx, blk in enumerate(blocks):
                qk_u = (qkv_units(*blocks[idx + 1])
                        if idx + 1 < len(blocks) else [])
                pr_u = []
                for k in span_proj[idx]:
                    pr_u.extend(proj_units(*blocks[k]))
                fillers = []
                if idx + 2 < len(blocks):
                    bbn, jn = blocks[idx + 2]
                    fillers.append(
                        (0.1, lambda bbn=bbn, jn=jn: emit_xt(bbn, jn)))
                # round-robin qkv and proj pieces: qkv early enough for the
                # next block, proj (which waits on this block's outT
                # predecessor) spread across the span
                qi = pi = 0
                while qi < len(qk_u) or pi < len(pr_u):
                    if qi < len(qk_u):
                        fillers.append(qk_u[qi])
                        qi += 1
                    if pi < len(pr_u):
                        fillers.append(pr_u[pi])
                        pi += 1
                emit_attn(*blk, fillers=fillers, last=(idx == len(blocks) - 1))
            emit_proj(*blocks[-1], last=True)

    nc.compile()
    return nc


def _build(b, t, d, cfg):
    """Build + compile the per-core Bass program."""
    if cfg == "d":
        return _build_d(b, t, d)
    import concourse.tile as tile
    from concourse import bacc, mybir
    from contextlib import ExitStack

    f32 = mybir.dt.float32
    f32r = mybir.dt.float32r
    bf16 = mybir.dt.bfloat16

    rmode = cfg == "r"
    bmode = cfg == "b"
    # dtype of every matmul-feeding tile
    MMDT = f32r if rmode else (bf16 if bmode else f32)
    # dtype of the DMA'd inputs (host converts for bf16)
    INDT = bf16 if bmode else f32

    def bcst(ap):
        return ap.bitcast(f32r) if rmode else ap

    bt = b * t
    KT = d // 128            # k-tiles over the model dim
    TBLK = min(512, t)       # t-block width for scores/attn
    NJ = t // TBLK           # t-blocks per batch
    NSB = bt // 128          # 128-row s-blocks over B*T
    SPT = TBLK // 128        # s-blocks per t-block

    nc = bacc.Bacc("TRN2", target_bir_lowering=False, debug=False)

    xT = nc.dram_tensor("xT", [d, bt], INDT, kind="ExternalInput").ap()
    wq = nc.dram_tensor("wq", [d, CH], INDT, kind="ExternalInput").ap()
    wk = nc.dram_tensor("wk", [d, CH], INDT, kind="ExternalInput").ap()
    wv = nc.dram_tensor("wv", [d, CH], INDT, kind="ExternalInput").ap()
    wp = nc.dram_tensor("wp", [CH, d], INDT, kind="ExternalInput").ap()
    cident = nc.dram_tensor("cident", [128, 128], INDT, kind="ExternalInput").ap()
    cmask = nc.dram_tensor("cmask", [128, TBLK + 384], INDT, kind="ExternalInput").ap()
    cones = nc.dram_tensor("cones", [128, NSB, HPC], INDT, kind="ExternalInput").ap()
    cone1 = nc.dram_tensor("cone1", [65, HD], INDT, kind="ExternalInput").ap()
    # partials are summed across cores on the host in f64; bf16 partial
    # stores halve the output DMA traffic for ~1e-3 extra absmax-rel error
    OUTDT = bf16 if bmode else f32
    out_p = nc.dram_tensor("out_p", [bt, d], OUTDT, kind="ExternalOutput").ap()

    with tile.TileContext(nc) as tc, ExitStack() as top:
        persist = top.enter_context(tc.tile_pool(name="persist", bufs=1))

        # ---- persistent tiles ----
        qT_sb = persist.tile([128, bt], MMDT, tag="qT")
        kT_sb = persist.tile([128, bt], MMDT, tag="kT")
        # [v_h0 | 1 | pad | v_h1 | 1 | pad] per 128-row s-block
        vaug = persist.tile([128, NSB, 66 * HPC], MMDT, tag="vaug")
        outT_sb = persist.tile([128, bt], MMDT, tag="outT")
        wq_sb = persist.tile([128, KT, CH], MMDT, tag="wq")
        wk_sb = persist.tile([128, KT, CH], MMDT, tag="wk")
        wv_sb = persist.tile([128, KT, CH], MMDT, tag="wv")
        wp_sb = persist.tile([128, d], MMDT, tag="wp")
        ident = persist.tile([128, 128], MMDT, tag="ident")
        # staircase mask, shifted: maskt[p, m] = 1 iff m >= p + 384
        maskt = persist.tile([128, TBLK + 384], MMDT, tag="mask")
        one1 = persist.tile([65, HD], MMDT, tag="one1")

        # startup DMAs on the scalar HWDGE queue (fast descriptor gen; the
        # Pool SWDGE takes ~1.1us per DMA), ordered by first use: ident
        # (act-table warm + block-0 transposes), big wq/wk/wv loads, then
        # attention consts; wp (needed only by the lagging proj) last.
        # wq in two halves so the first q matmuls start ~1.4us earlier
        for w_ap, w_sb, nsplit in ((wq, wq_sb, 2), (wk, wk_sb, 1), (wv, wv_sb, 1)):
            for s in range(nsplit):
                hk = slice(s * KT // nsplit, (s + 1) * KT // nsplit)
                nc.scalar.dma_start(
                    out=w_sb[:, hk, :],
                    in_=bcst(w_ap.rearrange("(kt p) m -> p kt m", p=128)[:, hk, :]),
                )
        # preload the Exp activation table under the startup DMAs
        actwarm = persist.tile([1, 8], f32, tag="actwarm")
        nc.scalar.activation(actwarm[:], wq_sb[0:1, 0, 0:8],
                             mybir.ActivationFunctionType.Exp, scale=0.125)
        # ident only feeds the PE-transpose path (non-bf16 modes), but the
        # load stays unconditional: dropping it shifts the startup DMA
        # phasing and measures 2.6us WORSE in bmode (scheduler alignment)
        nc.scalar.dma_start(out=ident[:], in_=bcst(cident))
        nc.scalar.dma_start(out=maskt[:], in_=bcst(cmask))
        nc.scalar.dma_start(out=one1[:], in_=bcst(cone1))
        for h in range(HPC):
            nc.scalar.dma_start(
                out=vaug[:, :, 66 * h + 64:66 * h + 65],
                in_=bcst(cones[:, :, h:h + 1]),
            )
        nc.scalar.dma_start(out=wp_sb[:], in_=bcst(wp))

        # ---- merged loop: per (batch, t-block): QKV -> attention -> proj ----
        # Attention for block j of batch bb needs q columns of block j and
        # k/v columns of blocks 0..j (same batch) -- all computed by the time
        # block j's QKV is done, so one fused loop pipelines everything:
        # xT loads prefetch under attention PE work, and output stores drain
        # under the next block's compute.
        PW = min(512, d)
        NIB = d // PW
        with ExitStack() as body:
            xpool = body.enter_context(tc.tile_pool(name="xpool", bufs=4 if bmode else 3))
            vtpool = body.enter_context(tc.tile_pool(name="vtpool", bufs=2))
            npool = body.enter_context(tc.tile_pool(name="npool", bufs=24 if bmode else 5))
            zpool = body.enter_context(tc.tile_pool(name="zpool", bufs=2))
            tmpool = body.enter_context(tc.tile_pool(name="tmpool", bufs=2))
            opool = body.enter_context(tc.tile_pool(name="opool", bufs=3))
            # PSUM budget (8 banks): qkv 2 + scores 2 + av 2 + tr/proj/bc 2
            ps_qkv = body.enter_context(tc.tile_pool(name="ps_qkv", bufs=2, space="PSUM"))
            ps_s = body.enter_context(tc.tile_pool(name="ps_s", bufs=2, space="PSUM"))
            ps_av = body.enter_context(tc.tile_pool(name="ps_av", bufs=2, space="PSUM"))
            ps_tp = body.enter_context(tc.tile_pool(name="ps_tp", bufs=2, space="PSUM"))

            xt_tiles = {}

            def emit_xt(bb, j, chunked=False):
                col0 = bb * t + j * TBLK
                tsl = slice(col0, col0 + TBLK)
                xt = xpool.tile([128, KT, TBLK], MMDT, tag="xt", name=f"xt_{bb}_{j}")
                if chunked:
                    for kt in range(KT):
                        nc.sync.dma_start(
                            out=xt[:, kt, :],
                            in_=bcst(xT[kt * 128:(kt + 1) * 128, tsl]),
                        )
                else:
                    nc.sync.dma_start(
                        out=xt[:],
                        in_=bcst(xT.rearrange("(kt p) c -> p kt c", p=128)[:, :, tsl]),
                    )
                xt_tiles[(bb, j)] = xt

            def emit_qkv(bb, j):
                col0 = bb * t + j * TBLK
                tsl = slice(col0, col0 + TBLK)
                xt = xt_tiles.pop((bb, j))
                for w_sb, dst in ((wq_sb, qT_sb), (wk_sb, kT_sb)):
                    ps = ps_qkv.tile([128, TBLK], f32, tag="ps_qkv",
                                     name=f"psq_{bb}_{j}_{dst.name}")
                    for kt in range(KT):
                        nc.tensor.matmul(ps[:], w_sb[:, kt, :], xt[:, kt, :],
                                         start=(kt == 0), stop=(kt == KT - 1))
                    nc.vector.tensor_copy(dst[:, tsl], ps[:])
                if bmode:
                    # bf16 runs 1 cyc/row at width 128: compute v directly in
                    # [s, e] orientation (lhsT = x tile), skipping the PE
                    # transpose and the vt staging copy entirely
                    ps = ps_qkv.tile([128, SPT, 128], f32, tag="ps_qkv",
                                     name=f"psv_{bb}_{j}")
                    for s4 in range(SPT):
                        for kt in range(KT):
                            nc.tensor.matmul(
                                ps[:, s4, :],
                                xt[:, kt, s4 * 128:(s4 + 1) * 128],
                                wv_sb[:, kt, :],
                                start=(kt == 0), stop=(kt == KT - 1),
                                skip_group_check=True)
                    for s4 in range(SPT):
                        sb_idx = (col0 // 128) + s4
                        nc.vector.tensor_copy(
                            vaug[:, sb_idx, :].rearrange(
                                "p (g c) -> p g c", g=HPC)[:, :, 0:HD],
                            ps[:, s4, :].rearrange("p (g c) -> p g c", g=HPC),
                        )
                else:
                    ps = ps_qkv.tile([128, TBLK], f32, tag="ps_qkv", name=f"psv_{bb}_{j}")
                    for kt in range(KT):
                        nc.tensor.matmul(ps[:], wv_sb[:, kt, :], xt[:, kt, :],
                                         start=(kt == 0), stop=(kt == KT - 1))
                    vt = vtpool.tile([128, TBLK], MMDT, tag="vt", name=f"vt_{bb}_{j}")
                    nc.vector.tensor_copy(vt[:], ps[:])
                    for s4 in range(SPT):
                        sb_idx = (col0 // 128) + s4
                        pt = ps_tp.tile([128, 128], MMDT, tag="ps_tp",
                                        name=f"ptr_{bb}_{j}_{s4}")
                        nc.tensor.transpose(pt[:], vt[:, s4 * 128:(s4 + 1) * 128],
                                            ident[:])
                        nc.vector.tensor_copy(
                            vaug[:, sb_idx, :].rearrange(
                                "p (g c) -> p g c", g=HPC)[:, :, 0:HD],
                            pt[:].rearrange("p (g c) -> p g c", g=HPC),
                        )

            def emit_attn(bb, j, last=False):
                col0 = bb * t + j * TBLK
                tsl = slice(col0, col0 + TBLK)
                n_i = (j + 1) * SPT
                avs = [ps_av.tile([65, TBLK], f32, tag="ps_av", name=f"av_{bb}_{j}_{h}")
                       for h in range(HPC)]

                def emit_av(i_, ddp_, nh_):
                    for h in range(HPC):
                        sb_idx = (bb * t + i_ * 128) // 128
                        nc.tensor.matmul(
                            avs[h][:, ddp_:], vaug[:, sb_idx, h * 66:h * 66 + HD + 1],
                            nh_[:, h * TBLK + ddp_:(h + 1) * TBLK],
                            start=(i_ == 0), stop=(i_ == n_i - 1),
                            skip_group_check=True)

                from collections import deque
                pend = deque()
                for i in range(n_i):
                    ssl = slice(bb * t + i * 128, bb * t + i * 128 + 128)
                    dd = 128 * i - TBLK * j
                    # column trim: scores/exp/av touch only cols >= ddp
                    # (f32r needs free dim >= 256 for the PE fast path;
                    # bf16 runs 1 cyc/row at any width so trim fully)
                    if rmode:
                        ddp = max(0, min(dd, TBLK - 256))
                    else:
                        ddp = max(0, dd)
                    nh = npool.tile([128, HPC * TBLK], MMDT, tag="nh",
                                    name=f"nh_{bb}_{j}_{i}")
                    for h in range(HPC):
                        hp = slice(h * HD, (h + 1) * HD)
                        ps = ps_s.tile([128, TBLK], f32, tag="ps_s",
                                       name=f"pss_{bb}_{j}_{i}_{h}")
                        nc.tensor.matmul(
                            ps[:, ddp:], kT_sb[hp, ssl],
                            qT_sb[hp, tsl][:, ddp:], start=True, stop=True)
                        nc.scalar.activation(
                            nh[:, h * TBLK + ddp:(h + 1) * TBLK], ps[:, ddp:],
                            mybir.ActivationFunctionType.Exp, scale=0.125)
                        if dd >= 0:
                            # mask cols [ddp, dd+128): staircase + trim slack
                            # (nh col c maps to mask col c + 384 - dd)
                            nc.gpsimd.tensor_mul(
                                nh[:, h * TBLK + ddp:h * TBLK + dd + 128],
                                nh[:, h * TBLK + ddp:h * TBLK + dd + 128],
                                maskt[:, 384 - dd + ddp:512])
                    # attn@v lags 12 i-steps (== fully deferred for most
                    # blocks): the scores/exp stream runs uninterrupted, then
                    # the AV batch runs at full PE rate against banked nh
                    # tiles -- measured best across lag 1..16
                    pend.append((i, ddp, nh))
                    if len(pend) > 12:
                        i_, ddp_, nh_ = pend.popleft()
                        emit_av(i_, ddp_, nh_)
                while pend:
                    i_, ddp_, nh_ = pend.popleft()
                    emit_av(i_, ddp_, nh_)

                # --- softmax normalization: out = av * (1/Z) ---
                rr = zpool.tile([65, HPC * TBLK], MMDT, tag="rr",
                                name=f"rrr_{bb}_{j}")
                with nc.allow_low_precision(reason="f32r PE broadcast of 1/Z"):
                    for h in range(HPC):
                        nc.vector.reciprocal(
                            rr[64:65, h * TBLK:(h + 1) * TBLK], avs[h][64:65, :])
                # h1 first: its outT write goes through a SBUF->SBUF DMA
                # (partition shift), so start it before h0's direct DVE write
                bcs_h = {}
                for h in reversed(range(HPC)):
                    # K=1 matmul broadcasts 1/Z across the 64 output partitions
                    bc = ps_tp.tile([HD, TBLK], f32, tag="ps_tp", name=f"bc_{bb}_{j}_{h}")
                    nc.tensor.matmul(bc[:], one1[64:65, :],
                                     rr[64:65, h * TBLK:(h + 1) * TBLK],
                                     start=True, stop=True)
                    # DVE may read only one PSUM operand: stage bc in SBUF
                    # (on DVE -- ACT is loaded with the exp stream)
                    bcs = tmpool.tile([HD, TBLK], f32, tag="bcs", name=f"bcs_{bb}_{j}_{h}")
                    nc.vector.tensor_copy(bcs[:], bc[:])
                    bcs_h[h] = bcs
                    if h == 0:
                        nc.vector.tensor_mul(outT_sb[0:HD, tsl], avs[h][0:HD, :], bcs[:])
                    else:
                        tmp = tmpool.tile([HD, TBLK], MMDT, tag="tmp", name=f"tm_{bb}_{j}")
                        nc.vector.tensor_mul(tmp[:], avs[h][0:HD, :], bcs[:])
                        nc.sync.dma_start(
                            out=outT_sb[h * HD:(h + 1) * HD, tsl], in_=tmp[:])

            def emit_proj(bb, j, last=False):
                col0 = bb * t + j * TBLK
                NTL = TBLK // 128
                ot = opool.tile([128, NTL, d], OUTDT, tag="ot", name=f"ot_{bb}_{j}")
                for tl in range(NTL):
                    tt = col0 // 128 + tl
                    for ib in range(NIB):
                        # drain only: scores pool is idle, alternate pools for
                        # a 4-deep ring so the matmul stream is not copy-paced
                        pools = ((ps_tp, "ps_tp"), (ps_s, "ps_s"),
                                 (ps_qkv, "ps_qkv"))
                        pp, ptag = pools[(tl * NIB + ib) % 3] if last else pools[0]
                        ps = pp.tile([128, PW], f32, tag=ptag,
                                     name=f"psp_{bb}_{j}_{tl}_{ib}")
                        nc.tensor.matmul(ps[:], outT_sb[:, tt * 128:(tt + 1) * 128],
                                         wp_sb[:, ib * PW:(ib + 1) * PW],
                                         start=True, stop=True)
                        # in the drain there is no exp stream: split copies
                        # between DVE and ACT and store per row-block pair so
                        # the store overlaps the remaining copies
                        if last and (tl * NIB + ib) % 2 == 1:
                            nc.scalar.copy(ot[:, tl, ib * PW:(ib + 1) * PW], ps[:])
                        else:
                            nc.vector.tensor_copy(
                                ot[:, tl, ib * PW:(ib + 1) * PW], ps[:])
                    if last:
                        nc.sync.dma_start(
                            out=out_p.rearrange("(tb p) c -> p tb c", p=128)[
                                :, col0 // 128 + tl:col0 // 128 + tl + 1, :],
                            in_=ot[:, tl:tl + 1, :])
                if not last:
                    # one store DMA per block: [p, tl, d] -> row-blocks of out_p
                    nc.sync.dma_start(
                        out=out_p.rearrange("(tb p) c -> p tb c", p=128)[
                            :, col0 // 128:col0 // 128 + NTL, :],
                        in_=ot[:])

            # software pipeline: QKV runs one t-block ahead of attention, and
            # the projection lags one block behind, so block-boundary DVE/DMA
            # latencies hide under attention PE work
            blocks = [(bb, j) for bb in range(b) for j in range(NJ)]
            emit_xt(*blocks[0], chunked=True)
            emit_xt(*blocks[1])
            emit_qkv(*blocks[0])
            deferred = {2, 4, 6} if len(blocks) == 8 else set()
            for idx, blk in enumerate(blocks):
                if idx + 2 < len(blocks):
                    emit_xt(*blocks[idx + 2])
                if idx + 1 < len(blocks):
                    emit_qkv(*blocks[idx + 1])
                if idx == len(blocks) - 1:
                    for dfx in sorted(deferred):
                        emit_proj(*blocks[dfx - 1])
                emit_attn(*blk, last=(idx == len(blocks) - 1))
                if idx >= 1 and idx not in deferred:
                    emit_proj(*blocks[idx - 1], last=(idx == len(blocks) - 1))
            emit_proj(*blocks[-1], last=True)

    nc.compile()
    return nc


def _get_nc(b=B, t=T, d=D, cfg="b"):
    key = (b, t, d, cfg)
    if key not in _CACHE:
        _CACHE[key] = _build(b, t, d, cfg)
    return _CACHE[key]


def _in_dtype(cfg):
    if cfg == "b":
        import ml_dtypes
        return np.dtype(ml_dtypes.bfloat16)
    return np.dtype(np.float32)


def _make_consts(b, t, d, dt):
    bt = b * t
    TBLK = min(512, t)
    NSB = bt // 128
    cident = np.eye(128, dtype=dt)
    p = np.arange(128, dtype=np.int64)[:, None]
    m = np.arange(TBLK + 384, dtype=np.int64)[None, :]
    cmask = (m >= p + 384).astype(dt)
    cones = np.ones((128, NSB, HPC), dtype=dt)
    cone1 = np.ones((65, HD), dtype=dt)
    return {"cident": cident, "cmask": cmask, "cones": cones, "cone1": cone1}


def _hilo(a):
    import ml_dtypes
    f8 = np.dtype(ml_dtypes.float8_e4m3)
    hi = a.astype(f8)
    lo = (a.astype(np.float32) - hi.astype(np.float32)).astype(f8)
    return np.ascontiguousarray(hi), np.ascontiguousarray(lo)


def _prepare_in_maps_d(x, Wq, Wk, Wv, Wp, b, t, d):
    import ml_dtypes
    f8 = np.dtype(ml_dtypes.float8_e4m3)
    bf = np.dtype(ml_dtypes.bfloat16)
    bt = b * t
    TBLK = min(512, t)
    NSB = bt // 128
    SPT = TBLK // 128
    xT = x.reshape(bt, d).T.astype(np.float32)
    xTh, xTl = _hilo(xT)
    p = np.arange(128, dtype=np.int64)[:, None]
    m = np.arange(TBLK + 384, dtype=np.int64)[None, :]
    cmask = (m >= p + 384).astype(f8)
    cmaskb = (m >= p + 384).astype(bf)
    cones = np.zeros((128, NSB // 2, 2, 160), dtype=f8)
    cones[:, :, :, 64] = 1.0
    cones[:, :, :, 144] = 1.0
    conesb = np.zeros((128, b, SPT, 132), dtype=bf)
    conesb[:, :, :, 64] = 1.0
    conesb[:, :, :, 130] = 1.0
    cone1 = np.ones((65, HD), dtype=bf)
    cshf = np.zeros((HD, 128), dtype=bf)
    cshf[np.arange(HD), HD + np.arange(HD)] = 1.0
    in_maps = []
    for c in range(NCORES):
        h0 = c * HPC
        im = {"xTh": xTh, "xTl": xTl, "cmask": cmask, "cmaskb": cmaskb,
              "cones": cones, "conesb": conesb, "cone1": cone1, "cshf": cshf}
        for nm, W in (("wq", Wq), ("wk", Wk), ("wv", Wv)):
            w_c = W[h0:h0 + HPC].reshape(CH, d).T.astype(np.float32)
            im[nm + "h"], im[nm + "l"] = _hilo(w_c)
        im["wp"] = np.ascontiguousarray(
            Wp[:, c * CH:(c + 1) * CH].T.astype(bf))
        in_maps.append(im)
    return in_maps


def _prepare_in_maps(x, Wq, Wk, Wv, Wp, b, t, d, cfg):
    if cfg == "d":
        return _prepare_in_maps_d(x, Wq, Wk, Wv, Wp, b, t, d)
    bt = b * t
    dt = _in_dtype(cfg)
    xT = np.ascontiguousarray(x.reshape(bt, d).T.astype(dt))
    consts = _make_consts(b, t, d, dt)
    in_maps = []
    for c in range(NCORES):
        h0 = c * HPC
        wq_c = np.ascontiguousarray(Wq[h0:h0 + HPC].reshape(CH, d).T.astype(dt))
        wk_c = np.ascontiguousarray(Wk[h0:h0 + HPC].reshape(CH, d).T.astype(dt))
        wv_c = np.ascontiguousarray(Wv[h0:h0 + HPC].reshape(CH, d).T.astype(dt))
        wp_c = np.ascontiguousarray(Wp[:, c * CH:(c + 1) * CH].T.astype(dt))
        in_maps.append({"xT": xT, "wq": wq_c, "wk": wk_c, "wv": wv_c, "wp": wp_c,
                        **consts})
    return in_maps


def _run(x, Wq, Wk, Wv, Wp, bp, b, t, d, cfg, trace=False):
    from concourse.bass_utils import run_bass_kernel_spmd
    nc = _get_nc(b, t, d, cfg)
    in_maps = _prepare_in_maps(x, Wq, Wk, Wv, Wp, b, t, d, cfg)
    res = run_bass_kernel_spmd(nc, in_maps, core_ids=list(range(NCORES)), trace=trace)
    acc = np.zeros((b * t, d), dtype=np.float64)
    for r in res.results:
        acc += r["out_p"].astype(np.float64)
    out = (acc + np.asarray(bp, dtype=np.float64)).astype(np.float32)
    return out.reshape(b, t, d), res


KERNEL_CFG = "d"


def kernel(x, Wq, Wk, Wv, Wp, bp):
    out, _ = _run(np.asarray(x), np.asarray(Wq), np.asarray(Wk), np.asarray(Wv),
                  np.asarray(Wp), np.asarray(bp), B, T, D, KERNEL_CFG, trace=False)
    return out

